# revision 44
# baseline (speedup 1.0000x reference)
import sys

for _p in ("/opt/trn_rl_repo", "/root/.axon_site/_ro/trn_rl_repo"):
    if _p not in sys.path:
        sys.path.insert(0, _p)

import numpy as np

from concourse import bacc, mybir, tile
import bass_rust

N_CORES = 8
N = 2048
D = 1024
HEADS = 16
DH = 64
H_LOC = 4          # heads per core
INNER_LOC = H_LOC * DH  # 256
QKV = INNER_LOC + 2 * DH  # 384 packed projection width
NEG = -1.0e30
EPS = 1e-5
F32 = mybir.dt.float32
F32R = mybir.dt.float32r
BF16 = mybir.dt.bfloat16
I32 = mybir.dt.int32

# rel-pos MLP sharding: 2048 useful reversed-position rows, 256 per core.
MLP_ROWS = 256
HFR_PAD = 64       # data lives at HFRD[64 : 64+2048]
HFRD_ROWS = 2752   # covers all reads [64, 2494]
MT_W = 2560        # master toeplitz width


def _ap(t, pattern, offset):
    a = t.ap().copy()
    a.ap = bass_rust.VecI64Pair(pattern)
    a.offset = offset
    return a


def _ln_stats(nc, pool, x_sb, width, scratch):
    """Row LayerNorm stats for [128, width] tile -> (mean, var, sd, rstd)."""
    s = pool.tile([128, 1], F32, tag="ln_s")
    ssq = pool.tile([128, 1], F32, tag="ln_ssq")
    mean = pool.tile([128, 1], F32, tag="ln_mean")
    var = pool.tile([128, 1], F32, tag="ln_var")
    sd = pool.tile([128, 1], F32, tag="ln_sd")
    rstd = pool.tile([128, 1], F32, tag="ln_rstd")
    nc.vector.tensor_reduce(out=s[:, :], in_=x_sb, axis=mybir.AxisListType.X,
                            op=mybir.AluOpType.add)
    nc.scalar.activation(out=scratch, in_=x_sb,
                         func=mybir.ActivationFunctionType.Square,
                         accum_out=ssq[:, :])
    m2 = pool.tile([128, 1], F32, tag="ln_m2")
    nc.vector.tensor_scalar_mul(out=mean[:, :], in0=s[:, :], scalar1=1.0 / width)
    nc.vector.tensor_tensor(out=m2[:, :], in0=mean[:, :], in1=mean[:, :],
                            op=mybir.AluOpType.mult)
    nc.vector.tensor_scalar(out=var[:, :], in0=ssq[:, :], scalar1=1.0 / width,
                            scalar2=None, op0=mybir.AluOpType.mult)
    nc.vector.tensor_tensor(out=var[:, :], in0=var[:, :], in1=m2[:, :],
                            op=mybir.AluOpType.subtract)
    return mean, var, sd, rstd


def build():
    build.NO_AV = globals().get('NO_AV', False)
    nc = bacc.Bacc("TRN2", target_bir_lowering=False, debug=False,
                   num_devices=N_CORES)

    # ---------------- parameters ----------------
    x_p = nc.declare_dram_parameter("x", [N, D], BF16, isOutput=False)
    wqkv_p = nc.declare_dram_parameter("wqkv", [D, QKV], BF16, isOutput=False)
    wsum_p = nc.declare_dram_parameter("wsum", [QKV], F32, isOutput=False)
    # qs8ks and null-k are passed partition-duplicated ([x | x] over 128
    # partitions) so odd heads can slice base-partition-64 operands.
    qs8ks_p = nc.declare_dram_parameter("qs8ks", [128], F32, isOutput=False)
    nkv_p = nc.declare_dram_parameter("nkv", [2, 128], F32, isOutput=False)
    nb_p = nc.declare_dram_parameter("nb", [H_LOC], F32, isOutput=False)
    w0_p = nc.declare_dram_parameter("w0v", [D], F32, isOutput=False)
    b0_p = nc.declare_dram_parameter("b0", [D], F32, isOutput=False)
    g0_p = nc.declare_dram_parameter("g0", [D], F32, isOutput=False)
    w1_p = nc.declare_dram_parameter("w1", [D, D], BF16, isOutput=False)
    b1_p = nc.declare_dram_parameter("b1", [D], F32, isOutput=False)
    g1_p = nc.declare_dram_parameter("g1", [D], F32, isOutput=False)
    w2_p = nc.declare_dram_parameter("w2", [D, HEADS], BF16, isOutput=False)
    b2_p = nc.declare_dram_parameter("b2", [HEADS], F32, isOutput=False)
    wout_p = nc.declare_dram_parameter("wout", [INNER_LOC, D], BF16,
                                       isOutput=False)
    gout_p = nc.declare_dram_parameter("g_out", [D], F32, isOutput=False)
    posb_p = nc.declare_dram_parameter("posb", [128], I32, isOutput=False)
    sel_p = nc.declare_dram_parameter("sel", [HEADS, H_LOC], F32, isOutput=False)
    out_p = nc.declare_dram_parameter("out", [N // 4, D], F32, isOutput=True)
    DBG = globals().get('DEBUG_TAPS', False)
    if DBG:
        dbg_qT = nc.declare_dram_parameter("dbg_qT", [128, N], BF16, isOutput=True)
        dbg_kT = nc.declare_dram_parameter("dbg_kT", [128, N], BF16, isOutput=True)
        dbg_v0 = nc.declare_dram_parameter("dbg_v0", [128, DH + 1], BF16, isOutput=True)
        dbg_avT = nc.declare_dram_parameter("dbg_avT", [DH, N], BF16, isOutput=True)
        dbg_po = nc.declare_dram_parameter("dbg_po", [128, D], BF16, isOutput=True)
        dbg_mt = nc.declare_dram_parameter("dbg_mt", [128, MT_W], BF16, isOutput=True)
        dbg_avps = nc.declare_dram_parameter("dbg_avps", [DH + 1, N], F32, isOutput=True)
        dbg_rb = nc.declare_dram_parameter("dbg_rb", [DH, N], BF16, isOutput=True)
        dbg_p4 = nc.declare_dram_parameter("dbg_p4", [128, 1024], BF16, isOutput=True)
        dbg_ps4 = nc.declare_dram_parameter("dbg_ps4", [128, 1024], F32, isOutput=True)

    # ---------------- internal DRAM ----------------
    hfr_loc = nc.dram_tensor("hfr_loc", [MLP_ROWS, HEADS], F32)
    hfr_g = nc.dram_tensor("hfr_g", [8 * MLP_ROWS, HEADS], F32)
    hfrd = nc.dram_tensor("hfrd", [H_LOC, HFRD_ROWS], BF16)
    po_q = [nc.dram_tensor(f"po_{i}", [N // 4, D], BF16) for i in range(4)]
    rs_q = [nc.dram_tensor(f"rs_{i}", [N // 16, D], BF16) for i in range(4)]

    dma = nc.sync.dma_start

    with tile.TileContext(nc) as tc:
        with (
            tc.tile_pool(name="const", bufs=1) as constp,
            tc.tile_pool(name="pers", bufs=1) as pers,
            tc.tile_pool(name="weights", bufs=1) as wp,
        ):
            ident = constp.tile([128, 128], F32)
            from concourse.masks import make_identity
            make_identity(nc, ident[:, :])
            identb = constp.tile([128, 128], BF16)
            nc.scalar.copy(out=identb[:, :], in_=ident[:, :])
            eps128 = constp.tile([128, 1], F32)
            nc.vector.memset(eps128[:, :], EPS)
            ones1 = constp.tile([1, 128], F32)
            nc.vector.memset(ones1[:, :], 1.0)

            # ---- all big input loads issued up front (single SP queue) ----
            early = tc.tile_pool(name="early", bufs=1)
            ep = early.__enter__()
            xall = ep.tile([128, 16 * D], BF16, name="xall")
            dma(out=xall[:, :], in_=_ap(x_p, [[D, 128], [128 * D, 16], [1, D]], 0))
            wqkv_sb = ep.tile([128, 8 * QKV], BF16)
            dma(out=wqkv_sb[:, :],
                in_=_ap(wqkv_p, [[QKV, 128], [128 * QKV, 8], [1, QKV]], 0))
            wout_sb = wp.tile([DH, 4 * D], BF16)
            dma(out=wout_sb[:, :],
                in_=_ap(wout_p, [[D, DH], [DH * D, 4], [1, D]], 0))
            w1_sb = ep.tile([128, 8 * D], BF16)  # chunk k at cols k*1024
            dma(out=w1_sb[:, :],
                in_=_ap(w1_p, [[D, 128], [128 * D, 8], [1, D]], 0))
            w2_sb = ep.tile([128, 8 * HEADS], BF16)
            dma(out=w2_sb[:, :],
                in_=_ap(w2_p, [[HEADS, 128], [128 * HEADS, 8], [1, HEADS]], 0))

            vecs = ep.tile([1, 4 * D + HEADS], F32)
            dma(out=vecs[:, 0:D], in_=_ap(w0_p, [[D, 1], [1, D]], 0))
            dma(out=vecs[:, D:2 * D], in_=_ap(b0_p, [[D, 1], [1, D]], 0))
            dma(out=vecs[:, 2 * D:3 * D], in_=_ap(g0_p, [[D, 1], [1, D]], 0))
            dma(out=vecs[:, 3 * D:4 * D], in_=_ap(b1_p, [[D, 1], [1, D]], 0))
            dma(out=vecs[:, 4 * D:4 * D + HEADS],
                in_=_ap(b2_p, [[HEADS, 1], [1, HEADS]], 0))
            g1v = ep.tile([1, D], F32)
            dma(out=g1v[:, :], in_=_ap(g1_p, [[D, 1], [1, D]], 0))
            goutv = wp.tile([1, D], F32)
            dma(out=goutv[:, :], in_=_ap(gout_p, [[D, 1], [1, D]], 0))
            wsum_f32 = ep.tile([1, QKV], F32)
            dma(out=wsum_f32[:, :], in_=_ap(wsum_p, [[QKV, 1], [1, QKV]], 0))
            wsum_row = ep.tile([1, QKV], BF16)
            nc.vector.tensor_copy(out=wsum_row[:, :], in_=wsum_f32[:, :])
            qs8ks_sb = pers.tile([128, 1], F32)
            dma(out=qs8ks_sb[:, :], in_=_ap(qs8ks_p, [[1, 128], [1, 1]], 0))
            nkT = pers.tile([128, 1], F32)
            dma(out=nkT[:, :], in_=_ap(nkv_p, [[1, 128], [1, 1]], 0))
            nv_sb = pers.tile([1, DH], F32)
            dma(out=nv_sb[:, :], in_=nkv_p.ap()[1:2, 0:DH])
            nb_sb = pers.tile([1, H_LOC], F32)
            dma(out=nb_sb[:, :], in_=_ap(nb_p, [[H_LOC, 1], [1, H_LOC]], 0))
            posi_t = pers.tile([128, 1], I32, name="posi")
            dma(out=posi_t[:, :], in_=_ap(posb_p, [[1, 128], [1, 1]], 0))
            sel_sb = wp.tile([HEADS, H_LOC], F32)
            dma(out=sel_sb[:, :], in_=sel_p.ap())

            # [nv | 1] bf16 row for null-key AV
            nv1 = pers.tile([1, DH + 1], BF16)
            nc.vector.tensor_copy(out=nv1[:, 0:DH], in_=nv_sb[:, :])
            nc.vector.memset(nv1[:, DH:DH + 1], 1.0)

            # ---------- Phase 0: rel-pos MLP (256 reversed rows) ----------
            with (
                tc.tile_pool(name="mlp", bufs=2) as mp,
                tc.tile_pool(name="mlp_ps", bufs=2, space="PSUM") as mpp,
                tc.tile_pool(name="mlp_ps2", bufs=2, space="PSUM") as mpp2,
            ):
                bcast = ep.tile([128, 4 * D + HEADS], F32)
                for off in range(0, 4 * D + HEADS, 512):
                    wdt = min(512, 4 * D + HEADS - off)
                    ps = mpp.tile([128, 512], F32, tag="bc")
                    nc.tensor.matmul(out=ps[:, 0:wdt],
                                     lhsT=ones1[:, :],
                                     rhs=vecs[:, off:off + wdt],
                                     start=True, stop=True)
                    nc.scalar.copy(out=bcast[:, off:off + wdt], in_=ps[:, 0:wdt])
                g1b = ep.tile([128, D], F32)
                goutb = wp.tile([128, D], F32)
                for off in range(0, D, 512):
                    ps = mpp.tile([128, 512], F32, tag="bc")
                    nc.tensor.matmul(out=ps[:, :], lhsT=ones1[:, :],
                                     rhs=g1v[:, off:off + 512],
                                     start=True, stop=True)
                    nc.scalar.copy(out=g1b[:, off:off + 512], in_=ps[:, :])
                    ps2 = mpp.tile([128, 512], F32, tag="bc")
                    nc.tensor.matmul(out=ps2[:, :], lhsT=ones1[:, :],
                                     rhs=goutv[:, off:off + 512],
                                     start=True, stop=True)
                    nc.scalar.copy(out=goutb[:, off:off + 512], in_=ps2[:, :])
                w0b = bcast[:, 0:D]
                b0b = bcast[:, D:2 * D]
                g0b = bcast[:, 2 * D:3 * D]
                b1b = bcast[:, 3 * D:4 * D]
                b2b = bcast[:, 4 * D:4 * D + HEADS]

                for t in range(2):
                    m0 = t * 128
                    posf = mp.tile([128, 1], F32, tag="posf")
                    nc.vector.tensor_scalar_add(out=posf[:, :], in0=posi_t[:, :],
                                                scalar1=float(-m0))
                    h0 = mp.tile([128, D], F32, tag="h0")
                    scratch = mp.tile([128, D], F32, tag="scr")
                    nc.vector.tensor_scalar(out=h0[:, :], in0=w0b,
                                            scalar1=posf[:, :], scalar2=None,
                                            op0=mybir.AluOpType.mult)
                    nc.vector.tensor_tensor(out=h0[:, :], in0=h0[:, :], in1=b0b,
                                            op=mybir.AluOpType.add)
                    mean, var, sd, rstd = _ln_stats(nc, mp, h0[:, :], D,
                                                    scratch[:, :])
                    nc.scalar.activation(out=sd[:, :], in_=var[:, :],
                                         func=mybir.ActivationFunctionType.Sqrt,
                                         bias=eps128[:, :])
                    nc.vector.reciprocal_approx_fast(out=rstd[:, :], in_=sd[:, :])
                    nc.vector.tensor_scalar(out=h0[:, :], in0=h0[:, :],
                                            scalar1=mean[:, :], scalar2=rstd[:, :],
                                            op0=mybir.AluOpType.subtract,
                                            op1=mybir.AluOpType.mult)
                    nc.vector.tensor_tensor(out=h0[:, :], in0=h0[:, :], in1=g0b,
                                            op=mybir.AluOpType.mult)
                    h0b = mp.tile([128, D], BF16, tag="h0b")
                    nc.scalar.activation(out=h0b[:, :], in_=h0[:, :],
                                         func=mybir.ActivationFunctionType.Silu)
                    # transpose h0b -> h0T (bf16)
                    h0T = mp.tile([128, D], BF16, tag="h0T")
                    for k in range(8):
                        pst = mpp2.tile([128, 128], BF16, tag="tp")
                        nc.tensor.matmul(out=pst[:, :],
                                         lhsT=h0b[:, k * 128:(k + 1) * 128],
                                         rhs=identb[:, :], is_transpose=True,
                                         start=True, stop=True)
                        nc.vector.tensor_copy(out=h0T[:, k * 128:(k + 1) * 128],
                                              in_=pst[:, :])
                    # h1 = h0 @ w1 + b1
                    h1 = mp.tile([128, D], F32, tag="h1")
                    for eb in range(2):
                        ps = mpp.tile([128, 512], F32, tag="h1ps")
                        for k in range(8):
                            nc.tensor.matmul(
                                out=ps[:, :],
                                lhsT=h0T[:, k * 128:(k + 1) * 128],
                                rhs=w1_sb[:, k * D + eb * 512:
                                          k * D + eb * 512 + 512],
                                start=(k == 0), stop=(k == 7))
                        nc.vector.tensor_tensor(out=h1[:, eb * 512:eb * 512 + 512],
                                                in0=ps[:, :],
                                                in1=b1b[:, eb * 512:eb * 512 + 512],
                                                op=mybir.AluOpType.add)
                    mean, var, sd, rstd = _ln_stats(nc, mp, h1[:, :], D,
                                                    scratch[:, :])
                    nc.scalar.activation(out=sd[:, :], in_=var[:, :],
                                         func=mybir.ActivationFunctionType.Sqrt,
                                         bias=eps128[:, :])
                    nc.vector.reciprocal_approx_fast(out=rstd[:, :], in_=sd[:, :])
                    nc.vector.tensor_scalar(out=h1[:, :], in0=h1[:, :],
                                            scalar1=mean[:, :], scalar2=rstd[:, :],
                                            op0=mybir.AluOpType.subtract,
                                            op1=mybir.AluOpType.mult)
                    nc.vector.tensor_tensor(out=h1[:, :], in0=h1[:, :], in1=g1b,
                                            op=mybir.AluOpType.mult)
                    h1b = mp.tile([128, D], BF16, tag="h1b")
                    nc.scalar.activation(out=h1b[:, :], in_=h1[:, :],
                                         func=mybir.ActivationFunctionType.Silu)
                    h1T = mp.tile([128, D], BF16, tag="h1T")
                    for k in range(8):
                        pst = mpp2.tile([128, 128], BF16, tag="tp")
                        nc.tensor.matmul(out=pst[:, :],
                                         lhsT=h1b[:, k * 128:(k + 1) * 128],
                                         rhs=identb[:, :], is_transpose=True,
                                         start=True, stop=True)
                        nc.vector.tensor_copy(out=h1T[:, k * 128:(k + 1) * 128],
                                              in_=pst[:, :])
                    psf = mpp2.tile([128, HEADS], F32, tag="hf")
                    for k in range(8):
                        nc.tensor.matmul(
                            out=psf[:, :],
                            lhsT=h1T[:, k * 128:(k + 1) * 128],
                            rhs=w2_sb[:, k * HEADS:(k + 1) * HEADS],
                            start=(k == 0), stop=(k == 7))
                    hfc = mp.tile([128, HEADS], F32, tag="hfc")
                    nc.vector.tensor_tensor(out=hfc[:, :], in0=psf[:, :], in1=b2b,
                                            op=mybir.AluOpType.add)
                    dma(out=hfr_loc.ap()[m0:m0 + 128, :], in_=hfc[:, :])

                nc.gpsimd.collective_compute(
                    "AllGather", mybir.AluOpType.bypass,
                    replica_groups=[list(range(N_CORES))],
                    ins=[hfr_loc.ap().opt()],
                    outs=[hfr_g.ap().opt()],
                )

            # ---------- Phase 1: stats + transposes + packed projections ----
            qTp = [pers.tile([128, N], BF16, tag=f"qTp{i}", name=f"qTp{i}")
                   for i in range(2)]
            kT = pers.tile([128, N], BF16, tag="kT", name="kT")
            v1 = [pers.tile([128, DH + 1], BF16, tag=f"v1_{j}", name=f"v1_{j}")
                  for j in range(16)]
            with (
                tc.tile_pool(name="xT", bufs=1) as xTp,
                tc.tile_pool(name="xt", bufs=2) as xtp,
                tc.tile_pool(name="xps", bufs=2, space="PSUM") as xpp,
                tc.tile_pool(name="xps2", bufs=2, space="PSUM") as xpp2,
                tc.tile_pool(name="xps3", bufs=1, space="PSUM") as xpp3,
            ):
                xT = xTp.tile([128, 8 * N], BF16)  # d-chunk k at cols k*2048
                negrow = xTp.tile([1, N], BF16, name="negrow")

                for tt in range(16):
                    xs = xall[:, tt * D:(tt + 1) * D]
                    # stats
                    sums = xtp.tile([128, 1], F32, tag="sums")
                    nc.vector.tensor_reduce(out=sums[:, :], in_=xs,
                                            axis=mybir.AxisListType.X,
                                            op=mybir.AluOpType.add)
                    scr = xtp.tile([128, D], BF16, tag="scr")
                    ssq = xtp.tile([128, 1], F32, tag="ssq")
                    nc.scalar.activation(out=scr[:, :], in_=xs,
                                         func=mybir.ActivationFunctionType.Square,
                                         accum_out=ssq[:, :])
                    mean = xtp.tile([128, 1], F32, tag="mean")
                    m2 = xtp.tile([128, 1], F32, tag="m2")
                    var = xtp.tile([128, 1], F32, tag="var")
                    sd = xtp.tile([128, 1], F32, tag="sd")
                    rstd = xtp.tile([128, 1], F32, tag="rstd")
                    nc.vector.tensor_scalar_mul(out=mean[:, :], in0=sums[:, :],
                                                scalar1=1.0 / D)
                    nc.vector.tensor_tensor(out=m2[:, :], in0=mean[:, :],
                                            in1=mean[:, :],
                                            op=mybir.AluOpType.mult)
                    nc.vector.tensor_scalar(out=var[:, :], in0=ssq[:, :],
                                            scalar1=1.0 / D, scalar2=None,
                                            op0=mybir.AluOpType.mult)
                    nc.vector.tensor_tensor(out=var[:, :], in0=var[:, :],
                                            in1=m2[:, :],
                                            op=mybir.AluOpType.subtract)
                    nc.scalar.activation(out=sd[:, :], in_=var[:, :],
                                         func=mybir.ActivationFunctionType.Sqrt,
                                         bias=eps128[:, :])
                    nc.vector.reciprocal_approx_fast(out=rstd[:, :], in_=sd[:, :])
                    # negsum row: transpose sums -> [1, 128] (wsum carries -1/D)
                    psr = xpp3.tile([128, 128], F32, tag="misc",
                                    name=f"psr{tt}")
                    nc.tensor.matmul(out=psr[0:1, :], lhsT=sums[:, :],
                                     rhs=ident[:, :], is_transpose=True,
                                     start=True, stop=True)
                    nc.vector.tensor_copy(out=negrow[:, tt * 128:tt * 128 + 128],
                                          in_=psr[0:1, :])
                    # x transposes (bf16)
                    for k in range(8):
                        pst = xpp2.tile([128, 128], BF16, tag="tp")
                        nc.tensor.matmul(out=pst[:, :],
                                         lhsT=xall[:, tt * D + k * 128:
                                                   tt * D + k * 128 + 128],
                                         rhs=identb[:, :], is_transpose=True,
                                         start=True, stop=True)
                        eng = nc.vector if k % 2 == 0 else nc.scalar
                        if k % 2 == 0:
                            nc.vector.tensor_copy(
                                out=xT[:, k * N + tt * 128:k * N + tt * 128 + 128],
                                in_=pst[:, :])
                        else:
                            nc.scalar.copy(
                                out=xT[:, k * N + tt * 128:k * N + tt * 128 + 128],
                                in_=pst[:, :])
                    # packed q|k|v projection with rank-1 mean correction
                    psq = xpp.tile([128, QKV], F32, tag="qkv")
                    for k in range(8):
                        nc.tensor.matmul(
                            out=psq[:, :],
                            lhsT=xT[:, k * N + tt * 128:k * N + tt * 128 + 128],
                            rhs=wqkv_sb[:, k * QKV:(k + 1) * QKV],
                            start=(k == 0), stop=False,
                            skip_group_check=True)
                    nc.tensor.matmul(out=psq[:, :],
                                     lhsT=negrow[:, tt * 128:tt * 128 + 128],
                                     rhs=wsum_row[:, :],
                                     start=False, stop=True,
                                     skip_group_check=True)
                    # per-head l2 norms (4 q heads + k) via fused mult-reduce
                    nrm = xtp.tile([128, 8], F32, tag="nrm")
                    scr2 = xtp.tile([128, DH], BF16, tag="scr2")
                    for j in range(5):
                        nc.scalar.activation(
                            out=scr2[:, :],
                            in_=psq[:, j * DH:(j + 1) * DH],
                            func=mybir.ActivationFunctionType.Square,
                            accum_out=nrm[:, j:j + 1])
                    sd5 = xtp.tile([128, 8], F32, tag="sd5")
                    rinv = xtp.tile([128, 8], F32, tag="rinv")
                    nc.scalar.activation(out=sd5[:, 0:5], in_=nrm[:, 0:5],
                                         func=mybir.ActivationFunctionType.Sqrt)
                    nc.vector.reciprocal(out=rinv[:, 0:5], in_=sd5[:, 0:5])
                    # scaled copies out of PSUM
                    qn = xtp.tile([128, INNER_LOC], BF16, tag="qn")
                    for h in range(4):
                        nc.vector.tensor_scalar(
                            out=qn[:, h * DH:(h + 1) * DH],
                            in0=psq[:, h * DH:(h + 1) * DH],
                            scalar1=rinv[:, h:h + 1], scalar2=None,
                            op0=mybir.AluOpType.mult)
                    # kn duplicated into both column halves so the transpose
                    # yields kT stacked twice along partitions
                    kn = xtp.tile([128, 128], BF16, tag="kn")
                    for kh in range(2):
                        nc.vector.tensor_scalar(
                            out=kn[:, kh * DH:(kh + 1) * DH],
                            in0=psq[:, INNER_LOC:INNER_LOC + DH],
                            scalar1=rinv[:, 4:5], scalar2=None,
                            op0=mybir.AluOpType.mult)
                    nc.vector.tensor_scalar(out=v1[tt][:, 0:DH],
                                            in0=psq[:, INNER_LOC + DH:QKV],
                                            scalar1=rstd[:, :], scalar2=None,
                                            op0=mybir.AluOpType.mult)
                    nc.vector.memset(v1[tt][:, DH:DH + 1], 1.0)
                    # q transposes -> head-pair tiles
                    for p in range(2):
                        pstq = xpp2.tile([128, 128], BF16, tag="tp")
                        nc.tensor.matmul(out=pstq[:, :],
                                         lhsT=qn[:, p * 128:(p + 1) * 128],
                                         rhs=identb[:, :], is_transpose=True,
                                         start=True, stop=True)
                        nc.scalar.copy(
                            out=qTp[p][:, tt * 128:tt * 128 + 128],
                            in_=pstq[:, :])
                    # k transpose with qs8ks scale folded in
                    pstk = xpp2.tile([128, 128], BF16, tag="tpk")
                    nc.tensor.matmul(out=pstk[:, :], lhsT=kn[:, :],
                                     rhs=identb[:, :], is_transpose=True,
                                     start=True, stop=True)
                    nc.vector.tensor_scalar(out=kT[:, tt * 128:tt * 128 + 128],
                                            in0=pstk[:, :],
                                            scalar1=qs8ks_sb[:, :], scalar2=None,
                                            op0=mybir.AluOpType.mult)

                if DBG:
                    dma(out=dbg_qT.ap(), in_=qTp[0][:, :])
                    dma(out=dbg_kT.ap(), in_=kT[:, :])
                    dma(out=dbg_v0.ap(), in_=v1[0][:, :])

                # null key normalize: nkn = l2norm(nk) * qs8ks  (dup over 128)
                ones64c_f = constp.tile([DH, 1], F32)
                nc.vector.memset(ones64c_f[:, :], 1.0)
                nsq = xtp.tile([128, 1], F32, tag="nsq")
                nc.scalar.activation(out=nsq[:, :], in_=nkT[:, :],
                                     func=mybir.ActivationFunctionType.Square)
                psn1 = xpp3.tile([128, 128], F32, tag="misc", name="psn1")
                nc.tensor.matmul(out=psn1[0:1, 0:1], lhsT=ones64c_f[:, :],
                                 rhs=nsq[0:DH, :], start=True, stop=True)
                rn1 = xtp.tile([1, 1], F32, tag="rn1")
                nc.scalar.activation(out=rn1[:, :], in_=psn1[0:1, 0:1],
                                     func=mybir.ActivationFunctionType.Sqrt)
                with nc.allow_low_precision(reason="f32r same bits as f32"):
                    nc.vector.reciprocal(out=rn1[:, :], in_=rn1[:, :])
                psb1 = xpp3.tile([128, 128], F32, tag="misc", name="psb1")
                nc.tensor.matmul(out=psb1[:, 0:1], lhsT=ones1[:, :],
                                 rhs=rn1[:, :], start=True, stop=True)
                nkn = pers.tile([128, 1], BF16)
                nc.vector.tensor_tensor(out=nkn[:, :], in0=nkT[:, :],
                                        in1=psb1[:, 0:1],
                                        op=mybir.AluOpType.mult)
                nc.vector.tensor_scalar(out=nkn[:, :], in0=nkn[:, :],
                                        scalar1=qs8ks_sb[:, :], scalar2=None,
                                        op0=mybir.AluOpType.mult)

            early.__exit__(None, None, None)

            # ---------- Phase 2: stage AllGathered MLP rows -> hfrd ----------
            with (
                tc.tile_pool(name="stg", bufs=2) as sgp,
                tc.tile_pool(name="stg_ps", bufs=2, space="PSUM") as sgpp,
            ):
                for chunk in range(16):
                    stg = sgp.tile([128, HEADS], F32, tag="stg")
                    dma(out=stg[:, :],
                        in_=hfr_g.ap()[chunk * 128:(chunk + 1) * 128, :])
                    pstT = sgpp.tile([HEADS, 128], F32, tag="tp")
                    nc.tensor.matmul(out=pstT[:, :], lhsT=stg[:, :],
                                     rhs=ident[:, :], is_transpose=True,
                                     start=True, stop=True)
                    stgT = sgp.tile([HEADS, 128], F32, tag="stgTs")
                    nc.scalar.copy(out=stgT[:, :], in_=pstT[:, :])
                    psl = sgpp.tile([H_LOC, 128], F32, tag="hf")
                    nc.tensor.matmul(out=psl[:, :], lhsT=sel_sb[:, :],
                                     rhs=stgT[:, :], start=True, stop=True)
                    stl = sgp.tile([H_LOC, 128], BF16, tag="stl")
                    nc.scalar.copy(out=stl[:, :], in_=psl[:, :])
                    dma(out=_ap(hfrd, [[HFRD_ROWS, H_LOC], [1, 128]],
                                HFR_PAD + chunk * 128),
                        in_=stl[:, :])
                poison = sgp.tile([H_LOC, HFRD_ROWS - 2112], BF16, name="poison")
                nc.vector.memset(poison[:, :], NEG)
                dma(out=_ap(hfrd, [[HFRD_ROWS, H_LOC],
                                   [1, HFRD_ROWS - 2112]], 2112),
                    in_=poison[:, :])

            # ---------- Phase 3: attention + per-quarter out-proj + RS ------
            avT = [pers.tile([DH, N], BF16, tag=f"avT{h}", name=f"avT{h}")
                   for h in range(H_LOC)]
            mt = [pers.tile([128, MT_W], BF16, tag=f"mt{h}", name=f"mt{h}")
                  for h in range(H_LOC)]
            for h in range(H_LOC):
                dma(out=mt[h][:, :],
                    in_=_ap(hfrd, [[1, 128], [1, MT_W]], h * HFRD_ROWS + 63))
            if build.NO_AV:
                for h in range(H_LOC):
                    nc.vector.memset(avT[h][:, :], 0.0)

            with (
                tc.tile_pool(name="at", bufs=3) as atp,
                tc.tile_pool(name="sim4", bufs=2, space="PSUM") as simpp,
                tc.tile_pool(name="avps", bufs=2, space="PSUM") as avpp,
                tc.tile_pool(name="tps", bufs=1, space="PSUM") as tpp,
                tc.tile_pool(name="oq", bufs=1) as oqp,
            ):
                ones65 = atp.tile([DH + 1, DH], F32, tag="ones65",
                                  name="ones65")
                nc.vector.memset(ones65[:, :], 1.0)
                for q in range(4):
                    for m in (2 * q, 2 * q + 1):
                        i0 = m * 256
                        njt = 2 * m + 2
                        for h in range(H_LOC):
                            hp = (h % 2) * DH
                            qh = qTp[h // 2][hp:hp + DH, i0:i0 + 256]
                            av_ps = None if build.NO_AV else avpp.tile(
                                [DH + 1, 256], F32, tag="av",
                                name=f"av_{m}_{h}")
                            GSZ = 4
                            groups = [list(range(g, min(g + GSZ, njt)))
                                      for g in range(0, njt, GSZ)]
                            if len(groups[-1]) == GSZ:
                                # keep a spare exp column chunk for the
                                # null-key logits in the final group
                                groups[-1] = groups[-1][:GSZ - 1]
                                groups.append([njt - 1])
                            for gi, jts in enumerate(groups):
                                gw = 256 * len(jts)
                                last = (gi == len(groups) - 1)
                                ps4 = simpp.tile([128, 1024], F32, tag="sim")
                                for ji, jt in enumerate(jts):
                                    j0 = jt * 128
                                    c0 = ji * 256
                                    nc.tensor.matmul(
                                        out=ps4[:, c0:c0 + 256],
                                        lhsT=kT[hp:hp + DH, j0:j0 + 128],
                                        rhs=qh,
                                        start=True, stop=False,
                                        skip_group_check=True)
                                    # Toeplitz bias add via identity matmul
                                    u0 = 2048 - i0 + jt * 128
                                    mtr = mt[h][:, :].copy()
                                    pat = [list(p) for p in mtr.ap.to_list()]
                                    pat[1] = [-1, 256]
                                    mtr.ap = bass_rust.VecI64Pair(pat)
                                    mtr.offset = mtr.offset + u0
                                    nc.tensor.matmul(
                                        out=ps4[:, c0:c0 + 256],
                                        lhsT=identb[:, :], rhs=mtr,
                                        start=False, stop=True,
                                        skip_group_check=True)
                                ew = gw
                                if last:
                                    # null-key logits ride along in the spare
                                    # columns of the final (partial) group
                                    nc.tensor.matmul(
                                        out=ps4[0:1, gw:gw + 256],
                                        lhsT=nkn[hp:hp + DH, :], rhs=qh,
                                        start=True, stop=True,
                                        skip_group_check=True)
                                    nc.vector.tensor_scalar_add(
                                        out=ps4[0:1, gw:gw + 256],
                                        in0=ps4[0:1, gw:gw + 256],
                                        scalar1=nb_sb[:, h:h + 1])
                                    ew = gw + 256
                                p4 = atp.tile([128, 1024], BF16, tag="p4")
                                nc.scalar.activation(
                                    out=p4[:, 0:ew], in_=ps4[:, 0:ew],
                                    func=mybir.ActivationFunctionType.Exp)
                                if DBG and m == 1 and h == 0 and gi == 0:
                                    dma(out=dbg_p4.ap(), in_=p4[:, :])
                                    dps4 = atp.tile([128, 1024], F32,
                                                    tag="dps4", name="dps4")
                                    nc.vector.tensor_copy(out=dps4[:, :],
                                                          in_=ps4[:, :])
                                    dma(out=dbg_ps4.ap(), in_=dps4[:, :])
                                # AV in transposed layout: out [65, 256] with
                                # the softmax row-sums landing in row 64
                                for ji, jt in (() if build.NO_AV else enumerate(jts)):
                                    nc.tensor.matmul(
                                        out=av_ps[:, :],
                                        lhsT=v1[jt][:, :],
                                        rhs=p4[:, ji * 256:ji * 256 + 256],
                                        start=(gi == 0 and ji == 0),
                                        stop=False,
                                        skip_group_check=True)
                                if last and not build.NO_AV:
                                    nc.tensor.matmul(
                                        out=av_ps[:, :],
                                        lhsT=nv1[:, :],
                                        rhs=p4[0:1, gw:gw + 256],
                                        start=False, stop=True,
                                        skip_group_check=True)
                            if build.NO_AV:
                                continue
                            # normalize columns by row-64 sums -> avT[h]
                            # (reciprocal + broadcast stay on partition 64)
                            rr = atp.tile([DH + 1, 256], F32, tag="rr")
                            # full-height recip: base-partition-64 DVE slices
                            # silently no-op, so compute all rows (only row 64
                            # is read by the selector matmul below)
                            nc.vector.reciprocal_approx_fast(
                                out=rr[:, :], in_=av_ps[:, :])
                            psb = tpp.tile([DH, 256], F32, tag="bc")
                            nc.tensor.matmul(out=psb[:, :],
                                             lhsT=ones65[DH:DH + 1, 0:DH],
                                             rhs=rr[DH:DH + 1, :],
                                             start=True, stop=True)
                            rb = atp.tile([DH, 256], BF16, tag="rb")
                            nc.scalar.copy(out=rb[:, :], in_=psb[:, :])
                            if DBG and h == 0:
                                davps = oqp.tile([DH + 1, N], F32, tag="davps",
                                                 name="davps")
                                nc.vector.tensor_copy(
                                    out=davps[:, i0:i0 + 256],
                                    in_=av_ps[:, :])
                                drb = oqp.tile([DH, N], BF16, tag="drb",
                                               name="drb")
                                nc.vector.tensor_copy(out=drb[:, i0:i0 + 256],
                                                      in_=rb[:, :])
                                if m == 7:
                                    dma(out=dbg_avps.ap(), in_=davps[:, :])
                                    dma(out=dbg_rb.ap(), in_=drb[:, :])
                            nc.vector.tensor_tensor(
                                out=avT[h][:, i0:i0 + 256],
                                in0=av_ps[0:DH, :], in1=rb[:, :],
                                op=mybir.AluOpType.mult)

                    # out projection for this quarter
                    for tl in range(4):
                        tt = q * 4 + tl
                        ps_po = simpp.tile([128, 1024], F32, tag="sim")
                        for eb in range(2):
                            for ch in range(H_LOC):
                                nc.tensor.matmul(
                                    out=ps_po[:, eb * 512:eb * 512 + 512],
                                    lhsT=avT[ch][:, tt * 128:tt * 128 + 128],
                                    rhs=wout_sb[:, ch * D + eb * 512:
                                                ch * D + eb * 512 + 512],
                                    start=(ch == 0), stop=(ch == H_LOC - 1),
                                    skip_group_check=True)
                        po_sb = oqp.tile([128, D], BF16, tag="po")
                        nc.vector.tensor_copy(out=po_sb[:, :], in_=ps_po[:, :])
                        dma(out=po_q[q].ap()[tl * 128:(tl + 1) * 128, :],
                            in_=po_sb[:, :])
                    nc.gpsimd.collective_compute(
                        "ReduceScatter", mybir.AluOpType.add,
                        replica_groups=[[0, 1, 2, 3], [4, 5, 6, 7]],
                        ins=[po_q[q].ap().opt()],
                        outs=[rs_q[q].ap().opt()],
                    )
                    # final LN for the 128 owned rows of this quarter
                    y = oqp.tile([128, D], BF16, tag="y")
                    dma(out=y[:, :], in_=rs_q[q].ap()[:, :])
                    yscr = oqp.tile([128, D], BF16, tag="yscr")
                    mean, var, sd, rstd = _ln_stats(nc, oqp, y[:, :], D,
                                                    yscr[:, :])
                    nc.scalar.activation(out=sd[:, :], in_=var[:, :],
                                         func=mybir.ActivationFunctionType.Sqrt,
                                         bias=eps128[:, :])
                    nc.vector.reciprocal_approx_fast(out=rstd[:, :], in_=sd[:, :])
                    yf = oqp.tile([128, D], F32, tag="yf")
                    nc.vector.tensor_scalar(out=yf[:, :], in0=y[:, :],
                                            scalar1=mean[:, :],
                                            scalar2=rstd[:, :],
                                            op0=mybir.AluOpType.subtract,
                                            op1=mybir.AluOpType.mult)
                    nc.vector.tensor_tensor(out=yf[:, :], in0=yf[:, :],
                                            in1=goutb,
                                            op=mybir.AluOpType.mult)
                    dma(out=out_p.ap()[q * 128:(q + 1) * 128, :], in_=yf[:, :])
                if DBG:
                    dma(out=dbg_avT.ap(), in_=avT[0][:, :])
                    dma(out=dbg_mt.ap(), in_=mt[0][:, :])
                    dma(out=dbg_po.ap(), in_=po_q[0].ap()[0:128, :])

    nc.compile()
    return nc


def make_in_maps(x, mask, g_norm, Wq, Wkv, q_scale, k_scale, null_kv,
                 null_attn_bias, w0, b0, g0, w1, b1, g1, w2, b2, Wout, g_out):
    import ml_dtypes
    assert bool(np.asarray(mask).all()), "kernel assumes all-True mask"
    f = np.float32
    bf = ml_dtypes.bfloat16
    gn = np.asarray(g_norm, f)
    in_maps = []
    for c in range(N_CORES):
        bi, hg = c // 4, c % 4
        posb = (2047 - c * MLP_ROWS - np.arange(128)).astype(np.int32)
        wq_hg = np.asarray(Wq, f)[:, hg * INNER_LOC:(hg + 1) * INNER_LOC]
        wqkv = np.concatenate(
            [wq_hg, np.asarray(Wkv, f)], axis=1) * gn[:, None]
        wsum = -(wqkv.sum(axis=0)) / float(D)
        m = {
            "x": np.ascontiguousarray(np.asarray(x, f)[bi]).astype(bf),
            "wqkv": np.ascontiguousarray(wqkv).astype(bf),
            "wsum": np.ascontiguousarray(wsum.astype(f)),
            "qs8ks": np.ascontiguousarray(np.tile(
                8.0 * np.asarray(q_scale, f) * np.asarray(k_scale, f), 2)),
            "nkv": np.ascontiguousarray(np.tile(np.asarray(null_kv, f),
                                                (1, 2))),
            "nb": np.ascontiguousarray(
                np.asarray(null_attn_bias, f)[hg * H_LOC:(hg + 1) * H_LOC]),
            "w0v": np.ascontiguousarray(np.asarray(w0, f).reshape(D)),
            "b0": np.ascontiguousarray(np.asarray(b0, f)),
            "g0": np.ascontiguousarray(np.asarray(g0, f)),
            "w1": np.ascontiguousarray(np.asarray(w1, f)).astype(bf),
            "b1": np.ascontiguousarray(np.asarray(b1, f)),
            "g1": np.ascontiguousarray(np.asarray(g1, f)),
            "w2": np.ascontiguousarray(np.asarray(w2, f)).astype(bf),
            "b2": np.ascontiguousarray(np.asarray(b2, f)),
            "wout": np.ascontiguousarray(
                np.asarray(Wout, f)[hg * INNER_LOC:(hg + 1) * INNER_LOC, :]
            ).astype(bf),
            "g_out": np.ascontiguousarray(np.asarray(g_out, f)),
            "posb": posb,
            "sel": np.eye(HEADS, dtype=f)[:, hg * H_LOC:(hg + 1) * H_LOC].copy(),
        }
        in_maps.append(m)
    return in_maps


_NC_CACHE = None


def kernel(**inputs):
    global _NC_CACHE
    from concourse.bass_utils import run_bass_kernel_spmd

    if _NC_CACHE is None:
        _NC_CACHE = build()
    nc = _NC_CACHE
    in_maps = make_in_maps(**inputs)
    res = run_bass_kernel_spmd(nc, in_maps, core_ids=list(range(N_CORES)))
    outs = res.results
    kernel.last_outs = outs
    full = np.empty((2, N, D), np.float32)
    for c in range(N_CORES):
        bi, hg = c // 4, c % 4
        o = np.asarray(outs[c]["out"], np.float32)
        for q in range(4):
            full[bi, q * 512 + hg * 128:q * 512 + (hg + 1) * 128, :] = \
                o[q * 128:(q + 1) * 128]
    return full


# revision 48
# speedup vs baseline: 1.2600x; 1.2600x over previous
import sys

for _p in ("/opt/trn_rl_repo", "/root/.axon_site/_ro/trn_rl_repo"):
    if _p not in sys.path:
        sys.path.insert(0, _p)

import numpy as np

from concourse import bacc, mybir, tile
import bass_rust

N_CORES = 8
N = 2048
D = 1024
HEADS = 16
DH = 64
H_LOC = 4          # heads per core
INNER_LOC = H_LOC * DH  # 256
QKV = INNER_LOC + 2 * DH  # 384 packed projection width
NEG = -1.0e30
EPS = 1e-5
F32 = mybir.dt.float32
F32R = mybir.dt.float32r
BF16 = mybir.dt.bfloat16
I32 = mybir.dt.int32

# rel-pos MLP sharding: 2048 useful reversed-position rows, 256 per core.
MLP_ROWS = 256
HFR_PAD = 64       # data lives at HFRD[64 : 64+2048]
HFRD_ROWS = 2752   # covers all reads [64, 2494]
MT_W = 2560        # master toeplitz width


def _ap(t, pattern, offset):
    a = t.ap().copy()
    a.ap = bass_rust.VecI64Pair(pattern)
    a.offset = offset
    return a


def _ln_stats(nc, pool, x_sb, width, scratch):
    """Row LayerNorm stats for [128, width] tile -> (mean, var, sd, rstd)."""
    s = pool.tile([128, 1], F32, tag="ln_s")
    ssq = pool.tile([128, 1], F32, tag="ln_ssq")
    mean = pool.tile([128, 1], F32, tag="ln_mean")
    var = pool.tile([128, 1], F32, tag="ln_var")
    sd = pool.tile([128, 1], F32, tag="ln_sd")
    rstd = pool.tile([128, 1], F32, tag="ln_rstd")
    nc.vector.tensor_reduce(out=s[:, :], in_=x_sb, axis=mybir.AxisListType.X,
                            op=mybir.AluOpType.add)
    nc.scalar.activation(out=scratch, in_=x_sb,
                         func=mybir.ActivationFunctionType.Square,
                         accum_out=ssq[:, :])
    m2 = pool.tile([128, 1], F32, tag="ln_m2")
    nc.vector.tensor_scalar_mul(out=mean[:, :], in0=s[:, :], scalar1=1.0 / width)
    nc.vector.tensor_tensor(out=m2[:, :], in0=mean[:, :], in1=mean[:, :],
                            op=mybir.AluOpType.mult)
    nc.vector.tensor_scalar(out=var[:, :], in0=ssq[:, :], scalar1=1.0 / width,
                            scalar2=None, op0=mybir.AluOpType.mult)
    nc.vector.tensor_tensor(out=var[:, :], in0=var[:, :], in1=m2[:, :],
                            op=mybir.AluOpType.subtract)
    return mean, var, sd, rstd


def build():
    build.NO_AV = globals().get('NO_AV', False)
    nc = bacc.Bacc("TRN2", target_bir_lowering=False, debug=False,
                   num_devices=N_CORES)

    # ---------------- parameters ----------------
    x_p = nc.declare_dram_parameter("x", [N, D], BF16, isOutput=False)
    wqkv_p = nc.declare_dram_parameter("wqkv", [D, QKV], BF16, isOutput=False)
    wsum_p = nc.declare_dram_parameter("wsum", [QKV], F32, isOutput=False)
    # qs8ks and null-k are passed partition-duplicated ([x | x] over 128
    # partitions) so odd heads can slice base-partition-64 operands.
    qs8ks_p = nc.declare_dram_parameter("qs8ks", [128], F32, isOutput=False)
    nkv_p = nc.declare_dram_parameter("nkv", [2, 128], F32, isOutput=False)
    nb_p = nc.declare_dram_parameter("nb", [H_LOC], F32, isOutput=False)
    w0_p = nc.declare_dram_parameter("w0v", [D], F32, isOutput=False)
    b0_p = nc.declare_dram_parameter("b0", [D], F32, isOutput=False)
    g0_p = nc.declare_dram_parameter("g0", [D], F32, isOutput=False)
    w1_p = nc.declare_dram_parameter("w1", [D, D], BF16, isOutput=False)
    b1_p = nc.declare_dram_parameter("b1", [D], F32, isOutput=False)
    g1_p = nc.declare_dram_parameter("g1", [D], F32, isOutput=False)
    w2_p = nc.declare_dram_parameter("w2", [D, HEADS], BF16, isOutput=False)
    b2_p = nc.declare_dram_parameter("b2", [HEADS], F32, isOutput=False)
    wout_p = nc.declare_dram_parameter("wout", [INNER_LOC, D], BF16,
                                       isOutput=False)
    gout_p = nc.declare_dram_parameter("g_out", [D], F32, isOutput=False)
    posb_p = nc.declare_dram_parameter("posb", [128], I32, isOutput=False)
    sel_p = nc.declare_dram_parameter("sel", [HEADS, H_LOC], F32, isOutput=False)
    out_p = nc.declare_dram_parameter("out", [N // 4, D], F32, isOutput=True)
    DBG = globals().get('DEBUG_TAPS', False)
    if DBG:
        dbg_qT = nc.declare_dram_parameter("dbg_qT", [128, N], BF16, isOutput=True)
        dbg_kT = nc.declare_dram_parameter("dbg_kT", [128, N], BF16, isOutput=True)
        dbg_v0 = nc.declare_dram_parameter("dbg_v0", [128, DH + 1], BF16, isOutput=True)
        dbg_avT = nc.declare_dram_parameter("dbg_avT", [DH, N], BF16, isOutput=True)
        dbg_po = nc.declare_dram_parameter("dbg_po", [128, D], BF16, isOutput=True)
        dbg_mt = nc.declare_dram_parameter("dbg_mt", [128, MT_W], BF16, isOutput=True)
        dbg_avps = nc.declare_dram_parameter("dbg_avps", [DH + 1, N], F32, isOutput=True)
        dbg_rb = nc.declare_dram_parameter("dbg_rb", [DH, N], BF16, isOutput=True)
        dbg_p4 = nc.declare_dram_parameter("dbg_p4", [128, 1024], BF16, isOutput=True)
        dbg_ps4 = nc.declare_dram_parameter("dbg_ps4", [128, 1024], F32, isOutput=True)

    # ---------------- internal DRAM ----------------
    hfr_loc = nc.dram_tensor("hfr_loc", [MLP_ROWS, HEADS], F32)
    hfr_g = nc.dram_tensor("hfr_g", [8 * MLP_ROWS, HEADS], F32)
    hfrd = nc.dram_tensor("hfrd", [H_LOC, HFRD_ROWS], BF16)
    po_q = [nc.dram_tensor(f"po_{i}", [N // 4, D], BF16) for i in range(4)]
    rs_q = [nc.dram_tensor(f"rs_{i}", [N // 16, D], BF16) for i in range(4)]

    dma = nc.sync.dma_start

    with tile.TileContext(nc) as tc:
        with (
            tc.tile_pool(name="const", bufs=1) as constp,
            tc.tile_pool(name="pers", bufs=1) as pers,
            tc.tile_pool(name="weights", bufs=1) as wp,
        ):
            ident = constp.tile([128, 128], F32)
            from concourse.masks import make_identity
            make_identity(nc, ident[:, :])
            identb = constp.tile([128, 128], BF16)
            nc.scalar.copy(out=identb[:, :], in_=ident[:, :])
            eps128 = constp.tile([128, 1], F32)
            nc.vector.memset(eps128[:, :], EPS)
            ones1 = constp.tile([1, 128], F32)
            nc.vector.memset(ones1[:, :], 1.0)

            # ---- all big input loads issued up front (single SP queue) ----
            early = tc.tile_pool(name="early", bufs=1)
            ep = early.__enter__()
            xall = ep.tile([128, 16 * D], BF16, name="xall")
            dma(out=xall[:, :], in_=_ap(x_p, [[D, 128], [128 * D, 16], [1, D]], 0))
            wqkv_sb = ep.tile([128, 8 * QKV], BF16)
            dma(out=wqkv_sb[:, :],
                in_=_ap(wqkv_p, [[QKV, 128], [128 * QKV, 8], [1, QKV]], 0))
            wout_sb = wp.tile([DH, 4 * D], BF16)
            dma(out=wout_sb[:, :],
                in_=_ap(wout_p, [[D, DH], [DH * D, 4], [1, D]], 0))
            w1_sb = ep.tile([128, 8 * D], BF16)  # chunk k at cols k*1024
            dma(out=w1_sb[:, :],
                in_=_ap(w1_p, [[D, 128], [128 * D, 8], [1, D]], 0))
            w2_sb = ep.tile([128, 8 * HEADS], BF16)
            dma(out=w2_sb[:, :],
                in_=_ap(w2_p, [[HEADS, 128], [128 * HEADS, 8], [1, HEADS]], 0))

            wsum_f32 = ep.tile([1, QKV], F32)
            dma(out=wsum_f32[:, :], in_=_ap(wsum_p, [[QKV, 1], [1, QKV]], 0))
            wsum_row = ep.tile([1, QKV], BF16)
            nc.vector.tensor_copy(out=wsum_row[:, :], in_=wsum_f32[:, :])
            qs8ks_sb = pers.tile([128, 1], F32)
            dma(out=qs8ks_sb[:, :], in_=_ap(qs8ks_p, [[1, 128], [1, 1]], 0))
            nkT = pers.tile([128, 1], F32)
            dma(out=nkT[:, :], in_=_ap(nkv_p, [[1, 128], [1, 1]], 0))
            nv_sb = pers.tile([1, DH], F32)
            dma(out=nv_sb[:, :], in_=nkv_p.ap()[1:2, 0:DH])
            nb_sb = pers.tile([1, H_LOC], F32)
            dma(out=nb_sb[:, :], in_=_ap(nb_p, [[H_LOC, 1], [1, H_LOC]], 0))
            posi_t = pers.tile([128, 1], I32, name="posi")
            dma(out=posi_t[:, :], in_=_ap(posb_p, [[1, 128], [1, 1]], 0))
            sel_sb = wp.tile([HEADS, H_LOC], F32)
            dma(out=sel_sb[:, :], in_=sel_p.ap())

            # [nv | 1] bf16 row for null-key AV
            nv1 = pers.tile([1, DH + 1], BF16)
            nc.vector.tensor_copy(out=nv1[:, 0:DH], in_=nv_sb[:, :])
            nc.vector.memset(nv1[:, DH:DH + 1], 1.0)

            # ---------- Phases 0-2 interleaved: rel-pos MLP generator is ----
            # pumped between projection steps so its long serial chain fills
            # engine gaps instead of blocking the in-order queues.
            qT2 = pers.tile([128, 2 * N], BF16, name="qT2")
            kT = pers.tile([128, N], BF16, tag="kT", name="kT")
            v1 = [pers.tile([128, DH + 1], BF16, tag=f"v1_{j}", name=f"v1_{j}")
                  for j in range(16)]
            nkn = pers.tile([128, 1], BF16)

            with (
                tc.tile_pool(name="bc_ps", bufs=2, space="PSUM") as bpp,
                tc.tile_pool(name="vst", bufs=3) as vstp,
            ):
                bcast = ep.tile([128, 4 * D + HEADS], BF16)
                g1b = ep.tile([128, D], BF16)
                goutb = wp.tile([128, D], F32)
                chunks = []
                for pi, par in enumerate((w0_p, b0_p, g0_p, b1_p)):
                    for half in range(2):
                        chunks.append((par, half * 512, 512, bcast,
                                       pi * D + half * 512))
                chunks.append((b2_p, 0, HEADS, bcast, 4 * D))
                for half in range(2):
                    chunks.append((g1_p, half * 512, 512, g1b, half * 512))
                    chunks.append((gout_p, half * 512, 512, goutb,
                                   half * 512))
                for par, poff, wdt, dst, doff in chunks:
                    vstage = vstp.tile([1, 512], F32, tag="vstage")
                    dma(out=vstage[:, 0:wdt],
                        in_=_ap(par, [[wdt, 1], [1, wdt]], poff))
                    ps = bpp.tile([128, 512], F32, tag="bc")
                    nc.tensor.matmul(out=ps[:, 0:wdt],
                                     lhsT=ones1[:, :],
                                     rhs=vstage[:, 0:wdt],
                                     start=True, stop=True)
                    nc.scalar.copy(out=dst[:, doff:doff + wdt],
                                   in_=ps[:, 0:wdt])
            w0b = bcast[:, 0:D]
            b0b = bcast[:, D:2 * D]
            g0b = bcast[:, 2 * D:3 * D]
            b1b = bcast[:, 3 * D:4 * D]
            b2b = bcast[:, 4 * D:4 * D + HEADS]

            with (
                tc.tile_pool(name="mlp", bufs=1) as mp,
                tc.tile_pool(name="mlp_ps", bufs=1, space="PSUM") as mpp,
                tc.tile_pool(name="mlp_ps2", bufs=1, space="PSUM") as mpp2,
                tc.tile_pool(name="xT", bufs=1) as xTp,
                tc.tile_pool(name="xt", bufs=2) as xtp,
                tc.tile_pool(name="xps", bufs=2, space="PSUM") as xpp,
                tc.tile_pool(name="xps2", bufs=1, space="PSUM") as xpp2,
                tc.tile_pool(name="xps3", bufs=1, space="PSUM") as xpp3,
            ):
                xT = xTp.tile([128, 8 * N], BF16)  # d-chunk k at cols k*2048
                negrow = wp.tile([1, N], BF16, name="negrow")

                def mlp_gen():
                    for t in range(2):
                        m0 = t * 128
                        posf = mp.tile([128, 1], F32, tag="posf",
                                       name=f"posf{t}")
                        nc.vector.tensor_scalar_add(out=posf[:, :],
                                                    in0=posi_t[:, :],
                                                    scalar1=float(-m0))
                        h0 = mp.tile([128, D], F32, tag="h0", name=f"h0_{t}")
                        scratch = mp.tile([128, D], BF16, tag="scr",
                                          name=f"scr{t}")
                        nc.vector.tensor_scalar(out=h0[:, :], in0=w0b,
                                                scalar1=posf[:, :], scalar2=None,
                                                op0=mybir.AluOpType.mult)
                        nc.vector.tensor_tensor(out=h0[:, :], in0=h0[:, :],
                                                in1=b0b,
                                                op=mybir.AluOpType.add)
                        yield
                        mean, var, sd, rstd = _ln_stats(nc, mp, h0[:, :], D,
                                                        scratch[:, :])
                        nc.scalar.activation(
                            out=sd[:, :], in_=var[:, :],
                            func=mybir.ActivationFunctionType.Sqrt,
                            bias=eps128[:, :])
                        nc.vector.reciprocal_approx_fast(out=rstd[:, :],
                                                         in_=sd[:, :])
                        yield
                        nc.vector.tensor_scalar(out=h0[:, :], in0=h0[:, :],
                                                scalar1=mean[:, :],
                                                scalar2=rstd[:, :],
                                                op0=mybir.AluOpType.subtract,
                                                op1=mybir.AluOpType.mult)
                        nc.vector.tensor_tensor(out=h0[:, :], in0=h0[:, :],
                                                in1=g0b,
                                                op=mybir.AluOpType.mult)
                        h0b = mp.tile([128, D], BF16, tag="h0b",
                                      name=f"h0b{t}")
                        nc.scalar.activation(
                            out=h0b[:, :], in_=h0[:, :],
                            func=mybir.ActivationFunctionType.Silu)
                        yield
                        h0T = mp.tile([128, D], BF16, tag="h0T",
                                      name=f"h0T{t}")
                        pst8m = mpp2.tile([128, D], BF16, tag="tp",
                                          name=f"tp0_{t}")
                        for k in range(8):
                            nc.tensor.matmul(
                                out=pst8m[:, k * 128:(k + 1) * 128],
                                lhsT=h0b[:, k * 128:(k + 1) * 128],
                                rhs=identb[:, :], is_transpose=True,
                                start=True, stop=True)
                            if k == 3:
                                yield
                        nc.vector.tensor_copy(out=h0T[:, :], in_=pst8m[:, :])
                        yield
                        h1 = mp.tile([128, D], F32, tag="h1", name=f"h1_{t}")
                        for eb in range(2):
                            ps = mpp.tile([128, 512], F32, tag="h1ps",
                                          name=f"h1ps{t}_{eb}")
                            for k in range(8):
                                nc.tensor.matmul(
                                    out=ps[:, :],
                                    lhsT=h0T[:, k * 128:(k + 1) * 128],
                                    rhs=w1_sb[:, k * D + eb * 512:
                                              k * D + eb * 512 + 512],
                                    start=(k == 0), stop=(k == 7))
                            nc.vector.tensor_tensor(
                                out=h1[:, eb * 512:eb * 512 + 512],
                                in0=ps[:, :],
                                in1=b1b[:, eb * 512:eb * 512 + 512],
                                op=mybir.AluOpType.add)
                            yield
                        mean, var, sd, rstd = _ln_stats(nc, mp, h1[:, :], D,
                                                        scratch[:, :])
                        nc.scalar.activation(
                            out=sd[:, :], in_=var[:, :],
                            func=mybir.ActivationFunctionType.Sqrt,
                            bias=eps128[:, :])
                        nc.vector.reciprocal_approx_fast(out=rstd[:, :],
                                                         in_=sd[:, :])
                        yield
                        nc.vector.tensor_scalar(out=h1[:, :], in0=h1[:, :],
                                                scalar1=mean[:, :],
                                                scalar2=rstd[:, :],
                                                op0=mybir.AluOpType.subtract,
                                                op1=mybir.AluOpType.mult)
                        nc.vector.tensor_tensor(out=h1[:, :], in0=h1[:, :],
                                                in1=g1b,
                                                op=mybir.AluOpType.mult)
                        h1b = mp.tile([128, D], BF16, tag="h1b",
                                      name=f"h1b{t}")
                        nc.scalar.activation(
                            out=h1b[:, :], in_=h1[:, :],
                            func=mybir.ActivationFunctionType.Silu)
                        yield
                        h1T = mp.tile([128, D], BF16, tag="h1T",
                                      name=f"h1T{t}")
                        pst8n = mpp2.tile([128, D], BF16, tag="tp",
                                          name=f"tp1_{t}")
                        for k in range(8):
                            nc.tensor.matmul(
                                out=pst8n[:, k * 128:(k + 1) * 128],
                                lhsT=h1b[:, k * 128:(k + 1) * 128],
                                rhs=identb[:, :], is_transpose=True,
                                start=True, stop=True)
                            if k == 3:
                                yield
                        nc.vector.tensor_copy(out=h1T[:, :], in_=pst8n[:, :])
                        yield
                        psf = mpp2.tile([128, HEADS], F32, tag="hf",
                                        name=f"hf{t}")
                        for k in range(8):
                            nc.tensor.matmul(
                                out=psf[:, :],
                                lhsT=h1T[:, k * 128:(k + 1) * 128],
                                rhs=w2_sb[:, k * HEADS:(k + 1) * HEADS],
                                start=(k == 0), stop=(k == 7))
                        hfc = mp.tile([128, HEADS], F32, tag="hfc",
                                      name=f"hfc{t}")
                        nc.vector.tensor_tensor(out=hfc[:, :], in0=psf[:, :],
                                                in1=b2b,
                                                op=mybir.AluOpType.add)
                        dma(out=hfr_loc.ap()[m0:m0 + 128, :], in_=hfc[:, :])
                        yield
                    nc.gpsimd.collective_compute(
                        "AllGather", mybir.AluOpType.bypass,
                        replica_groups=[list(range(N_CORES))],
                        ins=[hfr_loc.ap().opt()],
                        outs=[hfr_g.ap().opt()],
                    )

                mgen = mlp_gen()

                def pump(n=1):
                    for _ in range(n):
                        try:
                            next(mgen)
                        except StopIteration:
                            return

                for tt in range(16):
                    xs = xall[:, tt * D:(tt + 1) * D]
                    # stats
                    sums = xtp.tile([128, 1], F32, tag="sums")
                    nc.vector.tensor_reduce(out=sums[:, :], in_=xs,
                                            axis=mybir.AxisListType.X,
                                            op=mybir.AluOpType.add)
                    scr = xtp.tile([128, D], BF16, tag="scr")
                    ssq = xtp.tile([128, 1], F32, tag="ssq")
                    nc.scalar.activation(out=scr[:, :], in_=xs,
                                         func=mybir.ActivationFunctionType.Square,
                                         accum_out=ssq[:, :])
                    mean = xtp.tile([128, 1], F32, tag="mean")
                    m2 = xtp.tile([128, 1], F32, tag="m2")
                    var = xtp.tile([128, 1], F32, tag="var")
                    sd = xtp.tile([128, 1], F32, tag="sd")
                    rstd = xtp.tile([128, 1], F32, tag="rstd")
                    nc.vector.tensor_scalar_mul(out=mean[:, :], in0=sums[:, :],
                                                scalar1=1.0 / D)
                    nc.vector.tensor_tensor(out=m2[:, :], in0=mean[:, :],
                                            in1=mean[:, :],
                                            op=mybir.AluOpType.mult)
                    nc.vector.tensor_scalar(out=var[:, :], in0=ssq[:, :],
                                            scalar1=1.0 / D, scalar2=None,
                                            op0=mybir.AluOpType.mult)
                    nc.vector.tensor_tensor(out=var[:, :], in0=var[:, :],
                                            in1=m2[:, :],
                                            op=mybir.AluOpType.subtract)
                    nc.scalar.activation(out=sd[:, :], in_=var[:, :],
                                         func=mybir.ActivationFunctionType.Sqrt,
                                         bias=eps128[:, :])
                    nc.vector.reciprocal_approx_fast(out=rstd[:, :],
                                                     in_=sd[:, :])
                    # negsum row: transpose sums -> [1, 128] (wsum carries -1/D)
                    psr = xpp3.tile([128, 128], F32, tag="misc",
                                    name=f"psr{tt}")
                    nc.tensor.matmul(out=psr[0:1, :], lhsT=sums[:, :],
                                     rhs=ident[:, :], is_transpose=True,
                                     start=True, stop=True)
                    nc.vector.tensor_copy(out=negrow[:, tt * 128:tt * 128 + 128],
                                          in_=psr[0:1, :])
                    pump(1)
                    # x transposes (bf16): 8 into one psum tile, one fat copy
                    pst8 = xpp2.tile([128, D], BF16, tag="tp8",
                                     name=f"pst8_{tt}")
                    for k in range(8):
                        nc.tensor.matmul(out=pst8[:, k * 128:(k + 1) * 128],
                                         lhsT=xall[:, tt * D + k * 128:
                                                   tt * D + k * 128 + 128],
                                         rhs=identb[:, :], is_transpose=True,
                                         start=True, stop=True)
                    xTo = xT[:, :].copy()
                    xpat = [list(p) for p in xTo.ap.to_list()]
                    xpat = [xpat[0], [N, 8], [1, 128]]
                    xTo.ap = bass_rust.VecI64Pair(xpat)
                    xTo.offset = xTo.offset + tt * 128
                    nc.vector.tensor_copy(out=xTo, in_=pst8[:, :])
                    pump(1)
                    # packed q|k|v projection with rank-1 mean correction
                    psq = xpp.tile([128, QKV], F32, tag="qkv")
                    for k in range(8):
                        nc.tensor.matmul(
                            out=psq[:, :],
                            lhsT=xT[:, k * N + tt * 128:k * N + tt * 128 + 128],
                            rhs=wqkv_sb[:, k * QKV:(k + 1) * QKV],
                            start=(k == 0), stop=False,
                            skip_group_check=True)
                    nc.tensor.matmul(out=psq[:, :],
                                     lhsT=negrow[:, tt * 128:tt * 128 + 128],
                                     rhs=wsum_row[:, :],
                                     start=False, stop=True,
                                     skip_group_check=True)
                    # per-head l2 norms (4 q heads + k)
                    nrm = xtp.tile([128, 8], F32, tag="nrm")
                    scr2 = xtp.tile([128, DH], BF16, tag="scr2")
                    for j in range(5):
                        nc.scalar.activation(
                            out=scr2[:, :],
                            in_=psq[:, j * DH:(j + 1) * DH],
                            func=mybir.ActivationFunctionType.Square,
                            accum_out=nrm[:, j:j + 1])
                    sd5 = xtp.tile([128, 8], F32, tag="sd5")
                    rinv = xtp.tile([128, 8], F32, tag="rinv")
                    nc.scalar.activation(out=sd5[:, 0:5], in_=nrm[:, 0:5],
                                         func=mybir.ActivationFunctionType.Sqrt)
                    nc.vector.reciprocal(out=rinv[:, 0:5], in_=sd5[:, 0:5])
                    pump(1)
                    # scaled copies out of PSUM
                    qn = xtp.tile([128, INNER_LOC], BF16, tag="qn")
                    for h in range(4):
                        eng = nc.vector if h % 2 == 0 else None
                        if h % 2 == 0:
                            nc.vector.tensor_scalar(
                                out=qn[:, h * DH:(h + 1) * DH],
                                in0=psq[:, h * DH:(h + 1) * DH],
                                scalar1=rinv[:, h:h + 1], scalar2=None,
                                op0=mybir.AluOpType.mult)
                        else:
                            nc.scalar.activation(
                                out=qn[:, h * DH:(h + 1) * DH],
                                in_=psq[:, h * DH:(h + 1) * DH],
                                func=mybir.ActivationFunctionType.Copy,
                                scale=rinv[:, h:h + 1])
                    # kn duplicated into both column halves so the transpose
                    # yields kT stacked twice along partitions
                    kn = xtp.tile([128, 128], BF16, tag="kn")
                    for kh in range(2):
                        nc.vector.tensor_scalar(
                            out=kn[:, kh * DH:(kh + 1) * DH],
                            in0=psq[:, INNER_LOC:INNER_LOC + DH],
                            scalar1=rinv[:, 4:5], scalar2=None,
                            op0=mybir.AluOpType.mult)
                    nc.vector.tensor_scalar(out=v1[tt][:, 0:DH],
                                            in0=psq[:, INNER_LOC + DH:QKV],
                                            scalar1=rstd[:, :], scalar2=None,
                                            op0=mybir.AluOpType.mult)
                    nc.vector.memset(v1[tt][:, DH:DH + 1], 1.0)
                    # q pair + k transposes into one psum tile
                    pstqk = xpp2.tile([128, 384], BF16, tag="tpqk",
                                      name=f"pstqk{tt}")
                    for p in range(2):
                        nc.tensor.matmul(out=pstqk[:, p * 128:(p + 1) * 128],
                                         lhsT=qn[:, p * 128:(p + 1) * 128],
                                         rhs=identb[:, :], is_transpose=True,
                                         start=True, stop=True)
                    nc.tensor.matmul(out=pstqk[:, 256:384], lhsT=kn[:, :],
                                     rhs=identb[:, :], is_transpose=True,
                                     start=True, stop=True)
                    qTo = qT2[:, :].copy()
                    qpat = [list(p) for p in qTo.ap.to_list()]
                    qpat = [qpat[0], [N, 2], [1, 128]]
                    qTo.ap = bass_rust.VecI64Pair(qpat)
                    qTo.offset = qTo.offset + tt * 128
                    nc.scalar.copy(out=qTo, in_=pstqk[:, 0:256])
                    # k transpose with qs8ks scale folded in
                    nc.vector.tensor_scalar(out=kT[:, tt * 128:tt * 128 + 128],
                                            in0=pstqk[:, 256:384],
                                            scalar1=qs8ks_sb[:, :], scalar2=None,
                                            op0=mybir.AluOpType.mult)
                    pump(1)

                pump(100)

                if DBG:
                    dma(out=dbg_qT.ap(), in_=qT2[:, 0:N])
                    dma(out=dbg_kT.ap(), in_=kT[:, :])
                    dma(out=dbg_v0.ap(), in_=v1[0][:, :])

                # null key normalize: nkn = l2norm(nk) * qs8ks  (dup over 128)
                ones64c_f = constp.tile([DH, 1], F32)
                nc.vector.memset(ones64c_f[:, :], 1.0)
                nsq = xtp.tile([128, 1], F32, tag="nsq")
                nc.scalar.activation(out=nsq[:, :], in_=nkT[:, :],
                                     func=mybir.ActivationFunctionType.Square)
                psn1 = xpp3.tile([128, 128], F32, tag="misc", name="psn1")
                nc.tensor.matmul(out=psn1[0:1, 0:1], lhsT=ones64c_f[:, :],
                                 rhs=nsq[0:DH, :], start=True, stop=True)
                rn1 = xtp.tile([1, 1], F32, tag="rn1")
                nc.scalar.activation(out=rn1[:, :], in_=psn1[0:1, 0:1],
                                     func=mybir.ActivationFunctionType.Sqrt)
                with nc.allow_low_precision(reason="f32r same bits as f32"):
                    nc.vector.reciprocal(out=rn1[:, :], in_=rn1[:, :])
                psb1 = xpp3.tile([128, 128], F32, tag="misc", name="psb1")
                nc.tensor.matmul(out=psb1[:, 0:1], lhsT=ones1[:, :],
                                 rhs=rn1[:, :], start=True, stop=True)
                nc.vector.tensor_tensor(out=nkn[:, :], in0=nkT[:, :],
                                        in1=psb1[:, 0:1],
                                        op=mybir.AluOpType.mult)
                nc.vector.tensor_scalar(out=nkn[:, :], in0=nkn[:, :],
                                        scalar1=qs8ks_sb[:, :], scalar2=None,
                                        op0=mybir.AluOpType.mult)

                # ---- stage AllGathered MLP rows -> hfrd (inline) ----
                for chunk in range(16):
                    stg = xtp.tile([128, HEADS], F32, tag="stg")
                    dma(out=stg[:, :],
                        in_=hfr_g.ap()[chunk * 128:(chunk + 1) * 128, :])
                    pss = xpp.tile([128, QKV], F32, tag="qkv",
                                   name=f"stgps{chunk}")
                    nc.tensor.matmul(out=pss[0:HEADS, 0:128], lhsT=stg[:, :],
                                     rhs=ident[:, :], is_transpose=True,
                                     start=True, stop=True)
                    stgT = xtp.tile([HEADS, 128], F32, tag="stgTs")
                    nc.scalar.copy(out=stgT[:, :], in_=pss[0:HEADS, 0:128])
                    nc.tensor.matmul(out=pss[0:H_LOC, 128:256],
                                     lhsT=sel_sb[:, :],
                                     rhs=stgT[:, :], start=True, stop=True)
                    stl = xtp.tile([H_LOC, 128], BF16, tag="stl")
                    nc.scalar.copy(out=stl[:, :], in_=pss[0:H_LOC, 128:256])
                    dma(out=_ap(hfrd, [[HFRD_ROWS, H_LOC], [1, 128]],
                                HFR_PAD + chunk * 128),
                        in_=stl[:, :])
                poison = xtp.tile([H_LOC, HFRD_ROWS - 2112], BF16,
                                  name="poison")
                nc.vector.memset(poison[:, :], NEG)
                dma(out=_ap(hfrd, [[HFRD_ROWS, H_LOC],
                                   [1, HFRD_ROWS - 2112]], 2112),
                    in_=poison[:, :])

            early.__exit__(None, None, None)

            # ---------- Phase 3: attention + per-quarter out-proj + RS ------
            avT = [pers.tile([DH, N], BF16, tag=f"avT{h}", name=f"avT{h}")
                   for h in range(H_LOC)]
            mt = [pers.tile([128, MT_W], BF16, tag=f"mt{h}", name=f"mt{h}")
                  for h in range(H_LOC)]
            for h in range(H_LOC):
                dma(out=mt[h][:, :],
                    in_=_ap(hfrd, [[1, 128], [1, MT_W]], h * HFRD_ROWS + 63))
            if build.NO_AV:
                for h in range(H_LOC):
                    nc.vector.memset(avT[h][:, :], 0.0)

            with (
                tc.tile_pool(name="at", bufs=3) as atp,
                tc.tile_pool(name="sim4", bufs=2, space="PSUM") as simpp,
                tc.tile_pool(name="avps", bufs=2, space="PSUM") as avpp,
                tc.tile_pool(name="tps", bufs=1, space="PSUM") as tpp,
                tc.tile_pool(name="oq", bufs=1) as oqp,
            ):
                ones65 = atp.tile([DH + 1, DH], F32, tag="ones65",
                                  name="ones65")
                nc.vector.memset(ones65[:, :], 1.0)
                pend_tail = [None]

                def run_tail():
                    if pend_tail[0] is not None:
                        pend_tail[0]()
                        pend_tail[0] = None

                for q in range(4):
                    for m in (2 * q, 2 * q + 1):
                        i0 = m * 256
                        njt = 2 * m + 2
                        for h in range(H_LOC):
                            hp = (h % 2) * DH
                            qh = qT2[hp:hp + DH,
                                     (h // 2) * N + i0:(h // 2) * N + i0 + 256]
                            av_ps = avpp.tile([DH + 1, 256], F32, tag="av",
                                              name=f"av_{m}_{h}")
                            GSZ = 4
                            groups = [list(range(g, min(g + GSZ, njt)))
                                      for g in range(0, njt, GSZ)]
                            if len(groups[-1]) == GSZ:
                                # keep a spare exp column chunk for the
                                # null-key logits in the final group
                                groups[-1] = groups[-1][:GSZ - 1]
                                groups.append([njt - 1])
                            pend_av = None
                            av_state = [False]

                            def issue_av(pend, av_ps=av_ps, av_state=av_state):
                                pp4, pjts = pend
                                for ji, jt in enumerate(pjts):
                                    nc.tensor.matmul(
                                        out=av_ps[:, :],
                                        lhsT=v1[jt][:, :],
                                        rhs=pp4[:, ji * 256:ji * 256 + 256],
                                        start=(not av_state[0]), stop=False,
                                        skip_group_check=True)
                                    av_state[0] = True

                            for gi, jts in enumerate(groups):
                                gw = 256 * len(jts)
                                last = (gi == len(groups) - 1)
                                ps4 = simpp.tile([128, 1024], F32, tag="sim")
                                for ji, jt in enumerate(jts):
                                    j0 = jt * 128
                                    c0 = ji * 256
                                    # start=True only on the first chunk of
                                    # each 2KB psum bank: a start arms
                                    # zero-on-first-write for the whole bank
                                    nc.tensor.matmul(
                                        out=ps4[:, c0:c0 + 256],
                                        lhsT=kT[hp:hp + DH, j0:j0 + 128],
                                        rhs=qh,
                                        start=(c0 % 512 == 0), stop=False,
                                        skip_group_check=True)
                                # Toeplitz bias adds: two j-tiles merged per
                                # matmul via a 3D shifted AP (second touch of
                                # the armed bank, so plain accumulate)
                                for c0 in range(0, gw, 512):
                                    cn = min(2, (gw - c0) // 256)
                                    jt0 = jts[c0 // 256]
                                    u0 = 2048 - i0 + jt0 * 128
                                    mtr = mt[h][:, :].copy()
                                    pat = [list(p) for p in mtr.ap.to_list()]
                                    pat[1] = [128, cn]
                                    pat.append([-1, 256])
                                    mtr.ap = bass_rust.VecI64Pair(pat)
                                    mtr.offset = mtr.offset + u0
                                    nc.tensor.matmul(
                                        out=ps4[:, c0:c0 + cn * 256],
                                        lhsT=identb[:, :], rhs=mtr,
                                        start=False, stop=True,
                                        skip_group_check=True)
                                ew = gw
                                if last:
                                    # null-key logits ride along in the spare
                                    # columns of the final (partial) group
                                    nc.tensor.matmul(
                                        out=ps4[0:1, gw:gw + 256],
                                        lhsT=nkn[hp:hp + DH, :], rhs=qh,
                                        start=True, stop=True,
                                        skip_group_check=True)
                                    nc.vector.tensor_scalar_add(
                                        out=ps4[0:1, gw:gw + 256],
                                        in0=ps4[0:1, gw:gw + 256],
                                        scalar1=nb_sb[:, h:h + 1])
                                    ew = gw + 256
                                p4 = atp.tile([128, 1024], BF16, tag="p4")
                                nc.scalar.activation(
                                    out=p4[:, 0:ew], in_=ps4[:, 0:ew],
                                    func=mybir.ActivationFunctionType.Exp)
                                # software pipeline: issue deferred work now so
                                # the PE queue never parks waiting on this exp
                                if gi == 0:
                                    run_tail()
                                else:
                                    issue_av(pend_av)
                                pend_av = (p4, jts)

                            def tail(h=h, i0=i0, av_ps=av_ps, pend_av=pend_av,
                                     issue_av=issue_av,
                                     gw_last=256 * len(groups[-1])):
                                issue_av(pend_av)
                                nc.tensor.matmul(
                                    out=av_ps[:, :],
                                    lhsT=nv1[:, :],
                                    rhs=pend_av[0][0:1, gw_last:gw_last + 256],
                                    start=False, stop=True,
                                    skip_group_check=True)
                                # normalize columns by row-64 sums -> avT[h].
                                # full-height recip: base-partition-64 DVE
                                # slices silently no-op; only row 64 is read
                                # by the selector matmul below
                                rr = atp.tile([DH + 1, 256], F32, tag="rr")
                                nc.vector.reciprocal_approx_fast(
                                    out=rr[:, :], in_=av_ps[:, :])
                                psb = tpp.tile([DH, 256], F32, tag="bc")
                                nc.tensor.matmul(out=psb[:, :],
                                                 lhsT=ones65[DH:DH + 1, 0:DH],
                                                 rhs=rr[DH:DH + 1, :],
                                                 start=True, stop=True)
                                rb = atp.tile([DH, 256], BF16, tag="rb")
                                nc.scalar.copy(out=rb[:, :], in_=psb[:, :])
                                nc.vector.tensor_tensor(
                                    out=avT[h][:, i0:i0 + 256],
                                    in0=av_ps[0:DH, :], in1=rb[:, :],
                                    op=mybir.AluOpType.mult)
                            pend_tail[0] = tail

                    # out projection for this quarter
                    run_tail()
                    for tl in range(4):
                        tt = q * 4 + tl
                        ps_po = simpp.tile([128, 1024], F32, tag="sim")
                        for eb in range(2):
                            for ch in range(H_LOC):
                                nc.tensor.matmul(
                                    out=ps_po[:, eb * 512:eb * 512 + 512],
                                    lhsT=avT[ch][:, tt * 128:tt * 128 + 128],
                                    rhs=wout_sb[:, ch * D + eb * 512:
                                                ch * D + eb * 512 + 512],
                                    start=(ch == 0), stop=(ch == H_LOC - 1),
                                    skip_group_check=True)
                        po_sb = oqp.tile([128, D], BF16, tag="po")
                        nc.vector.tensor_copy(out=po_sb[:, :], in_=ps_po[:, :])
                        dma(out=po_q[q].ap()[tl * 128:(tl + 1) * 128, :],
                            in_=po_sb[:, :])
                    nc.gpsimd.collective_compute(
                        "ReduceScatter", mybir.AluOpType.add,
                        replica_groups=[[0, 1, 2, 3], [4, 5, 6, 7]],
                        ins=[po_q[q].ap().opt()],
                        outs=[rs_q[q].ap().opt()],
                    )
                    # final LN for the 128 owned rows of this quarter
                    y = oqp.tile([128, D], BF16, tag="y")
                    dma(out=y[:, :], in_=rs_q[q].ap()[:, :])
                    yscr = oqp.tile([128, D], BF16, tag="yscr")
                    mean, var, sd, rstd = _ln_stats(nc, oqp, y[:, :], D,
                                                    yscr[:, :])
                    nc.scalar.activation(out=sd[:, :], in_=var[:, :],
                                         func=mybir.ActivationFunctionType.Sqrt,
                                         bias=eps128[:, :])
                    nc.vector.reciprocal_approx_fast(out=rstd[:, :], in_=sd[:, :])
                    yf = oqp.tile([128, D], F32, tag="yf")
                    nc.vector.tensor_scalar(out=yf[:, :], in0=y[:, :],
                                            scalar1=mean[:, :],
                                            scalar2=rstd[:, :],
                                            op0=mybir.AluOpType.subtract,
                                            op1=mybir.AluOpType.mult)
                    nc.vector.tensor_tensor(out=yf[:, :], in0=yf[:, :],
                                            in1=goutb,
                                            op=mybir.AluOpType.mult)
                    dma(out=out_p.ap()[q * 128:(q + 1) * 128, :], in_=yf[:, :])
                if DBG:
                    dma(out=dbg_avT.ap(), in_=avT[0][:, :])
                    dma(out=dbg_mt.ap(), in_=mt[0][:, :])
                    dma(out=dbg_po.ap(), in_=po_q[0].ap()[0:128, :])

    nc.compile()
    return nc


def make_in_maps(x, mask, g_norm, Wq, Wkv, q_scale, k_scale, null_kv,
                 null_attn_bias, w0, b0, g0, w1, b1, g1, w2, b2, Wout, g_out):
    import ml_dtypes
    assert bool(np.asarray(mask).all()), "kernel assumes all-True mask"
    f = np.float32
    bf = ml_dtypes.bfloat16
    gn = np.asarray(g_norm, f)
    in_maps = []
    for c in range(N_CORES):
        bi, hg = c // 4, c % 4
        posb = (2047 - c * MLP_ROWS - np.arange(128)).astype(np.int32)
        wq_hg = np.asarray(Wq, f)[:, hg * INNER_LOC:(hg + 1) * INNER_LOC]
        wqkv = np.concatenate(
            [wq_hg, np.asarray(Wkv, f)], axis=1) * gn[:, None]
        wsum = -(wqkv.sum(axis=0)) / float(D)
        m = {
            "x": np.ascontiguousarray(np.asarray(x, f)[bi]).astype(bf),
            "wqkv": np.ascontiguousarray(wqkv).astype(bf),
            "wsum": np.ascontiguousarray(wsum.astype(f)),
            "qs8ks": np.ascontiguousarray(np.tile(
                8.0 * np.asarray(q_scale, f) * np.asarray(k_scale, f), 2)),
            "nkv": np.ascontiguousarray(np.tile(np.asarray(null_kv, f),
                                                (1, 2))),
            "nb": np.ascontiguousarray(
                np.asarray(null_attn_bias, f)[hg * H_LOC:(hg + 1) * H_LOC]),
            "w0v": np.ascontiguousarray(np.asarray(w0, f).reshape(D)),
            "b0": np.ascontiguousarray(np.asarray(b0, f)),
            "g0": np.ascontiguousarray(np.asarray(g0, f)),
            "w1": np.ascontiguousarray(np.asarray(w1, f)).astype(bf),
            "b1": np.ascontiguousarray(np.asarray(b1, f)),
            "g1": np.ascontiguousarray(np.asarray(g1, f)),
            "w2": np.ascontiguousarray(np.asarray(w2, f)).astype(bf),
            "b2": np.ascontiguousarray(np.asarray(b2, f)),
            "wout": np.ascontiguousarray(
                np.asarray(Wout, f)[hg * INNER_LOC:(hg + 1) * INNER_LOC, :]
            ).astype(bf),
            "g_out": np.ascontiguousarray(np.asarray(g_out, f)),
            "posb": posb,
            "sel": np.eye(HEADS, dtype=f)[:, hg * H_LOC:(hg + 1) * H_LOC].copy(),
        }
        in_maps.append(m)
    return in_maps


_NC_CACHE = None


def kernel(**inputs):
    global _NC_CACHE
    from concourse.bass_utils import run_bass_kernel_spmd

    if _NC_CACHE is None:
        _NC_CACHE = build()
    nc = _NC_CACHE
    in_maps = make_in_maps(**inputs)
    res = run_bass_kernel_spmd(nc, in_maps, core_ids=list(range(N_CORES)))
    outs = res.results
    kernel.last_outs = outs
    full = np.empty((2, N, D), np.float32)
    for c in range(N_CORES):
        bi, hg = c // 4, c % 4
        o = np.asarray(outs[c]["out"], np.float32)
        for q in range(4):
            full[bi, q * 512 + hg * 128:q * 512 + (hg + 1) * 128, :] = \
                o[q * 128:(q + 1) * 128]
    return full


# revision 50
# speedup vs baseline: 1.3784x; 1.0940x over previous
import sys

for _p in ("/opt/trn_rl_repo", "/root/.axon_site/_ro/trn_rl_repo"):
    if _p not in sys.path:
        sys.path.insert(0, _p)

import numpy as np

from concourse import bacc, mybir, tile
import bass_rust

N_CORES = 8
N = 2048
D = 1024
HEADS = 16
DH = 64
H_LOC = 4          # heads per core
INNER_LOC = H_LOC * DH  # 256
QKV = INNER_LOC + 2 * DH  # 384 packed projection width
NEG = -1.0e30
EPS = 1e-5
F32 = mybir.dt.float32
F32R = mybir.dt.float32r
BF16 = mybir.dt.bfloat16
I32 = mybir.dt.int32

# rel-pos MLP sharding: 2048 useful reversed-position rows, 256 per core.
MLP_ROWS = 256
HFR_PAD = 64       # data lives at HFRD[64 : 64+2048]
HFRD_ROWS = 2752   # covers all reads [64, 2494]
MT_W = 2560        # master toeplitz width


def _ap(t, pattern, offset):
    a = t.ap().copy()
    a.ap = bass_rust.VecI64Pair(pattern)
    a.offset = offset
    return a


def _ln_stats(nc, pool, x_sb, width, scratch):
    """Row LayerNorm stats for [128, width] tile -> (mean, var, sd, rstd)."""
    s = pool.tile([128, 1], F32, tag="ln_s")
    ssq = pool.tile([128, 1], F32, tag="ln_ssq")
    mean = pool.tile([128, 1], F32, tag="ln_mean")
    var = pool.tile([128, 1], F32, tag="ln_var")
    sd = pool.tile([128, 1], F32, tag="ln_sd")
    rstd = pool.tile([128, 1], F32, tag="ln_rstd")
    nc.vector.tensor_reduce(out=s[:, :], in_=x_sb, axis=mybir.AxisListType.X,
                            op=mybir.AluOpType.add)
    nc.scalar.activation(out=scratch, in_=x_sb,
                         func=mybir.ActivationFunctionType.Square,
                         accum_out=ssq[:, :])
    m2 = pool.tile([128, 1], F32, tag="ln_m2")
    nc.vector.tensor_scalar_mul(out=mean[:, :], in0=s[:, :], scalar1=1.0 / width)
    nc.vector.tensor_tensor(out=m2[:, :], in0=mean[:, :], in1=mean[:, :],
                            op=mybir.AluOpType.mult)
    nc.vector.tensor_scalar(out=var[:, :], in0=ssq[:, :], scalar1=1.0 / width,
                            scalar2=None, op0=mybir.AluOpType.mult)
    nc.vector.tensor_tensor(out=var[:, :], in0=var[:, :], in1=m2[:, :],
                            op=mybir.AluOpType.subtract)
    return mean, var, sd, rstd


def build():
    build.NO_AV = globals().get('NO_AV', False)
    nc = bacc.Bacc("TRN2", target_bir_lowering=False, debug=False,
                   num_devices=N_CORES)

    # ---------------- parameters ----------------
    x_p = nc.declare_dram_parameter("x", [N, D], BF16, isOutput=False)
    wqkv_p = nc.declare_dram_parameter("wqkv", [D, QKV], BF16, isOutput=False)
    wsum_p = nc.declare_dram_parameter("wsum", [QKV], F32, isOutput=False)
    # qs8ks and null-k are passed partition-duplicated ([x | x] over 128
    # partitions) so odd heads can slice base-partition-64 operands.
    qs8ks_p = nc.declare_dram_parameter("qs8ks", [128], F32, isOutput=False)
    nkv_p = nc.declare_dram_parameter("nkv", [2, 128], F32, isOutput=False)
    nb_p = nc.declare_dram_parameter("nb", [H_LOC], F32, isOutput=False)
    w0_p = nc.declare_dram_parameter("w0v", [D], F32, isOutput=False)
    b0_p = nc.declare_dram_parameter("b0", [D], F32, isOutput=False)
    g0_p = nc.declare_dram_parameter("g0", [D], F32, isOutput=False)
    w1_p = nc.declare_dram_parameter("w1", [D, D], BF16, isOutput=False)
    b1_p = nc.declare_dram_parameter("b1", [D], F32, isOutput=False)
    g1_p = nc.declare_dram_parameter("g1", [D], F32, isOutput=False)
    w2_p = nc.declare_dram_parameter("w2", [D, HEADS], BF16, isOutput=False)
    b2_p = nc.declare_dram_parameter("b2", [HEADS], F32, isOutput=False)
    wout_p = nc.declare_dram_parameter("wout", [INNER_LOC, D], BF16,
                                       isOutput=False)
    gout_p = nc.declare_dram_parameter("g_out", [D], F32, isOutput=False)
    posb_p = nc.declare_dram_parameter("posb", [128], I32, isOutput=False)
    sel_p = nc.declare_dram_parameter("sel", [HEADS, H_LOC], F32, isOutput=False)
    out_p = nc.declare_dram_parameter("out", [N // 4, D], F32, isOutput=True)
    DBG = globals().get('DEBUG_TAPS', False)
    if DBG:
        dbg_qT = nc.declare_dram_parameter("dbg_qT", [128, N], BF16, isOutput=True)
        dbg_kT = nc.declare_dram_parameter("dbg_kT", [128, N], BF16, isOutput=True)
        dbg_v0 = nc.declare_dram_parameter("dbg_v0", [128, DH + 1], BF16, isOutput=True)
        dbg_avT = nc.declare_dram_parameter("dbg_avT", [DH, N], BF16, isOutput=True)
        dbg_po = nc.declare_dram_parameter("dbg_po", [128, D], BF16, isOutput=True)
        dbg_mt = nc.declare_dram_parameter("dbg_mt", [128, MT_W], BF16, isOutput=True)
        dbg_avps = nc.declare_dram_parameter("dbg_avps", [DH + 1, N], F32, isOutput=True)
        dbg_rb = nc.declare_dram_parameter("dbg_rb", [DH, N], BF16, isOutput=True)
        dbg_p4 = nc.declare_dram_parameter("dbg_p4", [128, 1024], BF16, isOutput=True)
        dbg_ps4 = nc.declare_dram_parameter("dbg_ps4", [128, 1024], F32, isOutput=True)

    # ---------------- internal DRAM ----------------
    hfr_loc = nc.dram_tensor("hfr_loc", [MLP_ROWS, HEADS], F32)
    hfr_g = nc.dram_tensor("hfr_g", [8 * MLP_ROWS, HEADS], F32)
    hfrd = nc.dram_tensor("hfrd", [H_LOC, HFRD_ROWS], BF16)
    po_q = [nc.dram_tensor(f"po_{i}", [N // 4, D], BF16) for i in range(4)]
    rs_q = [nc.dram_tensor(f"rs_{i}", [N // 16, D], BF16) for i in range(4)]

    dma = nc.sync.dma_start

    with tile.TileContext(nc) as tc:
        with (
            tc.tile_pool(name="const", bufs=1) as constp,
            tc.tile_pool(name="pers", bufs=1) as pers,
            tc.tile_pool(name="weights", bufs=1) as wp,
        ):
            ident = constp.tile([128, 128], F32)
            from concourse.masks import make_identity
            make_identity(nc, ident[:, :])
            identb = constp.tile([128, 128], BF16)
            nc.scalar.copy(out=identb[:, :], in_=ident[:, :])
            eps128 = constp.tile([128, 1], F32)
            nc.vector.memset(eps128[:, :], EPS)
            ones1 = constp.tile([1, 128], F32)
            nc.vector.memset(ones1[:, :], 1.0)

            # ---- all big input loads issued up front (single SP queue) ----
            early = tc.tile_pool(name="early", bufs=1)
            ep = early.__enter__()
            xall = ep.tile([128, 16 * D], BF16, name="xall")
            dma(out=xall[:, :], in_=_ap(x_p, [[D, 128], [128 * D, 16], [1, D]], 0))
            wqkv_sb = ep.tile([128, 8 * QKV], BF16)
            dma(out=wqkv_sb[:, :],
                in_=_ap(wqkv_p, [[QKV, 128], [128 * QKV, 8], [1, QKV]], 0))
            wout_sb = wp.tile([DH, 4 * D], BF16)
            dma(out=wout_sb[:, :],
                in_=_ap(wout_p, [[D, DH], [DH * D, 4], [1, D]], 0))
            w1_sb = ep.tile([128, 8 * D], BF16)  # chunk k at cols k*1024
            dma(out=w1_sb[:, :],
                in_=_ap(w1_p, [[D, 128], [128 * D, 8], [1, D]], 0))
            w2_sb = ep.tile([128, 8 * HEADS], BF16)
            dma(out=w2_sb[:, :],
                in_=_ap(w2_p, [[HEADS, 128], [128 * HEADS, 8], [1, HEADS]], 0))

            wsum_f32 = ep.tile([1, QKV], F32)
            dma(out=wsum_f32[:, :], in_=_ap(wsum_p, [[QKV, 1], [1, QKV]], 0))
            wsum_row = ep.tile([1, QKV], BF16)
            nc.vector.tensor_copy(out=wsum_row[:, :], in_=wsum_f32[:, :])
            qs8ks_sb = pers.tile([128, 1], F32)
            dma(out=qs8ks_sb[:, :], in_=_ap(qs8ks_p, [[1, 128], [1, 1]], 0))
            nkT = pers.tile([128, 1], F32)
            dma(out=nkT[:, :], in_=_ap(nkv_p, [[1, 128], [1, 1]], 0))
            nv_sb = pers.tile([1, DH], F32)
            dma(out=nv_sb[:, :], in_=nkv_p.ap()[1:2, 0:DH])
            nb_sb = pers.tile([1, H_LOC], F32)
            dma(out=nb_sb[:, :], in_=_ap(nb_p, [[H_LOC, 1], [1, H_LOC]], 0))
            posi_t = pers.tile([128, 1], I32, name="posi")
            dma(out=posi_t[:, :], in_=_ap(posb_p, [[1, 128], [1, 1]], 0))
            sel_sb = wp.tile([HEADS, H_LOC], F32)
            dma(out=sel_sb[:, :], in_=sel_p.ap())

            # [nv | 1] bf16 row for null-key AV
            nv1 = pers.tile([1, DH + 1], BF16)
            nc.vector.tensor_copy(out=nv1[:, 0:DH], in_=nv_sb[:, :])
            nc.vector.memset(nv1[:, DH:DH + 1], 1.0)

            # ---------- Phases 0-2 interleaved: rel-pos MLP generator is ----
            # pumped between projection steps so its long serial chain fills
            # engine gaps instead of blocking the in-order queues.
            qT2 = pers.tile([128, 2 * N], BF16, name="qT2")
            kT = pers.tile([128, N], BF16, tag="kT", name="kT")
            v1 = [pers.tile([128, DH + 1], BF16, tag=f"v1_{j}", name=f"v1_{j}")
                  for j in range(16)]
            nkn = pers.tile([128, 1], BF16)

            with (
                tc.tile_pool(name="bc_ps", bufs=2, space="PSUM") as bpp,
                tc.tile_pool(name="vst", bufs=3) as vstp,
            ):
                bcast = ep.tile([128, 4 * D + HEADS], BF16)
                g1b = ep.tile([128, D], BF16)
                goutb = wp.tile([128, D], F32)
                chunks = []
                for pi, par in enumerate((w0_p, b0_p, g0_p, b1_p)):
                    for half in range(2):
                        chunks.append((par, half * 512, 512, bcast,
                                       pi * D + half * 512))
                chunks.append((b2_p, 0, HEADS, bcast, 4 * D))
                for half in range(2):
                    chunks.append((g1_p, half * 512, 512, g1b, half * 512))
                    chunks.append((gout_p, half * 512, 512, goutb,
                                   half * 512))
                for par, poff, wdt, dst, doff in chunks:
                    vstage = vstp.tile([1, 512], F32, tag="vstage")
                    dma(out=vstage[:, 0:wdt],
                        in_=_ap(par, [[wdt, 1], [1, wdt]], poff))
                    ps = bpp.tile([128, 512], F32, tag="bc")
                    nc.tensor.matmul(out=ps[:, 0:wdt],
                                     lhsT=ones1[:, :],
                                     rhs=vstage[:, 0:wdt],
                                     start=True, stop=True)
                    nc.scalar.copy(out=dst[:, doff:doff + wdt],
                                   in_=ps[:, 0:wdt])
            w0b = bcast[:, 0:D]
            b0b = bcast[:, D:2 * D]
            g0b = bcast[:, 2 * D:3 * D]
            b1b = bcast[:, 3 * D:4 * D]
            b2b = bcast[:, 4 * D:4 * D + HEADS]

            with (
                tc.tile_pool(name="mlp", bufs=1) as mp,
                tc.tile_pool(name="mlp_ps", bufs=1, space="PSUM") as mpp,
                tc.tile_pool(name="mlp_ps2", bufs=1, space="PSUM") as mpp2,
                tc.tile_pool(name="xT", bufs=1) as xTp,
                tc.tile_pool(name="xt", bufs=2) as xtp,
                tc.tile_pool(name="xps", bufs=2, space="PSUM") as xpp,
                tc.tile_pool(name="xps2", bufs=1, space="PSUM") as xpp2,
                tc.tile_pool(name="xps3", bufs=1, space="PSUM") as xpp3,
            ):
                xT = xTp.tile([128, 8 * N], BF16)  # d-chunk k at cols k*2048
                negrow = wp.tile([1, N], BF16, name="negrow")

                def mlp_gen():
                    for t in range(2):
                        m0 = t * 128
                        posf = mp.tile([128, 1], F32, tag="posf",
                                       name=f"posf{t}")
                        nc.vector.tensor_scalar_add(out=posf[:, :],
                                                    in0=posi_t[:, :],
                                                    scalar1=float(-m0))
                        h0 = mp.tile([128, D], F32, tag="h0", name=f"h0_{t}")
                        scratch = mp.tile([128, D], BF16, tag="scr",
                                          name=f"scr{t}")
                        nc.vector.tensor_scalar(out=h0[:, :], in0=w0b,
                                                scalar1=posf[:, :], scalar2=None,
                                                op0=mybir.AluOpType.mult)
                        nc.vector.tensor_tensor(out=h0[:, :], in0=h0[:, :],
                                                in1=b0b,
                                                op=mybir.AluOpType.add)
                        yield
                        mean, var, sd, rstd = _ln_stats(nc, mp, h0[:, :], D,
                                                        scratch[:, :])
                        nc.scalar.activation(
                            out=sd[:, :], in_=var[:, :],
                            func=mybir.ActivationFunctionType.Sqrt,
                            bias=eps128[:, :])
                        nc.vector.reciprocal_approx_fast(out=rstd[:, :],
                                                         in_=sd[:, :])
                        yield
                        nc.vector.tensor_scalar(out=h0[:, :], in0=h0[:, :],
                                                scalar1=mean[:, :],
                                                scalar2=rstd[:, :],
                                                op0=mybir.AluOpType.subtract,
                                                op1=mybir.AluOpType.mult)
                        nc.vector.tensor_tensor(out=h0[:, :], in0=h0[:, :],
                                                in1=g0b,
                                                op=mybir.AluOpType.mult)
                        h0b = mp.tile([128, D], BF16, tag="h0b",
                                      name=f"h0b{t}")
                        nc.scalar.activation(
                            out=h0b[:, :], in_=h0[:, :],
                            func=mybir.ActivationFunctionType.Silu)
                        yield
                        h0T = mp.tile([128, D], BF16, tag="h0T",
                                      name=f"h0T{t}")
                        pst8m = mpp2.tile([128, D], BF16, tag="tp",
                                          name=f"tp0_{t}")
                        for k in range(8):
                            nc.tensor.matmul(
                                out=pst8m[:, k * 128:(k + 1) * 128],
                                lhsT=h0b[:, k * 128:(k + 1) * 128],
                                rhs=identb[:, :], is_transpose=True,
                                start=True, stop=True)
                            if k == 3:
                                yield
                        nc.vector.tensor_copy(out=h0T[:, :], in_=pst8m[:, :])
                        yield
                        h1 = mp.tile([128, D], F32, tag="h1", name=f"h1_{t}")
                        for eb in range(2):
                            ps = mpp.tile([128, 512], F32, tag="h1ps",
                                          name=f"h1ps{t}_{eb}")
                            for k in range(8):
                                nc.tensor.matmul(
                                    out=ps[:, :],
                                    lhsT=h0T[:, k * 128:(k + 1) * 128],
                                    rhs=w1_sb[:, k * D + eb * 512:
                                              k * D + eb * 512 + 512],
                                    start=(k == 0), stop=(k == 7))
                            nc.vector.tensor_tensor(
                                out=h1[:, eb * 512:eb * 512 + 512],
                                in0=ps[:, :],
                                in1=b1b[:, eb * 512:eb * 512 + 512],
                                op=mybir.AluOpType.add)
                            yield
                        mean, var, sd, rstd = _ln_stats(nc, mp, h1[:, :], D,
                                                        scratch[:, :])
                        nc.scalar.activation(
                            out=sd[:, :], in_=var[:, :],
                            func=mybir.ActivationFunctionType.Sqrt,
                            bias=eps128[:, :])
                        nc.vector.reciprocal_approx_fast(out=rstd[:, :],
                                                         in_=sd[:, :])
                        yield
                        nc.vector.tensor_scalar(out=h1[:, :], in0=h1[:, :],
                                                scalar1=mean[:, :],
                                                scalar2=rstd[:, :],
                                                op0=mybir.AluOpType.subtract,
                                                op1=mybir.AluOpType.mult)
                        nc.vector.tensor_tensor(out=h1[:, :], in0=h1[:, :],
                                                in1=g1b,
                                                op=mybir.AluOpType.mult)
                        h1b = mp.tile([128, D], BF16, tag="h1b",
                                      name=f"h1b{t}")
                        nc.scalar.activation(
                            out=h1b[:, :], in_=h1[:, :],
                            func=mybir.ActivationFunctionType.Silu)
                        yield
                        h1T = mp.tile([128, D], BF16, tag="h1T",
                                      name=f"h1T{t}")
                        pst8n = mpp2.tile([128, D], BF16, tag="tp",
                                          name=f"tp1_{t}")
                        for k in range(8):
                            nc.tensor.matmul(
                                out=pst8n[:, k * 128:(k + 1) * 128],
                                lhsT=h1b[:, k * 128:(k + 1) * 128],
                                rhs=identb[:, :], is_transpose=True,
                                start=True, stop=True)
                            if k == 3:
                                yield
                        nc.vector.tensor_copy(out=h1T[:, :], in_=pst8n[:, :])
                        yield
                        psf = mpp2.tile([128, HEADS], F32, tag="hf",
                                        name=f"hf{t}")
                        for k in range(8):
                            nc.tensor.matmul(
                                out=psf[:, :],
                                lhsT=h1T[:, k * 128:(k + 1) * 128],
                                rhs=w2_sb[:, k * HEADS:(k + 1) * HEADS],
                                start=(k == 0), stop=(k == 7))
                        hfc = mp.tile([128, HEADS], F32, tag="hfc",
                                      name=f"hfc{t}")
                        nc.vector.tensor_tensor(out=hfc[:, :], in0=psf[:, :],
                                                in1=b2b,
                                                op=mybir.AluOpType.add)
                        dma(out=hfr_loc.ap()[m0:m0 + 128, :], in_=hfc[:, :])
                        yield
                    nc.gpsimd.collective_compute(
                        "AllGather", mybir.AluOpType.bypass,
                        replica_groups=[list(range(N_CORES))],
                        ins=[hfr_loc.ap().opt()],
                        outs=[hfr_g.ap().opt()],
                    )

                mgen = mlp_gen()

                def pump(n=1):
                    for _ in range(n):
                        try:
                            next(mgen)
                        except StopIteration:
                            return

                for tt in range(16):
                    xs = xall[:, tt * D:(tt + 1) * D]
                    # stats
                    sums = xtp.tile([128, 1], F32, tag="sums")
                    nc.vector.tensor_reduce(out=sums[:, :], in_=xs,
                                            axis=mybir.AxisListType.X,
                                            op=mybir.AluOpType.add)
                    scr = xtp.tile([128, D], BF16, tag="scr")
                    ssq = xtp.tile([128, 1], F32, tag="ssq")
                    nc.scalar.activation(out=scr[:, :], in_=xs,
                                         func=mybir.ActivationFunctionType.Square,
                                         accum_out=ssq[:, :])
                    mean = xtp.tile([128, 1], F32, tag="mean")
                    m2 = xtp.tile([128, 1], F32, tag="m2")
                    var = xtp.tile([128, 1], F32, tag="var")
                    sd = xtp.tile([128, 1], F32, tag="sd")
                    rstd = xtp.tile([128, 1], F32, tag="rstd")
                    nc.vector.tensor_scalar_mul(out=mean[:, :], in0=sums[:, :],
                                                scalar1=1.0 / D)
                    nc.vector.tensor_tensor(out=m2[:, :], in0=mean[:, :],
                                            in1=mean[:, :],
                                            op=mybir.AluOpType.mult)
                    nc.vector.tensor_scalar(out=var[:, :], in0=ssq[:, :],
                                            scalar1=1.0 / D, scalar2=None,
                                            op0=mybir.AluOpType.mult)
                    nc.vector.tensor_tensor(out=var[:, :], in0=var[:, :],
                                            in1=m2[:, :],
                                            op=mybir.AluOpType.subtract)
                    nc.scalar.activation(out=sd[:, :], in_=var[:, :],
                                         func=mybir.ActivationFunctionType.Sqrt,
                                         bias=eps128[:, :])
                    nc.vector.reciprocal_approx_fast(out=rstd[:, :],
                                                     in_=sd[:, :])
                    # negsum row: transpose sums -> [1, 128] (wsum carries -1/D)
                    psr = xpp3.tile([128, 128], F32, tag="misc",
                                    name=f"psr{tt}")
                    nc.tensor.matmul(out=psr[0:1, :], lhsT=sums[:, :],
                                     rhs=ident[:, :], is_transpose=True,
                                     start=True, stop=True)
                    nc.vector.tensor_copy(out=negrow[:, tt * 128:tt * 128 + 128],
                                          in_=psr[0:1, :])
                    pump(1)
                    # x transposes (bf16): 8 into one psum tile, one fat copy
                    pst8 = xpp2.tile([128, D], BF16, tag="tp8",
                                     name=f"pst8_{tt}")
                    for k in range(8):
                        nc.tensor.matmul(out=pst8[:, k * 128:(k + 1) * 128],
                                         lhsT=xall[:, tt * D + k * 128:
                                                   tt * D + k * 128 + 128],
                                         rhs=identb[:, :], is_transpose=True,
                                         start=True, stop=True)
                    xTo = xT[:, :].copy()
                    xpat = [list(p) for p in xTo.ap.to_list()]
                    xpat = [xpat[0], [N, 8], [1, 128]]
                    xTo.ap = bass_rust.VecI64Pair(xpat)
                    xTo.offset = xTo.offset + tt * 128
                    nc.vector.tensor_copy(out=xTo, in_=pst8[:, :])
                    pump(1)
                    # packed q|k|v projection with rank-1 mean correction
                    psq = xpp.tile([128, QKV], F32, tag="qkv")
                    for k in range(8):
                        nc.tensor.matmul(
                            out=psq[:, :],
                            lhsT=xT[:, k * N + tt * 128:k * N + tt * 128 + 128],
                            rhs=wqkv_sb[:, k * QKV:(k + 1) * QKV],
                            start=(k == 0), stop=False,
                            skip_group_check=True)
                    nc.tensor.matmul(out=psq[:, :],
                                     lhsT=negrow[:, tt * 128:tt * 128 + 128],
                                     rhs=wsum_row[:, :],
                                     start=False, stop=True,
                                     skip_group_check=True)
                    # per-head l2 norms (4 q heads + k)
                    nrm = xtp.tile([128, 8], F32, tag="nrm")
                    scr2 = xtp.tile([128, DH], BF16, tag="scr2")
                    for j in range(5):
                        nc.scalar.activation(
                            out=scr2[:, :],
                            in_=psq[:, j * DH:(j + 1) * DH],
                            func=mybir.ActivationFunctionType.Square,
                            accum_out=nrm[:, j:j + 1])
                    sd5 = xtp.tile([128, 8], F32, tag="sd5")
                    rinv = xtp.tile([128, 8], F32, tag="rinv")
                    nc.scalar.activation(out=sd5[:, 0:5], in_=nrm[:, 0:5],
                                         func=mybir.ActivationFunctionType.Sqrt)
                    nc.vector.reciprocal(out=rinv[:, 0:5], in_=sd5[:, 0:5])
                    pump(1)
                    # scaled copies out of PSUM
                    qn = xtp.tile([128, INNER_LOC], BF16, tag="qn")
                    for h in range(4):
                        eng = nc.vector if h % 2 == 0 else None
                        if h % 2 == 0:
                            nc.vector.tensor_scalar(
                                out=qn[:, h * DH:(h + 1) * DH],
                                in0=psq[:, h * DH:(h + 1) * DH],
                                scalar1=rinv[:, h:h + 1], scalar2=None,
                                op0=mybir.AluOpType.mult)
                        else:
                            nc.scalar.activation(
                                out=qn[:, h * DH:(h + 1) * DH],
                                in_=psq[:, h * DH:(h + 1) * DH],
                                func=mybir.ActivationFunctionType.Copy,
                                scale=rinv[:, h:h + 1])
                    # kn duplicated into both column halves so the transpose
                    # yields kT stacked twice along partitions
                    kn = xtp.tile([128, 128], BF16, tag="kn")
                    for kh in range(2):
                        nc.vector.tensor_scalar(
                            out=kn[:, kh * DH:(kh + 1) * DH],
                            in0=psq[:, INNER_LOC:INNER_LOC + DH],
                            scalar1=rinv[:, 4:5], scalar2=None,
                            op0=mybir.AluOpType.mult)
                    nc.vector.tensor_scalar(out=v1[tt][:, 0:DH],
                                            in0=psq[:, INNER_LOC + DH:QKV],
                                            scalar1=rstd[:, :], scalar2=None,
                                            op0=mybir.AluOpType.mult)
                    nc.vector.memset(v1[tt][:, DH:DH + 1], 1.0)
                    # q pair + k transposes into one psum tile
                    pstqk = xpp2.tile([128, 384], BF16, tag="tpqk",
                                      name=f"pstqk{tt}")
                    for p in range(2):
                        nc.tensor.matmul(out=pstqk[:, p * 128:(p + 1) * 128],
                                         lhsT=qn[:, p * 128:(p + 1) * 128],
                                         rhs=identb[:, :], is_transpose=True,
                                         start=True, stop=True)
                    nc.tensor.matmul(out=pstqk[:, 256:384], lhsT=kn[:, :],
                                     rhs=identb[:, :], is_transpose=True,
                                     start=True, stop=True)
                    qTo = qT2[:, :].copy()
                    qpat = [list(p) for p in qTo.ap.to_list()]
                    qpat = [qpat[0], [N, 2], [1, 128]]
                    qTo.ap = bass_rust.VecI64Pair(qpat)
                    qTo.offset = qTo.offset + tt * 128
                    nc.scalar.copy(out=qTo, in_=pstqk[:, 0:256])
                    # k transpose with qs8ks scale folded in
                    nc.vector.tensor_scalar(out=kT[:, tt * 128:tt * 128 + 128],
                                            in0=pstqk[:, 256:384],
                                            scalar1=qs8ks_sb[:, :], scalar2=None,
                                            op0=mybir.AluOpType.mult)
                    pump(1)

                pump(100)

                if DBG:
                    dma(out=dbg_qT.ap(), in_=qT2[:, 0:N])
                    dma(out=dbg_kT.ap(), in_=kT[:, :])
                    dma(out=dbg_v0.ap(), in_=v1[0][:, :])

                # null key normalize: nkn = l2norm(nk) * qs8ks  (dup over 128)
                ones64c_f = constp.tile([DH, 1], F32)
                nc.vector.memset(ones64c_f[:, :], 1.0)
                nsq = xtp.tile([128, 1], F32, tag="nsq")
                nc.scalar.activation(out=nsq[:, :], in_=nkT[:, :],
                                     func=mybir.ActivationFunctionType.Square)
                psn1 = xpp3.tile([128, 128], F32, tag="misc", name="psn1")
                nc.tensor.matmul(out=psn1[0:1, 0:1], lhsT=ones64c_f[:, :],
                                 rhs=nsq[0:DH, :], start=True, stop=True)
                rn1 = xtp.tile([1, 1], F32, tag="rn1")
                nc.scalar.activation(out=rn1[:, :], in_=psn1[0:1, 0:1],
                                     func=mybir.ActivationFunctionType.Sqrt)
                with nc.allow_low_precision(reason="f32r same bits as f32"):
                    nc.vector.reciprocal(out=rn1[:, :], in_=rn1[:, :])
                psb1 = xpp3.tile([128, 128], F32, tag="misc", name="psb1")
                nc.tensor.matmul(out=psb1[:, 0:1], lhsT=ones1[:, :],
                                 rhs=rn1[:, :], start=True, stop=True)
                nc.vector.tensor_tensor(out=nkn[:, :], in0=nkT[:, :],
                                        in1=psb1[:, 0:1],
                                        op=mybir.AluOpType.mult)
                nc.vector.tensor_scalar(out=nkn[:, :], in0=nkn[:, :],
                                        scalar1=qs8ks_sb[:, :], scalar2=None,
                                        op0=mybir.AluOpType.mult)

                # ---- stage AllGathered MLP rows -> hfrd (batched) ----
                stg = xtp.tile([128, 16 * HEADS], F32, tag="stg",
                               name="stg_all")
                dma(out=stg[:, :],
                    in_=_ap(hfr_g, [[HEADS, 128], [128 * HEADS, 16],
                                    [1, HEADS]], 0))
                stgT = xTp.tile([HEADS, 16 * 128], F32, name="stgT")
                for chunk in range(16):
                    pss = xpp.tile([128, QKV], F32, tag="qkv",
                                   name=f"stgps{chunk}")
                    nc.tensor.matmul(out=pss[0:HEADS, 0:128],
                                     lhsT=stg[:, chunk * HEADS:
                                              (chunk + 1) * HEADS],
                                     rhs=ident[:, :], is_transpose=True,
                                     start=True, stop=True)
                    eng = nc.scalar if chunk % 2 == 0 else nc.vector
                    if chunk % 2 == 0:
                        nc.scalar.copy(
                            out=stgT[:, chunk * 128:(chunk + 1) * 128],
                            in_=pss[0:HEADS, 0:128])
                    else:
                        nc.vector.tensor_copy(
                            out=stgT[:, chunk * 128:(chunk + 1) * 128],
                            in_=pss[0:HEADS, 0:128])
                # select local heads and write hfrd in 512-col pieces
                for piece in range(4):
                    psl = xpp3.tile([128, 128], F32, tag="misc",
                                    name=f"psl{piece}")
                    stl = xtp.tile([H_LOC, 512], BF16, tag="stl")
                    for sub in range(4):
                        col = piece * 512 + sub * 128
                        nc.tensor.matmul(out=psl[0:H_LOC, 0:128],
                                         lhsT=sel_sb[:, :],
                                         rhs=stgT[:, col:col + 128],
                                         start=True, stop=True)
                        if sub % 2 == 0:
                            nc.scalar.copy(out=stl[:, sub * 128:sub * 128 + 128],
                                           in_=psl[0:H_LOC, 0:128])
                        else:
                            nc.vector.tensor_copy(
                                out=stl[:, sub * 128:sub * 128 + 128],
                                in_=psl[0:H_LOC, 0:128])
                    dma(out=_ap(hfrd, [[HFRD_ROWS, H_LOC], [1, 512]],
                                HFR_PAD + piece * 512),
                        in_=stl[:, :])
                poison = xtp.tile([H_LOC, HFRD_ROWS - 2112], BF16,
                                  name="poison")
                nc.vector.memset(poison[:, :], NEG)
                dma(out=_ap(hfrd, [[HFRD_ROWS, H_LOC],
                                   [1, HFRD_ROWS - 2112]], 2112),
                    in_=poison[:, :])

            early.__exit__(None, None, None)

            # ---------- Phase 3: attention + per-quarter out-proj + RS ------
            avT = [pers.tile([DH, N], BF16, tag=f"avT{h}", name=f"avT{h}")
                   for h in range(H_LOC)]
            mt = [pers.tile([128, MT_W], BF16, tag=f"mt{h}", name=f"mt{h}")
                  for h in range(H_LOC)]
            for h in range(H_LOC):
                dma(out=mt[h][:, :],
                    in_=_ap(hfrd, [[1, 128], [1, MT_W]], h * HFRD_ROWS + 63))
            if build.NO_AV:
                for h in range(H_LOC):
                    nc.vector.memset(avT[h][:, :], 0.0)

            with (
                tc.tile_pool(name="at", bufs=3) as atp,
                tc.tile_pool(name="sim4", bufs=2, space="PSUM") as simpp,
                tc.tile_pool(name="avps", bufs=2, space="PSUM") as avpp,
                tc.tile_pool(name="tps", bufs=1, space="PSUM") as tpp,
                tc.tile_pool(name="oq", bufs=2) as oqp,
            ):
                ones65 = atp.tile([DH + 1, DH], F32, tag="ones65",
                                  name="ones65")
                nc.vector.memset(ones65[:, :], 1.0)
                pend_tail = [None]

                def run_tail():
                    if pend_tail[0] is not None:
                        pend_tail[0]()
                        pend_tail[0] = None

                for q in range(4):
                    for m in (2 * q, 2 * q + 1):
                        i0 = m * 256
                        njt = 2 * m + 2
                        for h in range(H_LOC):
                            hp = (h % 2) * DH
                            qh = qT2[hp:hp + DH,
                                     (h // 2) * N + i0:(h // 2) * N + i0 + 256]
                            av_ps = avpp.tile([DH + 1, 256], F32, tag="av",
                                              name=f"av_{m}_{h}")
                            GSZ = 4
                            groups = [list(range(g, min(g + GSZ, njt)))
                                      for g in range(0, njt, GSZ)]
                            if len(groups[-1]) == GSZ:
                                # keep a spare exp column chunk for the
                                # null-key logits in the final group
                                groups[-1] = groups[-1][:GSZ - 1]
                                groups.append([njt - 1])
                            pend_av = None
                            av_state = [False]

                            def issue_av(pend, av_ps=av_ps, av_state=av_state):
                                pp4, pjts = pend
                                for ji, jt in enumerate(pjts):
                                    nc.tensor.matmul(
                                        out=av_ps[:, :],
                                        lhsT=v1[jt][:, :],
                                        rhs=pp4[:, ji * 256:ji * 256 + 256],
                                        start=(not av_state[0]), stop=False,
                                        skip_group_check=True)
                                    av_state[0] = True

                            for gi, jts in enumerate(groups):
                                gw = 256 * len(jts)
                                last = (gi == len(groups) - 1)
                                ps4 = simpp.tile([128, 1024], F32, tag="sim")
                                for ji, jt in enumerate(jts):
                                    j0 = jt * 128
                                    c0 = ji * 256
                                    # start=True only on the first chunk of
                                    # each 2KB psum bank: a start arms
                                    # zero-on-first-write for the whole bank
                                    nc.tensor.matmul(
                                        out=ps4[:, c0:c0 + 256],
                                        lhsT=kT[hp:hp + DH, j0:j0 + 128],
                                        rhs=qh,
                                        start=(c0 % 512 == 0), stop=False,
                                        skip_group_check=True)
                                # Toeplitz bias adds: two j-tiles merged per
                                # matmul via a 3D shifted AP (second touch of
                                # the armed bank, so plain accumulate)
                                for c0 in range(0, gw, 512):
                                    cn = min(2, (gw - c0) // 256)
                                    jt0 = jts[c0 // 256]
                                    u0 = 2048 - i0 + jt0 * 128
                                    mtr = mt[h][:, :].copy()
                                    pat = [list(p) for p in mtr.ap.to_list()]
                                    pat[1] = [128, cn]
                                    pat.append([-1, 256])
                                    mtr.ap = bass_rust.VecI64Pair(pat)
                                    mtr.offset = mtr.offset + u0
                                    nc.tensor.matmul(
                                        out=ps4[:, c0:c0 + cn * 256],
                                        lhsT=identb[:, :], rhs=mtr,
                                        start=False, stop=True,
                                        skip_group_check=True)
                                ew = gw
                                if last:
                                    # null-key logits ride along in the spare
                                    # columns of the final (partial) group
                                    nc.tensor.matmul(
                                        out=ps4[0:1, gw:gw + 256],
                                        lhsT=nkn[hp:hp + DH, :], rhs=qh,
                                        start=True, stop=True,
                                        skip_group_check=True)
                                    nc.vector.tensor_scalar_add(
                                        out=ps4[0:1, gw:gw + 256],
                                        in0=ps4[0:1, gw:gw + 256],
                                        scalar1=nb_sb[:, h:h + 1])
                                    ew = gw + 256
                                p4 = atp.tile([128, 1024], BF16, tag="p4")
                                nc.scalar.activation(
                                    out=p4[:, 0:ew], in_=ps4[:, 0:ew],
                                    func=mybir.ActivationFunctionType.Exp)
                                # software pipeline: issue deferred work now so
                                # the PE queue never parks waiting on this exp
                                if gi == 0:
                                    run_tail()
                                else:
                                    issue_av(pend_av)
                                pend_av = (p4, jts)

                            def tail(h=h, i0=i0, av_ps=av_ps, pend_av=pend_av,
                                     issue_av=issue_av,
                                     gw_last=256 * len(groups[-1])):
                                issue_av(pend_av)
                                nc.tensor.matmul(
                                    out=av_ps[:, :],
                                    lhsT=nv1[:, :],
                                    rhs=pend_av[0][0:1, gw_last:gw_last + 256],
                                    start=False, stop=True,
                                    skip_group_check=True)
                                # normalize columns by row-64 sums -> avT[h].
                                # full-height recip: base-partition-64 DVE
                                # slices silently no-op; only row 64 is read
                                # by the selector matmul below
                                rr = atp.tile([DH + 1, 256], F32, tag="rr")
                                nc.vector.reciprocal_approx_fast(
                                    out=rr[:, :], in_=av_ps[:, :])
                                psb = tpp.tile([DH, 256], F32, tag="bc")
                                nc.tensor.matmul(out=psb[:, :],
                                                 lhsT=ones65[DH:DH + 1, 0:DH],
                                                 rhs=rr[DH:DH + 1, :],
                                                 start=True, stop=True)
                                rb = atp.tile([DH, 256], BF16, tag="rb")
                                nc.scalar.copy(out=rb[:, :], in_=psb[:, :])
                                nc.vector.tensor_tensor(
                                    out=avT[h][:, i0:i0 + 256],
                                    in0=av_ps[0:DH, :], in1=rb[:, :],
                                    op=mybir.AluOpType.mult)
                            pend_tail[0] = tail

                    # out projection for this quarter
                    run_tail()
                    for tl in range(4):
                        tt = q * 4 + tl
                        ps_po = simpp.tile([128, 1024], F32, tag="sim")
                        for eb in range(2):
                            for ch in range(H_LOC):
                                nc.tensor.matmul(
                                    out=ps_po[:, eb * 512:eb * 512 + 512],
                                    lhsT=avT[ch][:, tt * 128:tt * 128 + 128],
                                    rhs=wout_sb[:, ch * D + eb * 512:
                                                ch * D + eb * 512 + 512],
                                    start=(ch == 0), stop=(ch == H_LOC - 1),
                                    skip_group_check=True)
                        po_sb = oqp.tile([128, D], BF16, tag="po")
                        nc.vector.tensor_copy(out=po_sb[:, :], in_=ps_po[:, :])
                        dma(out=po_q[q].ap()[tl * 128:(tl + 1) * 128, :],
                            in_=po_sb[:, :])
                    nc.gpsimd.collective_compute(
                        "ReduceScatter", mybir.AluOpType.add,
                        replica_groups=[[0, 1, 2, 3], [4, 5, 6, 7]],
                        ins=[po_q[q].ap().opt()],
                        outs=[rs_q[q].ap().opt()],
                    )
                    # final LN for the 128 owned rows of this quarter
                    y = oqp.tile([128, D], BF16, tag="y")
                    dma(out=y[:, :], in_=rs_q[q].ap()[:, :])
                    yscr = oqp.tile([128, D], BF16, tag="yscr")
                    mean, var, sd, rstd = _ln_stats(nc, oqp, y[:, :], D,
                                                    yscr[:, :])
                    nc.scalar.activation(out=sd[:, :], in_=var[:, :],
                                         func=mybir.ActivationFunctionType.Sqrt,
                                         bias=eps128[:, :])
                    nc.vector.reciprocal_approx_fast(out=rstd[:, :], in_=sd[:, :])
                    yf = oqp.tile([128, D], F32, tag="yf")
                    nc.vector.tensor_scalar(out=yf[:, :], in0=y[:, :],
                                            scalar1=mean[:, :],
                                            scalar2=rstd[:, :],
                                            op0=mybir.AluOpType.subtract,
                                            op1=mybir.AluOpType.mult)
                    nc.vector.tensor_tensor(out=yf[:, :], in0=yf[:, :],
                                            in1=goutb,
                                            op=mybir.AluOpType.mult)
                    dma(out=out_p.ap()[q * 128:(q + 1) * 128, :], in_=yf[:, :])
                if DBG:
                    dma(out=dbg_avT.ap(), in_=avT[0][:, :])
                    dma(out=dbg_mt.ap(), in_=mt[0][:, :])
                    dma(out=dbg_po.ap(), in_=po_q[0].ap()[0:128, :])

    nc.compile()
    return nc


def make_in_maps(x, mask, g_norm, Wq, Wkv, q_scale, k_scale, null_kv,
                 null_attn_bias, w0, b0, g0, w1, b1, g1, w2, b2, Wout, g_out):
    import ml_dtypes
    assert bool(np.asarray(mask).all()), "kernel assumes all-True mask"
    f = np.float32
    bf = ml_dtypes.bfloat16
    gn = np.asarray(g_norm, f)
    in_maps = []
    for c in range(N_CORES):
        bi, hg = c // 4, c % 4
        posb = (2047 - c * MLP_ROWS - np.arange(128)).astype(np.int32)
        wq_hg = np.asarray(Wq, f)[:, hg * INNER_LOC:(hg + 1) * INNER_LOC]
        wqkv = np.concatenate(
            [wq_hg, np.asarray(Wkv, f)], axis=1) * gn[:, None]
        wsum = -(wqkv.sum(axis=0)) / float(D)
        m = {
            "x": np.ascontiguousarray(np.asarray(x, f)[bi]).astype(bf),
            "wqkv": np.ascontiguousarray(wqkv).astype(bf),
            "wsum": np.ascontiguousarray(wsum.astype(f)),
            "qs8ks": np.ascontiguousarray(np.tile(
                8.0 * np.asarray(q_scale, f) * np.asarray(k_scale, f), 2)),
            "nkv": np.ascontiguousarray(np.tile(np.asarray(null_kv, f),
                                                (1, 2))),
            "nb": np.ascontiguousarray(
                np.asarray(null_attn_bias, f)[hg * H_LOC:(hg + 1) * H_LOC]),
            "w0v": np.ascontiguousarray(np.asarray(w0, f).reshape(D)),
            "b0": np.ascontiguousarray(np.asarray(b0, f)),
            "g0": np.ascontiguousarray(np.asarray(g0, f)),
            "w1": np.ascontiguousarray(np.asarray(w1, f)).astype(bf),
            "b1": np.ascontiguousarray(np.asarray(b1, f)),
            "g1": np.ascontiguousarray(np.asarray(g1, f)),
            "w2": np.ascontiguousarray(np.asarray(w2, f)).astype(bf),
            "b2": np.ascontiguousarray(np.asarray(b2, f)),
            "wout": np.ascontiguousarray(
                np.asarray(Wout, f)[hg * INNER_LOC:(hg + 1) * INNER_LOC, :]
            ).astype(bf),
            "g_out": np.ascontiguousarray(np.asarray(g_out, f)),
            "posb": posb,
            "sel": np.eye(HEADS, dtype=f)[:, hg * H_LOC:(hg + 1) * H_LOC].copy(),
        }
        in_maps.append(m)
    return in_maps


_NC_CACHE = None


def kernel(**inputs):
    global _NC_CACHE
    from concourse.bass_utils import run_bass_kernel_spmd

    if _NC_CACHE is None:
        _NC_CACHE = build()
    nc = _NC_CACHE
    in_maps = make_in_maps(**inputs)
    res = run_bass_kernel_spmd(nc, in_maps, core_ids=list(range(N_CORES)))
    outs = res.results
    kernel.last_outs = outs
    full = np.empty((2, N, D), np.float32)
    for c in range(N_CORES):
        bi, hg = c // 4, c % 4
        o = np.asarray(outs[c]["out"], np.float32)
        for q in range(4):
            full[bi, q * 512 + hg * 128:q * 512 + (hg + 1) * 128, :] = \
                o[q * 128:(q + 1) * 128]
    return full


# revision 51
# speedup vs baseline: 1.3999x; 1.0156x over previous
import sys

for _p in ("/opt/trn_rl_repo", "/root/.axon_site/_ro/trn_rl_repo"):
    if _p not in sys.path:
        sys.path.insert(0, _p)

import numpy as np

from concourse import bacc, mybir, tile
import bass_rust

N_CORES = 8
N = 2048
D = 1024
HEADS = 16
DH = 64
H_LOC = 4          # heads per core
INNER_LOC = H_LOC * DH  # 256
QKV = INNER_LOC + 2 * DH  # 384 packed projection width
NEG = -1.0e30
EPS = 1e-5
F32 = mybir.dt.float32
F32R = mybir.dt.float32r
BF16 = mybir.dt.bfloat16
I32 = mybir.dt.int32

# rel-pos MLP sharding: 2048 useful reversed-position rows, 256 per core.
MLP_ROWS = 256
HFR_PAD = 64       # data lives at HFRD[64 : 64+2048]
HFRD_ROWS = 2752   # covers all reads [64, 2494]
MT_W = 2560        # master toeplitz width


def _ap(t, pattern, offset):
    a = t.ap().copy()
    a.ap = bass_rust.VecI64Pair(pattern)
    a.offset = offset
    return a


def _ln_stats(nc, pool, x_sb, width):
    """Row LayerNorm stats via one-pass bn_stats.

    Returns (mean, var, sd, rstd) where mean/var are slices of the bn_aggr
    output and sd/rstd are empty tiles for the caller's sqrt/recip."""
    nch = (width + 511) // 512
    stats = pool.tile([128, 6 * nch], F32, tag="ln_bns")
    for c in range(nch):
        w = min(512, width - c * 512)
        nc.vector.bn_stats(out=stats[:, c * 6:(c + 1) * 6],
                           in_=x_sb[:, c * 512:c * 512 + w])
    mv = pool.tile([128, 2], F32, tag="ln_mv")
    nc.vector.bn_aggr(out=mv[:, :], in_=stats[:, :])
    sd = pool.tile([128, 1], F32, tag="ln_sd")
    rstd = pool.tile([128, 1], F32, tag="ln_rstd")
    return mv[:, 0:1], mv[:, 1:2], sd, rstd


def build():
    build.NO_AV = globals().get('NO_AV', False)
    nc = bacc.Bacc("TRN2", target_bir_lowering=False, debug=False,
                   num_devices=N_CORES)

    # ---------------- parameters ----------------
    x_p = nc.declare_dram_parameter("x", [N, D], BF16, isOutput=False)
    wqkv_p = nc.declare_dram_parameter("wqkv", [D, QKV], BF16, isOutput=False)
    wsum_p = nc.declare_dram_parameter("wsum", [QKV], F32, isOutput=False)
    # qs8ks and null-k are passed partition-duplicated ([x | x] over 128
    # partitions) so odd heads can slice base-partition-64 operands.
    qs8ks_p = nc.declare_dram_parameter("qs8ks", [128], F32, isOutput=False)
    nkv_p = nc.declare_dram_parameter("nkv", [2, 128], F32, isOutput=False)
    nb_p = nc.declare_dram_parameter("nb", [H_LOC], F32, isOutput=False)
    w0_p = nc.declare_dram_parameter("w0v", [D], F32, isOutput=False)
    b0_p = nc.declare_dram_parameter("b0", [D], F32, isOutput=False)
    g0_p = nc.declare_dram_parameter("g0", [D], F32, isOutput=False)
    w1_p = nc.declare_dram_parameter("w1", [D, D], BF16, isOutput=False)
    b1_p = nc.declare_dram_parameter("b1", [D], F32, isOutput=False)
    g1_p = nc.declare_dram_parameter("g1", [D], F32, isOutput=False)
    w2_p = nc.declare_dram_parameter("w2", [D, HEADS], BF16, isOutput=False)
    b2_p = nc.declare_dram_parameter("b2", [HEADS], F32, isOutput=False)
    wout_p = nc.declare_dram_parameter("wout", [INNER_LOC, D], BF16,
                                       isOutput=False)
    gout_p = nc.declare_dram_parameter("g_out", [D], F32, isOutput=False)
    posb_p = nc.declare_dram_parameter("posb", [128], I32, isOutput=False)
    sel_p = nc.declare_dram_parameter("sel", [HEADS, H_LOC], F32, isOutput=False)
    out_p = nc.declare_dram_parameter("out", [N // 4, D], F32, isOutput=True)
    DBG = globals().get('DEBUG_TAPS', False)
    if DBG:
        dbg_qT = nc.declare_dram_parameter("dbg_qT", [128, N], BF16, isOutput=True)
        dbg_kT = nc.declare_dram_parameter("dbg_kT", [128, N], BF16, isOutput=True)
        dbg_v0 = nc.declare_dram_parameter("dbg_v0", [128, DH + 1], BF16, isOutput=True)
        dbg_avT = nc.declare_dram_parameter("dbg_avT", [DH, N], BF16, isOutput=True)
        dbg_po = nc.declare_dram_parameter("dbg_po", [128, D], BF16, isOutput=True)
        dbg_mt = nc.declare_dram_parameter("dbg_mt", [128, MT_W], BF16, isOutput=True)
        dbg_avps = nc.declare_dram_parameter("dbg_avps", [DH + 1, N], F32, isOutput=True)
        dbg_rb = nc.declare_dram_parameter("dbg_rb", [DH, N], BF16, isOutput=True)
        dbg_p4 = nc.declare_dram_parameter("dbg_p4", [128, 1024], BF16, isOutput=True)
        dbg_ps4 = nc.declare_dram_parameter("dbg_ps4", [128, 1024], F32, isOutput=True)

    # ---------------- internal DRAM ----------------
    hfr_loc = nc.dram_tensor("hfr_loc", [MLP_ROWS, HEADS], F32)
    hfr_g = nc.dram_tensor("hfr_g", [8 * MLP_ROWS, HEADS], F32)
    hfrd = nc.dram_tensor("hfrd", [H_LOC, HFRD_ROWS], BF16)
    po_q = [nc.dram_tensor(f"po_{i}", [N // 4, D], BF16) for i in range(4)]
    rs_q = [nc.dram_tensor(f"rs_{i}", [N // 16, D], BF16) for i in range(4)]

    dma = nc.sync.dma_start

    with tile.TileContext(nc) as tc:
        with (
            tc.tile_pool(name="const", bufs=1) as constp,
            tc.tile_pool(name="pers", bufs=1) as pers,
            tc.tile_pool(name="weights", bufs=1) as wp,
        ):
            ident = constp.tile([128, 128], F32)
            from concourse.masks import make_identity
            make_identity(nc, ident[:, :])
            identb = constp.tile([128, 128], BF16)
            nc.scalar.copy(out=identb[:, :], in_=ident[:, :])
            eps128 = constp.tile([128, 1], F32)
            nc.vector.memset(eps128[:, :], EPS)
            ones1 = constp.tile([1, 128], F32)
            nc.vector.memset(ones1[:, :], 1.0)

            # ---- all big input loads issued up front (single SP queue) ----
            early = tc.tile_pool(name="early", bufs=1)
            ep = early.__enter__()
            xall = ep.tile([128, 16 * D], BF16, name="xall")
            dma(out=xall[:, :], in_=_ap(x_p, [[D, 128], [128 * D, 16], [1, D]], 0))
            wqkv_sb = ep.tile([128, 8 * QKV], BF16)
            dma(out=wqkv_sb[:, :],
                in_=_ap(wqkv_p, [[QKV, 128], [128 * QKV, 8], [1, QKV]], 0))
            wout_sb = wp.tile([DH, 4 * D], BF16)
            dma(out=wout_sb[:, :],
                in_=_ap(wout_p, [[D, DH], [DH * D, 4], [1, D]], 0))
            w1_sb = ep.tile([128, 8 * D], BF16)  # chunk k at cols k*1024
            dma(out=w1_sb[:, :],
                in_=_ap(w1_p, [[D, 128], [128 * D, 8], [1, D]], 0))
            w2_sb = ep.tile([128, 8 * HEADS], BF16)
            dma(out=w2_sb[:, :],
                in_=_ap(w2_p, [[HEADS, 128], [128 * HEADS, 8], [1, HEADS]], 0))

            wsum_f32 = ep.tile([1, QKV], F32)
            dma(out=wsum_f32[:, :], in_=_ap(wsum_p, [[QKV, 1], [1, QKV]], 0))
            wsum_row = ep.tile([1, QKV], BF16)
            nc.vector.tensor_copy(out=wsum_row[:, :], in_=wsum_f32[:, :])
            qs8ks_sb = pers.tile([128, 1], F32)
            dma(out=qs8ks_sb[:, :], in_=_ap(qs8ks_p, [[1, 128], [1, 1]], 0))
            nkT = pers.tile([128, 1], F32)
            dma(out=nkT[:, :], in_=_ap(nkv_p, [[1, 128], [1, 1]], 0))
            nv_sb = pers.tile([1, DH], F32)
            dma(out=nv_sb[:, :], in_=nkv_p.ap()[1:2, 0:DH])
            nb_sb = pers.tile([1, H_LOC], F32)
            dma(out=nb_sb[:, :], in_=_ap(nb_p, [[H_LOC, 1], [1, H_LOC]], 0))
            posi_t = pers.tile([128, 1], I32, name="posi")
            dma(out=posi_t[:, :], in_=_ap(posb_p, [[1, 128], [1, 1]], 0))
            sel_sb = wp.tile([HEADS, H_LOC], F32)
            dma(out=sel_sb[:, :], in_=sel_p.ap())

            # [nv | 1] bf16 row for null-key AV
            nv1 = pers.tile([1, DH + 1], BF16)
            nc.vector.tensor_copy(out=nv1[:, 0:DH], in_=nv_sb[:, :])
            nc.vector.memset(nv1[:, DH:DH + 1], 1.0)

            # ---------- Phases 0-2 interleaved: rel-pos MLP generator is ----
            # pumped between projection steps so its long serial chain fills
            # engine gaps instead of blocking the in-order queues.
            qT2 = pers.tile([128, 2 * N], BF16, name="qT2")
            kT = pers.tile([128, N], BF16, tag="kT", name="kT")
            v1 = [pers.tile([128, DH + 1], BF16, tag=f"v1_{j}", name=f"v1_{j}")
                  for j in range(16)]
            nkn = pers.tile([128, 1], BF16)

            with (
                tc.tile_pool(name="bc_ps", bufs=2, space="PSUM") as bpp,
                tc.tile_pool(name="vst", bufs=3) as vstp,
            ):
                bcast = ep.tile([128, 4 * D + HEADS], BF16)
                g1b = ep.tile([128, D], BF16)
                goutb = wp.tile([128, D], F32)
                chunks = []
                for pi, par in enumerate((w0_p, b0_p, g0_p, b1_p)):
                    for half in range(2):
                        chunks.append((par, half * 512, 512, bcast,
                                       pi * D + half * 512))
                chunks.append((b2_p, 0, HEADS, bcast, 4 * D))
                for half in range(2):
                    chunks.append((g1_p, half * 512, 512, g1b, half * 512))
                    chunks.append((gout_p, half * 512, 512, goutb,
                                   half * 512))
                for par, poff, wdt, dst, doff in chunks:
                    vstage = vstp.tile([1, 512], F32, tag="vstage")
                    dma(out=vstage[:, 0:wdt],
                        in_=_ap(par, [[wdt, 1], [1, wdt]], poff))
                    ps = bpp.tile([128, 512], F32, tag="bc")
                    nc.tensor.matmul(out=ps[:, 0:wdt],
                                     lhsT=ones1[:, :],
                                     rhs=vstage[:, 0:wdt],
                                     start=True, stop=True)
                    nc.scalar.copy(out=dst[:, doff:doff + wdt],
                                   in_=ps[:, 0:wdt])
            w0b = bcast[:, 0:D]
            b0b = bcast[:, D:2 * D]
            g0b = bcast[:, 2 * D:3 * D]
            b1b = bcast[:, 3 * D:4 * D]
            b2b = bcast[:, 4 * D:4 * D + HEADS]

            with (
                tc.tile_pool(name="mlp", bufs=1) as mp,
                tc.tile_pool(name="mlp_ps", bufs=1, space="PSUM") as mpp,
                tc.tile_pool(name="mlp_ps2", bufs=1, space="PSUM") as mpp2,
                tc.tile_pool(name="xT", bufs=1) as xTp,
                tc.tile_pool(name="xt", bufs=2) as xtp,
                tc.tile_pool(name="xps", bufs=2, space="PSUM") as xpp,
                tc.tile_pool(name="xps2", bufs=1, space="PSUM") as xpp2,
                tc.tile_pool(name="xps3", bufs=1, space="PSUM") as xpp3,
            ):
                xT = xTp.tile([128, 8 * N], BF16)  # d-chunk k at cols k*2048
                negrow = wp.tile([1, N], BF16, name="negrow")

                def mlp_gen():
                    for t in range(2):
                        m0 = t * 128
                        posf = mp.tile([128, 1], F32, tag="posf",
                                       name=f"posf{t}")
                        nc.vector.tensor_scalar_add(out=posf[:, :],
                                                    in0=posi_t[:, :],
                                                    scalar1=float(-m0))
                        h0 = mp.tile([128, D], F32, tag="h0", name=f"h0_{t}")
                        nc.vector.tensor_scalar(out=h0[:, :], in0=w0b,
                                                scalar1=posf[:, :], scalar2=None,
                                                op0=mybir.AluOpType.mult)
                        nc.vector.tensor_tensor(out=h0[:, :], in0=h0[:, :],
                                                in1=b0b,
                                                op=mybir.AluOpType.add)
                        yield
                        mean, var, sd, rstd = _ln_stats(nc, mp, h0[:, :], D)
                        nc.scalar.activation(
                            out=sd[:, :], in_=var,
                            func=mybir.ActivationFunctionType.Sqrt,
                            bias=eps128[:, :])
                        nc.vector.reciprocal_approx_fast(out=rstd[:, :],
                                                         in_=sd[:, :])
                        yield
                        nc.vector.tensor_scalar(out=h0[:, :], in0=h0[:, :],
                                                scalar1=mean,
                                                scalar2=rstd[:, :],
                                                op0=mybir.AluOpType.subtract,
                                                op1=mybir.AluOpType.mult)
                        nc.vector.tensor_tensor(out=h0[:, :], in0=h0[:, :],
                                                in1=g0b,
                                                op=mybir.AluOpType.mult)
                        h0b = mp.tile([128, D], BF16, tag="h0b",
                                      name=f"h0b{t}")
                        nc.scalar.activation(
                            out=h0b[:, :], in_=h0[:, :],
                            func=mybir.ActivationFunctionType.Silu)
                        yield
                        h0T = mp.tile([128, D], BF16, tag="h0T",
                                      name=f"h0T{t}")
                        pst8m = mpp2.tile([128, D], BF16, tag="tp",
                                          name=f"tp0_{t}")
                        for k in range(8):
                            nc.tensor.matmul(
                                out=pst8m[:, k * 128:(k + 1) * 128],
                                lhsT=h0b[:, k * 128:(k + 1) * 128],
                                rhs=identb[:, :], is_transpose=True,
                                start=True, stop=True)
                            if k == 3:
                                yield
                        nc.vector.tensor_copy(out=h0T[:, :], in_=pst8m[:, :])
                        yield
                        h1 = mp.tile([128, D], F32, tag="h1", name=f"h1_{t}")
                        for eb in range(2):
                            ps = mpp.tile([128, 512], F32, tag="h1ps",
                                          name=f"h1ps{t}_{eb}")
                            for k in range(8):
                                nc.tensor.matmul(
                                    out=ps[:, :],
                                    lhsT=h0T[:, k * 128:(k + 1) * 128],
                                    rhs=w1_sb[:, k * D + eb * 512:
                                              k * D + eb * 512 + 512],
                                    start=(k == 0), stop=(k == 7))
                            nc.vector.tensor_tensor(
                                out=h1[:, eb * 512:eb * 512 + 512],
                                in0=ps[:, :],
                                in1=b1b[:, eb * 512:eb * 512 + 512],
                                op=mybir.AluOpType.add)
                            yield
                        mean, var, sd, rstd = _ln_stats(nc, mp, h1[:, :], D)
                        nc.scalar.activation(
                            out=sd[:, :], in_=var,
                            func=mybir.ActivationFunctionType.Sqrt,
                            bias=eps128[:, :])
                        nc.vector.reciprocal_approx_fast(out=rstd[:, :],
                                                         in_=sd[:, :])
                        yield
                        nc.vector.tensor_scalar(out=h1[:, :], in0=h1[:, :],
                                                scalar1=mean,
                                                scalar2=rstd[:, :],
                                                op0=mybir.AluOpType.subtract,
                                                op1=mybir.AluOpType.mult)
                        nc.vector.tensor_tensor(out=h1[:, :], in0=h1[:, :],
                                                in1=g1b,
                                                op=mybir.AluOpType.mult)
                        h1b = mp.tile([128, D], BF16, tag="h1b",
                                      name=f"h1b{t}")
                        nc.scalar.activation(
                            out=h1b[:, :], in_=h1[:, :],
                            func=mybir.ActivationFunctionType.Silu)
                        yield
                        h1T = mp.tile([128, D], BF16, tag="h1T",
                                      name=f"h1T{t}")
                        pst8n = mpp2.tile([128, D], BF16, tag="tp",
                                          name=f"tp1_{t}")
                        for k in range(8):
                            nc.tensor.matmul(
                                out=pst8n[:, k * 128:(k + 1) * 128],
                                lhsT=h1b[:, k * 128:(k + 1) * 128],
                                rhs=identb[:, :], is_transpose=True,
                                start=True, stop=True)
                            if k == 3:
                                yield
                        nc.vector.tensor_copy(out=h1T[:, :], in_=pst8n[:, :])
                        yield
                        psf = mpp2.tile([128, HEADS], F32, tag="hf",
                                        name=f"hf{t}")
                        for k in range(8):
                            nc.tensor.matmul(
                                out=psf[:, :],
                                lhsT=h1T[:, k * 128:(k + 1) * 128],
                                rhs=w2_sb[:, k * HEADS:(k + 1) * HEADS],
                                start=(k == 0), stop=(k == 7))
                        hfc = mp.tile([128, HEADS], F32, tag="hfc",
                                      name=f"hfc{t}")
                        nc.vector.tensor_tensor(out=hfc[:, :], in0=psf[:, :],
                                                in1=b2b,
                                                op=mybir.AluOpType.add)
                        dma(out=hfr_loc.ap()[m0:m0 + 128, :], in_=hfc[:, :])
                        yield
                    nc.gpsimd.collective_compute(
                        "AllGather", mybir.AluOpType.bypass,
                        replica_groups=[list(range(N_CORES))],
                        ins=[hfr_loc.ap().opt()],
                        outs=[hfr_g.ap().opt()],
                    )

                mgen = mlp_gen()

                def pump(n=1):
                    for _ in range(n):
                        try:
                            next(mgen)
                        except StopIteration:
                            return

                for tt in range(16):
                    xs = xall[:, tt * D:(tt + 1) * D]
                    # stats (one-pass bn_stats; wsum carries -colsum so the
                    # rank-1 correction uses the mean row directly)
                    mean, var, sd, rstd = _ln_stats(nc, xtp, xs, D)
                    nc.scalar.activation(out=sd[:, :], in_=var,
                                         func=mybir.ActivationFunctionType.Sqrt,
                                         bias=eps128[:, :])
                    nc.vector.reciprocal_approx_fast(out=rstd[:, :],
                                                     in_=sd[:, :])
                    psr = xpp3.tile([128, 128], F32, tag="misc",
                                    name=f"psr{tt}")
                    nc.tensor.matmul(out=psr[0:1, :], lhsT=mean,
                                     rhs=ident[:, :], is_transpose=True,
                                     start=True, stop=True)
                    nc.vector.tensor_copy(out=negrow[:, tt * 128:tt * 128 + 128],
                                          in_=psr[0:1, :])
                    pump(1)
                    # x transposes (bf16): 8 into one psum tile, one fat copy
                    pst8 = xpp2.tile([128, D], BF16, tag="tp8",
                                     name=f"pst8_{tt}")
                    for k in range(8):
                        nc.tensor.matmul(out=pst8[:, k * 128:(k + 1) * 128],
                                         lhsT=xall[:, tt * D + k * 128:
                                                   tt * D + k * 128 + 128],
                                         rhs=identb[:, :], is_transpose=True,
                                         start=True, stop=True)
                    xTo = xT[:, :].copy()
                    xpat = [list(p) for p in xTo.ap.to_list()]
                    xpat = [xpat[0], [N, 8], [1, 128]]
                    xTo.ap = bass_rust.VecI64Pair(xpat)
                    xTo.offset = xTo.offset + tt * 128
                    nc.vector.tensor_copy(out=xTo, in_=pst8[:, :])
                    pump(1)
                    # packed q|k|v projection with rank-1 mean correction
                    psq = xpp.tile([128, QKV], F32, tag="qkv")
                    for k in range(8):
                        nc.tensor.matmul(
                            out=psq[:, :],
                            lhsT=xT[:, k * N + tt * 128:k * N + tt * 128 + 128],
                            rhs=wqkv_sb[:, k * QKV:(k + 1) * QKV],
                            start=(k == 0), stop=False,
                            skip_group_check=True)
                    nc.tensor.matmul(out=psq[:, :],
                                     lhsT=negrow[:, tt * 128:tt * 128 + 128],
                                     rhs=wsum_row[:, :],
                                     start=False, stop=True,
                                     skip_group_check=True)
                    # per-head l2 norms (4 q heads + k)
                    nrm = xtp.tile([128, 8], F32, tag="nrm")
                    scr2 = xtp.tile([128, DH], BF16, tag="scr2")
                    for j in range(5):
                        nc.scalar.activation(
                            out=scr2[:, :],
                            in_=psq[:, j * DH:(j + 1) * DH],
                            func=mybir.ActivationFunctionType.Square,
                            accum_out=nrm[:, j:j + 1])
                    sd5 = xtp.tile([128, 8], F32, tag="sd5")
                    rinv = xtp.tile([128, 8], F32, tag="rinv")
                    nc.scalar.activation(out=sd5[:, 0:5], in_=nrm[:, 0:5],
                                         func=mybir.ActivationFunctionType.Sqrt)
                    nc.vector.reciprocal(out=rinv[:, 0:5], in_=sd5[:, 0:5])
                    pump(1)
                    # scaled copies out of PSUM
                    qn = xtp.tile([128, INNER_LOC], BF16, tag="qn")
                    for h in range(4):
                        eng = nc.vector if h % 2 == 0 else None
                        if h % 2 == 0:
                            nc.vector.tensor_scalar(
                                out=qn[:, h * DH:(h + 1) * DH],
                                in0=psq[:, h * DH:(h + 1) * DH],
                                scalar1=rinv[:, h:h + 1], scalar2=None,
                                op0=mybir.AluOpType.mult)
                        else:
                            nc.scalar.activation(
                                out=qn[:, h * DH:(h + 1) * DH],
                                in_=psq[:, h * DH:(h + 1) * DH],
                                func=mybir.ActivationFunctionType.Copy,
                                scale=rinv[:, h:h + 1])
                    # kn duplicated into both column halves so the transpose
                    # yields kT stacked twice along partitions
                    kn = xtp.tile([128, 128], BF16, tag="kn")
                    for kh in range(2):
                        nc.vector.tensor_scalar(
                            out=kn[:, kh * DH:(kh + 1) * DH],
                            in0=psq[:, INNER_LOC:INNER_LOC + DH],
                            scalar1=rinv[:, 4:5], scalar2=None,
                            op0=mybir.AluOpType.mult)
                    nc.vector.tensor_scalar(out=v1[tt][:, 0:DH],
                                            in0=psq[:, INNER_LOC + DH:QKV],
                                            scalar1=rstd[:, :], scalar2=None,
                                            op0=mybir.AluOpType.mult)
                    nc.vector.memset(v1[tt][:, DH:DH + 1], 1.0)
                    # q pair + k transposes into one psum tile
                    pstqk = xpp2.tile([128, 384], BF16, tag="tpqk",
                                      name=f"pstqk{tt}")
                    for p in range(2):
                        nc.tensor.matmul(out=pstqk[:, p * 128:(p + 1) * 128],
                                         lhsT=qn[:, p * 128:(p + 1) * 128],
                                         rhs=identb[:, :], is_transpose=True,
                                         start=True, stop=True)
                    nc.tensor.matmul(out=pstqk[:, 256:384], lhsT=kn[:, :],
                                     rhs=identb[:, :], is_transpose=True,
                                     start=True, stop=True)
                    qTo = qT2[:, :].copy()
                    qpat = [list(p) for p in qTo.ap.to_list()]
                    qpat = [qpat[0], [N, 2], [1, 128]]
                    qTo.ap = bass_rust.VecI64Pair(qpat)
                    qTo.offset = qTo.offset + tt * 128
                    nc.scalar.copy(out=qTo, in_=pstqk[:, 0:256])
                    # k transpose with qs8ks scale folded in
                    nc.vector.tensor_scalar(out=kT[:, tt * 128:tt * 128 + 128],
                                            in0=pstqk[:, 256:384],
                                            scalar1=qs8ks_sb[:, :], scalar2=None,
                                            op0=mybir.AluOpType.mult)
                    pump(1)

                pump(100)

                if DBG:
                    dma(out=dbg_qT.ap(), in_=qT2[:, 0:N])
                    dma(out=dbg_kT.ap(), in_=kT[:, :])
                    dma(out=dbg_v0.ap(), in_=v1[0][:, :])

                # null key normalize: nkn = l2norm(nk) * qs8ks  (dup over 128)
                ones64c_f = constp.tile([DH, 1], F32)
                nc.vector.memset(ones64c_f[:, :], 1.0)
                nsq = xtp.tile([128, 1], F32, tag="nsq")
                nc.scalar.activation(out=nsq[:, :], in_=nkT[:, :],
                                     func=mybir.ActivationFunctionType.Square)
                psn1 = xpp3.tile([128, 128], F32, tag="misc", name="psn1")
                nc.tensor.matmul(out=psn1[0:1, 0:1], lhsT=ones64c_f[:, :],
                                 rhs=nsq[0:DH, :], start=True, stop=True)
                rn1 = xtp.tile([1, 1], F32, tag="rn1")
                nc.scalar.activation(out=rn1[:, :], in_=psn1[0:1, 0:1],
                                     func=mybir.ActivationFunctionType.Sqrt)
                with nc.allow_low_precision(reason="f32r same bits as f32"):
                    nc.vector.reciprocal(out=rn1[:, :], in_=rn1[:, :])
                psb1 = xpp3.tile([128, 128], F32, tag="misc", name="psb1")
                nc.tensor.matmul(out=psb1[:, 0:1], lhsT=ones1[:, :],
                                 rhs=rn1[:, :], start=True, stop=True)
                nc.vector.tensor_tensor(out=nkn[:, :], in0=nkT[:, :],
                                        in1=psb1[:, 0:1],
                                        op=mybir.AluOpType.mult)
                nc.vector.tensor_scalar(out=nkn[:, :], in0=nkn[:, :],
                                        scalar1=qs8ks_sb[:, :], scalar2=None,
                                        op0=mybir.AluOpType.mult)

                # ---- stage AllGathered MLP rows -> hfrd (batched) ----
                stg = xtp.tile([128, 16 * HEADS], F32, tag="stg",
                               name="stg_all")
                dma(out=stg[:, :],
                    in_=_ap(hfr_g, [[HEADS, 128], [128 * HEADS, 16],
                                    [1, HEADS]], 0))
                stgT = xTp.tile([HEADS, 16 * 128], F32, name="stgT")
                for chunk in range(16):
                    pss = xpp.tile([128, QKV], F32, tag="qkv",
                                   name=f"stgps{chunk}")
                    nc.tensor.matmul(out=pss[0:HEADS, 0:128],
                                     lhsT=stg[:, chunk * HEADS:
                                              (chunk + 1) * HEADS],
                                     rhs=ident[:, :], is_transpose=True,
                                     start=True, stop=True)
                    eng = nc.scalar if chunk % 2 == 0 else nc.vector
                    if chunk % 2 == 0:
                        nc.scalar.copy(
                            out=stgT[:, chunk * 128:(chunk + 1) * 128],
                            in_=pss[0:HEADS, 0:128])
                    else:
                        nc.vector.tensor_copy(
                            out=stgT[:, chunk * 128:(chunk + 1) * 128],
                            in_=pss[0:HEADS, 0:128])
                # select local heads and write hfrd in 512-col pieces
                for piece in range(4):
                    psl = xpp3.tile([128, 128], F32, tag="misc",
                                    name=f"psl{piece}")
                    stl = xtp.tile([H_LOC, 512], BF16, tag="stl")
                    for sub in range(4):
                        col = piece * 512 + sub * 128
                        nc.tensor.matmul(out=psl[0:H_LOC, 0:128],
                                         lhsT=sel_sb[:, :],
                                         rhs=stgT[:, col:col + 128],
                                         start=True, stop=True)
                        if sub % 2 == 0:
                            nc.scalar.copy(out=stl[:, sub * 128:sub * 128 + 128],
                                           in_=psl[0:H_LOC, 0:128])
                        else:
                            nc.vector.tensor_copy(
                                out=stl[:, sub * 128:sub * 128 + 128],
                                in_=psl[0:H_LOC, 0:128])
                    dma(out=_ap(hfrd, [[HFRD_ROWS, H_LOC], [1, 512]],
                                HFR_PAD + piece * 512),
                        in_=stl[:, :])
                poison = xtp.tile([H_LOC, HFRD_ROWS - 2112], BF16,
                                  name="poison")
                nc.vector.memset(poison[:, :], NEG)
                dma(out=_ap(hfrd, [[HFRD_ROWS, H_LOC],
                                   [1, HFRD_ROWS - 2112]], 2112),
                    in_=poison[:, :])

            early.__exit__(None, None, None)

            # ---------- Phase 3: attention + per-quarter out-proj + RS ------
            avT = [pers.tile([DH, N], BF16, tag=f"avT{h}", name=f"avT{h}")
                   for h in range(H_LOC)]
            mt = [pers.tile([128, MT_W], BF16, tag=f"mt{h}", name=f"mt{h}")
                  for h in range(H_LOC)]
            for h in range(H_LOC):
                dma(out=mt[h][:, :],
                    in_=_ap(hfrd, [[1, 128], [1, MT_W]], h * HFRD_ROWS + 63))
            if build.NO_AV:
                for h in range(H_LOC):
                    nc.vector.memset(avT[h][:, :], 0.0)

            with (
                tc.tile_pool(name="at", bufs=3) as atp,
                tc.tile_pool(name="sim4", bufs=2, space="PSUM") as simpp,
                tc.tile_pool(name="avps", bufs=2, space="PSUM") as avpp,
                tc.tile_pool(name="tps", bufs=1, space="PSUM") as tpp,
                tc.tile_pool(name="oq", bufs=2) as oqp,
            ):
                ones65 = atp.tile([DH + 1, DH], F32, tag="ones65",
                                  name="ones65")
                nc.vector.memset(ones65[:, :], 1.0)
                pend_tail = [None]

                def run_tail():
                    if pend_tail[0] is not None:
                        pend_tail[0]()
                        pend_tail[0] = None

                for q in range(4):
                    for m in (2 * q, 2 * q + 1):
                        i0 = m * 256
                        njt = 2 * m + 2
                        for h in range(H_LOC):
                            hp = (h % 2) * DH
                            qh = qT2[hp:hp + DH,
                                     (h // 2) * N + i0:(h // 2) * N + i0 + 256]
                            av_ps = avpp.tile([DH + 1, 256], F32, tag="av",
                                              name=f"av_{m}_{h}")
                            GSZ = 4
                            groups = [list(range(g, min(g + GSZ, njt)))
                                      for g in range(0, njt, GSZ)]
                            if len(groups[-1]) == GSZ:
                                # keep a spare exp column chunk for the
                                # null-key logits in the final group
                                groups[-1] = groups[-1][:GSZ - 1]
                                groups.append([njt - 1])
                            pend_av = None
                            av_state = [False]

                            def issue_av(pend, av_ps=av_ps, av_state=av_state):
                                pp4, pjts = pend
                                for ji, jt in enumerate(pjts):
                                    nc.tensor.matmul(
                                        out=av_ps[:, :],
                                        lhsT=v1[jt][:, :],
                                        rhs=pp4[:, ji * 256:ji * 256 + 256],
                                        start=(not av_state[0]), stop=False,
                                        skip_group_check=True)
                                    av_state[0] = True

                            for gi, jts in enumerate(groups):
                                gw = 256 * len(jts)
                                last = (gi == len(groups) - 1)
                                ps4 = simpp.tile([128, 1024], F32, tag="sim")
                                for ji, jt in enumerate(jts):
                                    j0 = jt * 128
                                    c0 = ji * 256
                                    # start=True only on the first chunk of
                                    # each 2KB psum bank: a start arms
                                    # zero-on-first-write for the whole bank
                                    nc.tensor.matmul(
                                        out=ps4[:, c0:c0 + 256],
                                        lhsT=kT[hp:hp + DH, j0:j0 + 128],
                                        rhs=qh,
                                        start=(c0 % 512 == 0), stop=False,
                                        skip_group_check=True)
                                # Toeplitz bias adds: two j-tiles merged per
                                # matmul via a 3D shifted AP (second touch of
                                # the armed bank, so plain accumulate)
                                for c0 in range(0, gw, 512):
                                    cn = min(2, (gw - c0) // 256)
                                    jt0 = jts[c0 // 256]
                                    u0 = 2048 - i0 + jt0 * 128
                                    mtr = mt[h][:, :].copy()
                                    pat = [list(p) for p in mtr.ap.to_list()]
                                    pat[1] = [128, cn]
                                    pat.append([-1, 256])
                                    mtr.ap = bass_rust.VecI64Pair(pat)
                                    mtr.offset = mtr.offset + u0
                                    nc.tensor.matmul(
                                        out=ps4[:, c0:c0 + cn * 256],
                                        lhsT=identb[:, :], rhs=mtr,
                                        start=False, stop=True,
                                        skip_group_check=True)
                                ew = gw
                                if last:
                                    # null-key logits ride along in the spare
                                    # columns of the final (partial) group
                                    nc.tensor.matmul(
                                        out=ps4[0:1, gw:gw + 256],
                                        lhsT=nkn[hp:hp + DH, :], rhs=qh,
                                        start=True, stop=True,
                                        skip_group_check=True)
                                    nc.vector.tensor_scalar_add(
                                        out=ps4[0:1, gw:gw + 256],
                                        in0=ps4[0:1, gw:gw + 256],
                                        scalar1=nb_sb[:, h:h + 1])
                                    ew = gw + 256
                                p4 = atp.tile([128, 1024], BF16, tag="p4")
                                nc.scalar.activation(
                                    out=p4[:, 0:ew], in_=ps4[:, 0:ew],
                                    func=mybir.ActivationFunctionType.Exp)
                                # software pipeline: issue deferred work now so
                                # the PE queue never parks waiting on this exp
                                if gi == 0:
                                    run_tail()
                                else:
                                    issue_av(pend_av)
                                pend_av = (p4, jts)

                            def tail(h=h, i0=i0, av_ps=av_ps, pend_av=pend_av,
                                     issue_av=issue_av,
                                     gw_last=256 * len(groups[-1])):
                                issue_av(pend_av)
                                nc.tensor.matmul(
                                    out=av_ps[:, :],
                                    lhsT=nv1[:, :],
                                    rhs=pend_av[0][0:1, gw_last:gw_last + 256],
                                    start=False, stop=True,
                                    skip_group_check=True)
                                # normalize columns by row-64 sums -> avT[h].
                                # full-height recip: base-partition-64 DVE
                                # slices silently no-op; only row 64 is read
                                # by the selector matmul below
                                rr = atp.tile([DH + 1, 256], F32, tag="rr")
                                nc.vector.reciprocal_approx_fast(
                                    out=rr[:, :], in_=av_ps[:, :])
                                psb = tpp.tile([DH, 256], F32, tag="bc")
                                nc.tensor.matmul(out=psb[:, :],
                                                 lhsT=ones65[DH:DH + 1, 0:DH],
                                                 rhs=rr[DH:DH + 1, :],
                                                 start=True, stop=True)
                                rb = atp.tile([DH, 256], BF16, tag="rb")
                                nc.scalar.copy(out=rb[:, :], in_=psb[:, :])
                                nc.vector.tensor_tensor(
                                    out=avT[h][:, i0:i0 + 256],
                                    in0=av_ps[0:DH, :], in1=rb[:, :],
                                    op=mybir.AluOpType.mult)
                            pend_tail[0] = tail

                    # out projection for this quarter
                    run_tail()
                    for tl in range(4):
                        tt = q * 4 + tl
                        ps_po = simpp.tile([128, 1024], F32, tag="sim")
                        for eb in range(2):
                            for ch in range(H_LOC):
                                nc.tensor.matmul(
                                    out=ps_po[:, eb * 512:eb * 512 + 512],
                                    lhsT=avT[ch][:, tt * 128:tt * 128 + 128],
                                    rhs=wout_sb[:, ch * D + eb * 512:
                                                ch * D + eb * 512 + 512],
                                    start=(ch == 0), stop=(ch == H_LOC - 1),
                                    skip_group_check=True)
                        po_sb = oqp.tile([128, D], BF16, tag="po")
                        nc.vector.tensor_copy(out=po_sb[:, :], in_=ps_po[:, :])
                        dma(out=po_q[q].ap()[tl * 128:(tl + 1) * 128, :],
                            in_=po_sb[:, :])
                    nc.gpsimd.collective_compute(
                        "ReduceScatter", mybir.AluOpType.add,
                        replica_groups=[[0, 1, 2, 3], [4, 5, 6, 7]],
                        ins=[po_q[q].ap().opt()],
                        outs=[rs_q[q].ap().opt()],
                    )
                    # final LN for the 128 owned rows of this quarter
                    y = oqp.tile([128, D], BF16, tag="y")
                    dma(out=y[:, :], in_=rs_q[q].ap()[:, :])
                    mean, var, sd, rstd = _ln_stats(nc, oqp, y[:, :], D)
                    nc.scalar.activation(out=sd[:, :], in_=var,
                                         func=mybir.ActivationFunctionType.Sqrt,
                                         bias=eps128[:, :])
                    nc.vector.reciprocal_approx_fast(out=rstd[:, :], in_=sd[:, :])
                    yf = oqp.tile([128, D], F32, tag="yf")
                    nc.vector.tensor_scalar(out=yf[:, :], in0=y[:, :],
                                            scalar1=mean,
                                            scalar2=rstd[:, :],
                                            op0=mybir.AluOpType.subtract,
                                            op1=mybir.AluOpType.mult)
                    nc.vector.tensor_tensor(out=yf[:, :], in0=yf[:, :],
                                            in1=goutb,
                                            op=mybir.AluOpType.mult)
                    dma(out=out_p.ap()[q * 128:(q + 1) * 128, :], in_=yf[:, :])
                if DBG:
                    dma(out=dbg_avT.ap(), in_=avT[0][:, :])
                    dma(out=dbg_mt.ap(), in_=mt[0][:, :])
                    dma(out=dbg_po.ap(), in_=po_q[0].ap()[0:128, :])

    nc.compile()
    return nc


def make_in_maps(x, mask, g_norm, Wq, Wkv, q_scale, k_scale, null_kv,
                 null_attn_bias, w0, b0, g0, w1, b1, g1, w2, b2, Wout, g_out):
    import ml_dtypes
    assert bool(np.asarray(mask).all()), "kernel assumes all-True mask"
    f = np.float32
    bf = ml_dtypes.bfloat16
    gn = np.asarray(g_norm, f)
    in_maps = []
    for c in range(N_CORES):
        bi, hg = c // 4, c % 4
        posb = (2047 - c * MLP_ROWS - np.arange(128)).astype(np.int32)
        wq_hg = np.asarray(Wq, f)[:, hg * INNER_LOC:(hg + 1) * INNER_LOC]
        wqkv = np.concatenate(
            [wq_hg, np.asarray(Wkv, f)], axis=1) * gn[:, None]
        wsum = -(wqkv.sum(axis=0))
        m = {
            "x": np.ascontiguousarray(np.asarray(x, f)[bi]).astype(bf),
            "wqkv": np.ascontiguousarray(wqkv).astype(bf),
            "wsum": np.ascontiguousarray(wsum.astype(f)),
            "qs8ks": np.ascontiguousarray(np.tile(
                8.0 * np.asarray(q_scale, f) * np.asarray(k_scale, f), 2)),
            "nkv": np.ascontiguousarray(np.tile(np.asarray(null_kv, f),
                                                (1, 2))),
            "nb": np.ascontiguousarray(
                np.asarray(null_attn_bias, f)[hg * H_LOC:(hg + 1) * H_LOC]),
            "w0v": np.ascontiguousarray(np.asarray(w0, f).reshape(D)),
            "b0": np.ascontiguousarray(np.asarray(b0, f)),
            "g0": np.ascontiguousarray(np.asarray(g0, f)),
            "w1": np.ascontiguousarray(np.asarray(w1, f)).astype(bf),
            "b1": np.ascontiguousarray(np.asarray(b1, f)),
            "g1": np.ascontiguousarray(np.asarray(g1, f)),
            "w2": np.ascontiguousarray(np.asarray(w2, f)).astype(bf),
            "b2": np.ascontiguousarray(np.asarray(b2, f)),
            "wout": np.ascontiguousarray(
                np.asarray(Wout, f)[hg * INNER_LOC:(hg + 1) * INNER_LOC, :]
            ).astype(bf),
            "g_out": np.ascontiguousarray(np.asarray(g_out, f)),
            "posb": posb,
            "sel": np.eye(HEADS, dtype=f)[:, hg * H_LOC:(hg + 1) * H_LOC].copy(),
        }
        in_maps.append(m)
    return in_maps


_NC_CACHE = None


def kernel(**inputs):
    global _NC_CACHE
    from concourse.bass_utils import run_bass_kernel_spmd

    if _NC_CACHE is None:
        _NC_CACHE = build()
    nc = _NC_CACHE
    in_maps = make_in_maps(**inputs)
    res = run_bass_kernel_spmd(nc, in_maps, core_ids=list(range(N_CORES)))
    outs = res.results
    kernel.last_outs = outs
    full = np.empty((2, N, D), np.float32)
    for c in range(N_CORES):
        bi, hg = c // 4, c % 4
        o = np.asarray(outs[c]["out"], np.float32)
        for q in range(4):
            full[bi, q * 512 + hg * 128:q * 512 + (hg + 1) * 128, :] = \
                o[q * 128:(q + 1) * 128]
    return full


# revision 52
# speedup vs baseline: 1.4211x; 1.0151x over previous
import sys

for _p in ("/opt/trn_rl_repo", "/root/.axon_site/_ro/trn_rl_repo"):
    if _p not in sys.path:
        sys.path.insert(0, _p)

import numpy as np

from concourse import bacc, mybir, tile
import bass_rust

N_CORES = 8
N = 2048
D = 1024
HEADS = 16
DH = 64
H_LOC = 4          # heads per core
INNER_LOC = H_LOC * DH  # 256
QKV = INNER_LOC + 2 * DH  # 384 packed projection width
NEG = -1.0e30
EPS = 1e-5
F32 = mybir.dt.float32
F32R = mybir.dt.float32r
BF16 = mybir.dt.bfloat16
I32 = mybir.dt.int32

# rel-pos MLP sharding: 2048 useful reversed-position rows, 256 per core.
MLP_ROWS = 256
HFR_PAD = 64       # data lives at HFRD[64 : 64+2048]
HFRD_ROWS = 2752   # covers all reads [64, 2494]
MT_W = 2560        # master toeplitz width


def _ap(t, pattern, offset):
    a = t.ap().copy()
    a.ap = bass_rust.VecI64Pair(pattern)
    a.offset = offset
    return a


def _ln_stats(nc, pool, x_sb, width):
    """Row LayerNorm stats via one-pass bn_stats.

    Returns (mean, var, sd, rstd) where mean/var are slices of the bn_aggr
    output and sd/rstd are empty tiles for the caller's sqrt/recip."""
    nch = (width + 511) // 512
    stats = pool.tile([128, 6 * nch], F32, tag="ln_bns")
    for c in range(nch):
        w = min(512, width - c * 512)
        nc.vector.bn_stats(out=stats[:, c * 6:(c + 1) * 6],
                           in_=x_sb[:, c * 512:c * 512 + w])
    mv = pool.tile([128, 2], F32, tag="ln_mv")
    nc.vector.bn_aggr(out=mv[:, :], in_=stats[:, :])
    sd = pool.tile([128, 1], F32, tag="ln_sd")
    rstd = pool.tile([128, 1], F32, tag="ln_rstd")
    return mv[:, 0:1], mv[:, 1:2], sd, rstd


def build():
    build.NO_AV = globals().get('NO_AV', False)
    nc = bacc.Bacc("TRN2", target_bir_lowering=False, debug=False,
                   num_devices=N_CORES)

    # ---------------- parameters ----------------
    x_p = nc.declare_dram_parameter("x", [N, D], BF16, isOutput=False)
    wqkv_p = nc.declare_dram_parameter("wqkv", [D, QKV], BF16, isOutput=False)
    wsum_p = nc.declare_dram_parameter("wsum", [QKV], F32, isOutput=False)
    # qs8ks and null-k are passed partition-duplicated ([x | x] over 128
    # partitions) so odd heads can slice base-partition-64 operands.
    qs8ks_p = nc.declare_dram_parameter("qs8ks", [128], F32, isOutput=False)
    nkv_p = nc.declare_dram_parameter("nkv", [2, 128], F32, isOutput=False)
    nb_p = nc.declare_dram_parameter("nb", [H_LOC], F32, isOutput=False)
    w0_p = nc.declare_dram_parameter("w0v", [D], F32, isOutput=False)
    b0_p = nc.declare_dram_parameter("b0", [D], F32, isOutput=False)
    g0_p = nc.declare_dram_parameter("g0", [D], F32, isOutput=False)
    w1_p = nc.declare_dram_parameter("w1", [D, D], BF16, isOutput=False)
    b1_p = nc.declare_dram_parameter("b1", [D], F32, isOutput=False)
    g1_p = nc.declare_dram_parameter("g1", [D], F32, isOutput=False)
    w2_p = nc.declare_dram_parameter("w2", [D, HEADS], BF16, isOutput=False)
    b2_p = nc.declare_dram_parameter("b2", [HEADS], F32, isOutput=False)
    wout_p = nc.declare_dram_parameter("wout", [INNER_LOC, D], BF16,
                                       isOutput=False)
    gout_p = nc.declare_dram_parameter("g_out", [D], F32, isOutput=False)
    posb_p = nc.declare_dram_parameter("posb", [128], I32, isOutput=False)
    sel_p = nc.declare_dram_parameter("sel", [HEADS, H_LOC], F32, isOutput=False)
    out_p = nc.declare_dram_parameter("out", [N // 4, D], F32, isOutput=True)
    DBG = globals().get('DEBUG_TAPS', False)
    if DBG:
        dbg_qT = nc.declare_dram_parameter("dbg_qT", [128, N], BF16, isOutput=True)
        dbg_kT = nc.declare_dram_parameter("dbg_kT", [128, N], BF16, isOutput=True)
        dbg_v0 = nc.declare_dram_parameter("dbg_v0", [128, DH + 1], BF16, isOutput=True)
        dbg_avT = nc.declare_dram_parameter("dbg_avT", [DH, N], BF16, isOutput=True)
        dbg_po = nc.declare_dram_parameter("dbg_po", [128, D], BF16, isOutput=True)
        dbg_mt = nc.declare_dram_parameter("dbg_mt", [128, MT_W], BF16, isOutput=True)
        dbg_avps = nc.declare_dram_parameter("dbg_avps", [DH + 1, N], F32, isOutput=True)
        dbg_rb = nc.declare_dram_parameter("dbg_rb", [DH, N], BF16, isOutput=True)
        dbg_p4 = nc.declare_dram_parameter("dbg_p4", [128, 1024], BF16, isOutput=True)
        dbg_ps4 = nc.declare_dram_parameter("dbg_ps4", [128, 1024], F32, isOutput=True)

    # ---------------- internal DRAM ----------------
    hfr_loc = nc.dram_tensor("hfr_loc", [MLP_ROWS, HEADS], F32)
    hfr_g = nc.dram_tensor("hfr_g", [8 * MLP_ROWS, HEADS], F32)
    hfrd = nc.dram_tensor("hfrd", [H_LOC, HFRD_ROWS], BF16)
    po_q = [nc.dram_tensor(f"po_{i}", [N // 4, D], BF16) for i in range(4)]
    rs_q = [nc.dram_tensor(f"rs_{i}", [N // 16, D], BF16) for i in range(4)]

    dma = nc.sync.dma_start

    with tile.TileContext(nc) as tc:
        with (
            tc.tile_pool(name="const", bufs=1) as constp,
            tc.tile_pool(name="pers", bufs=1) as pers,
            tc.tile_pool(name="weights", bufs=1) as wp,
        ):
            ident = constp.tile([128, 128], F32)
            from concourse.masks import make_identity
            make_identity(nc, ident[:, :])
            identb = constp.tile([128, 128], BF16)
            nc.scalar.copy(out=identb[:, :], in_=ident[:, :])
            eps128 = constp.tile([128, 1], F32)
            nc.vector.memset(eps128[:, :], EPS)
            ones1 = constp.tile([1, 128], F32)
            nc.vector.memset(ones1[:, :], 1.0)

            # ---- all big input loads issued up front (single SP queue) ----
            early = tc.tile_pool(name="early", bufs=1)
            ep = early.__enter__()
            xall = ep.tile([128, 16 * D], BF16, name="xall")
            dma(out=xall[:, :], in_=_ap(x_p, [[D, 128], [128 * D, 16], [1, D]], 0))
            wqkv_sb = ep.tile([128, 8 * QKV], BF16)
            dma(out=wqkv_sb[:, :],
                in_=_ap(wqkv_p, [[QKV, 128], [128 * QKV, 8], [1, QKV]], 0))
            wout_sb = wp.tile([DH, 4 * D], BF16)
            dma(out=wout_sb[:, :],
                in_=_ap(wout_p, [[D, DH], [DH * D, 4], [1, D]], 0))
            w1_sb = ep.tile([128, 8 * D], BF16)  # chunk k at cols k*1024
            dma(out=w1_sb[:, :],
                in_=_ap(w1_p, [[D, 128], [128 * D, 8], [1, D]], 0))
            w2_sb = ep.tile([128, 8 * HEADS], BF16)
            dma(out=w2_sb[:, :],
                in_=_ap(w2_p, [[HEADS, 128], [128 * HEADS, 8], [1, HEADS]], 0))

            wsum_f32 = ep.tile([1, QKV], F32)
            dma(out=wsum_f32[:, :], in_=_ap(wsum_p, [[QKV, 1], [1, QKV]], 0))
            wsum_row = ep.tile([1, QKV], BF16)
            nc.vector.tensor_copy(out=wsum_row[:, :], in_=wsum_f32[:, :])
            qs8ks_sb = pers.tile([128, 1], F32)
            dma(out=qs8ks_sb[:, :], in_=_ap(qs8ks_p, [[1, 128], [1, 1]], 0))
            nkT = pers.tile([128, 1], F32)
            dma(out=nkT[:, :], in_=_ap(nkv_p, [[1, 128], [1, 1]], 0))
            nv_sb = pers.tile([1, DH], F32)
            dma(out=nv_sb[:, :], in_=nkv_p.ap()[1:2, 0:DH])
            nb_sb = pers.tile([1, H_LOC], F32)
            dma(out=nb_sb[:, :], in_=_ap(nb_p, [[H_LOC, 1], [1, H_LOC]], 0))
            posi_t = pers.tile([128, 1], I32, name="posi")
            dma(out=posi_t[:, :], in_=_ap(posb_p, [[1, 128], [1, 1]], 0))
            sel_sb = wp.tile([HEADS, H_LOC], F32)
            dma(out=sel_sb[:, :], in_=sel_p.ap())

            # [nv | 1] * exp(nb[h]) bf16 rows for null-key AV (nb_p holds
            # exp(null_attn_bias) so the bias add disappears into the AV)
            nv1f = pers.tile([1, DH + 1], F32)
            nc.vector.tensor_copy(out=nv1f[:, 0:DH], in_=nv_sb[:, :])
            nc.vector.memset(nv1f[:, DH:DH + 1], 1.0)
            nv1e = pers.tile([1, H_LOC * (DH + 1)], BF16, name="nv1e")
            for hh in range(H_LOC):
                nc.vector.tensor_scalar(
                    out=nv1e[:, hh * (DH + 1):(hh + 1) * (DH + 1)],
                    in0=nv1f[:, :], scalar1=nb_sb[:, hh:hh + 1], scalar2=None,
                    op0=mybir.AluOpType.mult)

            # ---------- Phases 0-2 interleaved: rel-pos MLP generator is ----
            # pumped between projection steps so its long serial chain fills
            # engine gaps instead of blocking the in-order queues.
            qT2 = pers.tile([128, 2 * N], BF16, name="qT2")
            kT = pers.tile([128, N], BF16, tag="kT", name="kT")
            v1 = [pers.tile([128, DH + 1], BF16, tag=f"v1_{j}", name=f"v1_{j}")
                  for j in range(16)]
            nkn = pers.tile([128, 1], BF16)

            with (
                tc.tile_pool(name="bc_ps", bufs=2, space="PSUM") as bpp,
                tc.tile_pool(name="vst", bufs=3) as vstp,
            ):
                bcast = ep.tile([128, 4 * D + HEADS], BF16)
                g1b = ep.tile([128, D], BF16)
                goutb = wp.tile([128, D], F32)
                chunks = []
                for pi, par in enumerate((w0_p, b0_p, g0_p, b1_p)):
                    for half in range(2):
                        chunks.append((par, half * 512, 512, bcast,
                                       pi * D + half * 512))
                chunks.append((b2_p, 0, HEADS, bcast, 4 * D))
                for half in range(2):
                    chunks.append((g1_p, half * 512, 512, g1b, half * 512))
                    chunks.append((gout_p, half * 512, 512, goutb,
                                   half * 512))
                for par, poff, wdt, dst, doff in chunks:
                    vstage = vstp.tile([1, 512], F32, tag="vstage")
                    dma(out=vstage[:, 0:wdt],
                        in_=_ap(par, [[wdt, 1], [1, wdt]], poff))
                    ps = bpp.tile([128, 512], F32, tag="bc")
                    nc.tensor.matmul(out=ps[:, 0:wdt],
                                     lhsT=ones1[:, :],
                                     rhs=vstage[:, 0:wdt],
                                     start=True, stop=True)
                    nc.scalar.copy(out=dst[:, doff:doff + wdt],
                                   in_=ps[:, 0:wdt])
            w0b = bcast[:, 0:D]
            b0b = bcast[:, D:2 * D]
            g0b = bcast[:, 2 * D:3 * D]
            b1b = bcast[:, 3 * D:4 * D]
            b2b = bcast[:, 4 * D:4 * D + HEADS]

            with (
                tc.tile_pool(name="mlp", bufs=1) as mp,
                tc.tile_pool(name="mlp_ps", bufs=1, space="PSUM") as mpp,
                tc.tile_pool(name="mlp_ps2", bufs=1, space="PSUM") as mpp2,
                tc.tile_pool(name="xT", bufs=1) as xTp,
                tc.tile_pool(name="xt", bufs=2) as xtp,
                tc.tile_pool(name="xps", bufs=2, space="PSUM") as xpp,
                tc.tile_pool(name="xps2", bufs=1, space="PSUM") as xpp2,
                tc.tile_pool(name="xps3", bufs=1, space="PSUM") as xpp3,
            ):
                xT = xTp.tile([128, 8 * N], BF16)  # d-chunk k at cols k*2048
                negrow = wp.tile([1, N], BF16, name="negrow")

                def mlp_gen():
                    for t in range(2):
                        m0 = t * 128
                        posf = mp.tile([128, 1], F32, tag="posf",
                                       name=f"posf{t}")
                        nc.vector.tensor_scalar_add(out=posf[:, :],
                                                    in0=posi_t[:, :],
                                                    scalar1=float(-m0))
                        h0 = mp.tile([128, D], F32, tag="h0", name=f"h0_{t}")
                        nc.vector.tensor_scalar(out=h0[:, :], in0=w0b,
                                                scalar1=posf[:, :], scalar2=None,
                                                op0=mybir.AluOpType.mult)
                        nc.vector.tensor_tensor(out=h0[:, :], in0=h0[:, :],
                                                in1=b0b,
                                                op=mybir.AluOpType.add)
                        yield
                        mean, var, sd, rstd = _ln_stats(nc, mp, h0[:, :], D)
                        nc.scalar.activation(
                            out=sd[:, :], in_=var,
                            func=mybir.ActivationFunctionType.Sqrt,
                            bias=eps128[:, :])
                        nc.vector.reciprocal_approx_fast(out=rstd[:, :],
                                                         in_=sd[:, :])
                        yield
                        nc.vector.tensor_scalar(out=h0[:, :], in0=h0[:, :],
                                                scalar1=mean,
                                                scalar2=rstd[:, :],
                                                op0=mybir.AluOpType.subtract,
                                                op1=mybir.AluOpType.mult)
                        nc.vector.tensor_tensor(out=h0[:, :], in0=h0[:, :],
                                                in1=g0b,
                                                op=mybir.AluOpType.mult)
                        h0b = mp.tile([128, D], BF16, tag="h0b",
                                      name=f"h0b{t}")
                        nc.scalar.activation(
                            out=h0b[:, :], in_=h0[:, :],
                            func=mybir.ActivationFunctionType.Silu)
                        yield
                        h0T = mp.tile([128, D], BF16, tag="h0T",
                                      name=f"h0T{t}")
                        pst8m = mpp2.tile([128, D], BF16, tag="tp",
                                          name=f"tp0_{t}")
                        for k in range(8):
                            nc.tensor.matmul(
                                out=pst8m[:, k * 128:(k + 1) * 128],
                                lhsT=h0b[:, k * 128:(k + 1) * 128],
                                rhs=identb[:, :], is_transpose=True,
                                start=True, stop=True)
                            if k == 3:
                                yield
                        nc.vector.tensor_copy(out=h0T[:, :], in_=pst8m[:, :])
                        yield
                        h1 = mp.tile([128, D], F32, tag="h1", name=f"h1_{t}")
                        for eb in range(2):
                            ps = mpp.tile([128, 512], F32, tag="h1ps",
                                          name=f"h1ps{t}_{eb}")
                            for k in range(8):
                                nc.tensor.matmul(
                                    out=ps[:, :],
                                    lhsT=h0T[:, k * 128:(k + 1) * 128],
                                    rhs=w1_sb[:, k * D + eb * 512:
                                              k * D + eb * 512 + 512],
                                    start=(k == 0), stop=(k == 7))
                            nc.vector.tensor_tensor(
                                out=h1[:, eb * 512:eb * 512 + 512],
                                in0=ps[:, :],
                                in1=b1b[:, eb * 512:eb * 512 + 512],
                                op=mybir.AluOpType.add)
                            yield
                        mean, var, sd, rstd = _ln_stats(nc, mp, h1[:, :], D)
                        nc.scalar.activation(
                            out=sd[:, :], in_=var,
                            func=mybir.ActivationFunctionType.Sqrt,
                            bias=eps128[:, :])
                        nc.vector.reciprocal_approx_fast(out=rstd[:, :],
                                                         in_=sd[:, :])
                        yield
                        nc.vector.tensor_scalar(out=h1[:, :], in0=h1[:, :],
                                                scalar1=mean,
                                                scalar2=rstd[:, :],
                                                op0=mybir.AluOpType.subtract,
                                                op1=mybir.AluOpType.mult)
                        nc.vector.tensor_tensor(out=h1[:, :], in0=h1[:, :],
                                                in1=g1b,
                                                op=mybir.AluOpType.mult)
                        h1b = mp.tile([128, D], BF16, tag="h1b",
                                      name=f"h1b{t}")
                        nc.scalar.activation(
                            out=h1b[:, :], in_=h1[:, :],
                            func=mybir.ActivationFunctionType.Silu)
                        yield
                        h1T = mp.tile([128, D], BF16, tag="h1T",
                                      name=f"h1T{t}")
                        pst8n = mpp2.tile([128, D], BF16, tag="tp",
                                          name=f"tp1_{t}")
                        for k in range(8):
                            nc.tensor.matmul(
                                out=pst8n[:, k * 128:(k + 1) * 128],
                                lhsT=h1b[:, k * 128:(k + 1) * 128],
                                rhs=identb[:, :], is_transpose=True,
                                start=True, stop=True)
                            if k == 3:
                                yield
                        nc.vector.tensor_copy(out=h1T[:, :], in_=pst8n[:, :])
                        yield
                        psf = mpp2.tile([128, HEADS], F32, tag="hf",
                                        name=f"hf{t}")
                        for k in range(8):
                            nc.tensor.matmul(
                                out=psf[:, :],
                                lhsT=h1T[:, k * 128:(k + 1) * 128],
                                rhs=w2_sb[:, k * HEADS:(k + 1) * HEADS],
                                start=(k == 0), stop=(k == 7))
                        hfc = mp.tile([128, HEADS], F32, tag="hfc",
                                      name=f"hfc{t}")
                        nc.vector.tensor_tensor(out=hfc[:, :], in0=psf[:, :],
                                                in1=b2b,
                                                op=mybir.AluOpType.add)
                        dma(out=hfr_loc.ap()[m0:m0 + 128, :], in_=hfc[:, :])
                        yield
                    nc.gpsimd.collective_compute(
                        "AllGather", mybir.AluOpType.bypass,
                        replica_groups=[list(range(N_CORES))],
                        ins=[hfr_loc.ap().opt()],
                        outs=[hfr_g.ap().opt()],
                    )

                mgen = mlp_gen()

                def pump(n=1):
                    for _ in range(n):
                        try:
                            next(mgen)
                        except StopIteration:
                            return

                for tt in range(16):
                    xs = xall[:, tt * D:(tt + 1) * D]
                    # stats (one-pass bn_stats; wsum carries -colsum so the
                    # rank-1 correction uses the mean row directly)
                    mean, var, sd, rstd = _ln_stats(nc, xtp, xs, D)
                    nc.scalar.activation(out=sd[:, :], in_=var,
                                         func=mybir.ActivationFunctionType.Sqrt,
                                         bias=eps128[:, :])
                    nc.vector.reciprocal_approx_fast(out=rstd[:, :],
                                                     in_=sd[:, :])
                    psr = xpp3.tile([128, 128], F32, tag="misc",
                                    name=f"psr{tt}")
                    nc.tensor.matmul(out=psr[0:1, :], lhsT=mean,
                                     rhs=ident[:, :], is_transpose=True,
                                     start=True, stop=True)
                    nc.vector.tensor_copy(out=negrow[:, tt * 128:tt * 128 + 128],
                                          in_=psr[0:1, :])
                    pump(1)
                    # x transposes (bf16): 8 into one psum tile, one fat copy
                    pst8 = xpp2.tile([128, D], BF16, tag="tp8",
                                     name=f"pst8_{tt}")
                    for k in range(8):
                        nc.tensor.matmul(out=pst8[:, k * 128:(k + 1) * 128],
                                         lhsT=xall[:, tt * D + k * 128:
                                                   tt * D + k * 128 + 128],
                                         rhs=identb[:, :], is_transpose=True,
                                         start=True, stop=True)
                    xTo = xT[:, :].copy()
                    xpat = [list(p) for p in xTo.ap.to_list()]
                    xpat = [xpat[0], [N, 8], [1, 128]]
                    xTo.ap = bass_rust.VecI64Pair(xpat)
                    xTo.offset = xTo.offset + tt * 128
                    nc.vector.tensor_copy(out=xTo, in_=pst8[:, :])
                    pump(1)
                    # packed q|k|v projection with rank-1 mean correction
                    psq = xpp.tile([128, QKV], F32, tag="qkv")
                    for k in range(8):
                        nc.tensor.matmul(
                            out=psq[:, :],
                            lhsT=xT[:, k * N + tt * 128:k * N + tt * 128 + 128],
                            rhs=wqkv_sb[:, k * QKV:(k + 1) * QKV],
                            start=(k == 0), stop=False,
                            skip_group_check=True)
                    nc.tensor.matmul(out=psq[:, :],
                                     lhsT=negrow[:, tt * 128:tt * 128 + 128],
                                     rhs=wsum_row[:, :],
                                     start=False, stop=True,
                                     skip_group_check=True)
                    # per-head l2 norms (4 q heads + k)
                    nrm = xtp.tile([128, 8], F32, tag="nrm")
                    scr2 = xtp.tile([128, DH], BF16, tag="scr2")
                    for j in range(5):
                        nc.scalar.activation(
                            out=scr2[:, :],
                            in_=psq[:, j * DH:(j + 1) * DH],
                            func=mybir.ActivationFunctionType.Square,
                            accum_out=nrm[:, j:j + 1])
                    sd5 = xtp.tile([128, 8], F32, tag="sd5")
                    rinv = xtp.tile([128, 8], F32, tag="rinv")
                    nc.scalar.activation(out=sd5[:, 0:5], in_=nrm[:, 0:5],
                                         func=mybir.ActivationFunctionType.Sqrt)
                    nc.vector.reciprocal(out=rinv[:, 0:5], in_=sd5[:, 0:5])
                    pump(1)
                    # scaled copies out of PSUM
                    qn = xtp.tile([128, INNER_LOC], BF16, tag="qn")
                    for h in range(4):
                        eng = nc.vector if h % 2 == 0 else None
                        if h % 2 == 0:
                            nc.vector.tensor_scalar(
                                out=qn[:, h * DH:(h + 1) * DH],
                                in0=psq[:, h * DH:(h + 1) * DH],
                                scalar1=rinv[:, h:h + 1], scalar2=None,
                                op0=mybir.AluOpType.mult)
                        else:
                            nc.scalar.activation(
                                out=qn[:, h * DH:(h + 1) * DH],
                                in_=psq[:, h * DH:(h + 1) * DH],
                                func=mybir.ActivationFunctionType.Copy,
                                scale=rinv[:, h:h + 1])
                    # kn duplicated into both column halves so the transpose
                    # yields kT stacked twice along partitions
                    kn = xtp.tile([128, 128], BF16, tag="kn")
                    for kh in range(2):
                        nc.vector.tensor_scalar(
                            out=kn[:, kh * DH:(kh + 1) * DH],
                            in0=psq[:, INNER_LOC:INNER_LOC + DH],
                            scalar1=rinv[:, 4:5], scalar2=None,
                            op0=mybir.AluOpType.mult)
                    nc.vector.tensor_scalar(out=v1[tt][:, 0:DH],
                                            in0=psq[:, INNER_LOC + DH:QKV],
                                            scalar1=rstd[:, :], scalar2=None,
                                            op0=mybir.AluOpType.mult)
                    nc.vector.memset(v1[tt][:, DH:DH + 1], 1.0)
                    # q pair + k transposes into one psum tile
                    pstqk = xpp2.tile([128, 384], BF16, tag="tpqk",
                                      name=f"pstqk{tt}")
                    for p in range(2):
                        nc.tensor.matmul(out=pstqk[:, p * 128:(p + 1) * 128],
                                         lhsT=qn[:, p * 128:(p + 1) * 128],
                                         rhs=identb[:, :], is_transpose=True,
                                         start=True, stop=True)
                    nc.tensor.matmul(out=pstqk[:, 256:384], lhsT=kn[:, :],
                                     rhs=identb[:, :], is_transpose=True,
                                     start=True, stop=True)
                    qTo = qT2[:, :].copy()
                    qpat = [list(p) for p in qTo.ap.to_list()]
                    qpat = [qpat[0], [N, 2], [1, 128]]
                    qTo.ap = bass_rust.VecI64Pair(qpat)
                    qTo.offset = qTo.offset + tt * 128
                    nc.scalar.copy(out=qTo, in_=pstqk[:, 0:256])
                    # k transpose with qs8ks scale folded in
                    nc.vector.tensor_scalar(out=kT[:, tt * 128:tt * 128 + 128],
                                            in0=pstqk[:, 256:384],
                                            scalar1=qs8ks_sb[:, :], scalar2=None,
                                            op0=mybir.AluOpType.mult)
                    pump(1)

                pump(100)

                if DBG:
                    dma(out=dbg_qT.ap(), in_=qT2[:, 0:N])
                    dma(out=dbg_kT.ap(), in_=kT[:, :])
                    dma(out=dbg_v0.ap(), in_=v1[0][:, :])

                # null key normalize: nkn = l2norm(nk) * qs8ks  (dup over 128)
                ones64c_f = constp.tile([DH, 1], F32)
                nc.vector.memset(ones64c_f[:, :], 1.0)
                nsq = xtp.tile([128, 1], F32, tag="nsq")
                nc.scalar.activation(out=nsq[:, :], in_=nkT[:, :],
                                     func=mybir.ActivationFunctionType.Square)
                psn1 = xpp3.tile([128, 128], F32, tag="misc", name="psn1")
                nc.tensor.matmul(out=psn1[0:1, 0:1], lhsT=ones64c_f[:, :],
                                 rhs=nsq[0:DH, :], start=True, stop=True)
                rn1 = xtp.tile([1, 1], F32, tag="rn1")
                nc.scalar.activation(out=rn1[:, :], in_=psn1[0:1, 0:1],
                                     func=mybir.ActivationFunctionType.Sqrt)
                with nc.allow_low_precision(reason="f32r same bits as f32"):
                    nc.vector.reciprocal(out=rn1[:, :], in_=rn1[:, :])
                psb1 = xpp3.tile([128, 128], F32, tag="misc", name="psb1")
                nc.tensor.matmul(out=psb1[:, 0:1], lhsT=ones1[:, :],
                                 rhs=rn1[:, :], start=True, stop=True)
                nc.vector.tensor_tensor(out=nkn[:, :], in0=nkT[:, :],
                                        in1=psb1[:, 0:1],
                                        op=mybir.AluOpType.mult)
                nc.vector.tensor_scalar(out=nkn[:, :], in0=nkn[:, :],
                                        scalar1=qs8ks_sb[:, :], scalar2=None,
                                        op0=mybir.AluOpType.mult)

                # ---- stage AllGathered MLP rows -> hfrd (batched) ----
                stg = xtp.tile([128, 16 * HEADS], F32, tag="stg",
                               name="stg_all")
                dma(out=stg[:, :],
                    in_=_ap(hfr_g, [[HEADS, 128], [128 * HEADS, 16],
                                    [1, HEADS]], 0))
                stgT = xTp.tile([HEADS, 16 * 128], F32, name="stgT")
                for chunk in range(16):
                    pss = xpp.tile([128, QKV], F32, tag="qkv",
                                   name=f"stgps{chunk}")
                    nc.tensor.matmul(out=pss[0:HEADS, 0:128],
                                     lhsT=stg[:, chunk * HEADS:
                                              (chunk + 1) * HEADS],
                                     rhs=ident[:, :], is_transpose=True,
                                     start=True, stop=True)
                    eng = nc.scalar if chunk % 2 == 0 else nc.vector
                    if chunk % 2 == 0:
                        nc.scalar.copy(
                            out=stgT[:, chunk * 128:(chunk + 1) * 128],
                            in_=pss[0:HEADS, 0:128])
                    else:
                        nc.vector.tensor_copy(
                            out=stgT[:, chunk * 128:(chunk + 1) * 128],
                            in_=pss[0:HEADS, 0:128])
                # select local heads and write hfrd in 512-col pieces
                for piece in range(4):
                    psl = xpp3.tile([128, 128], F32, tag="misc",
                                    name=f"psl{piece}")
                    stl = xtp.tile([H_LOC, 512], BF16, tag="stl")
                    for sub in range(4):
                        col = piece * 512 + sub * 128
                        nc.tensor.matmul(out=psl[0:H_LOC, 0:128],
                                         lhsT=sel_sb[:, :],
                                         rhs=stgT[:, col:col + 128],
                                         start=True, stop=True)
                        if sub % 2 == 0:
                            nc.scalar.copy(out=stl[:, sub * 128:sub * 128 + 128],
                                           in_=psl[0:H_LOC, 0:128])
                        else:
                            nc.vector.tensor_copy(
                                out=stl[:, sub * 128:sub * 128 + 128],
                                in_=psl[0:H_LOC, 0:128])
                    dma(out=_ap(hfrd, [[HFRD_ROWS, H_LOC], [1, 512]],
                                HFR_PAD + piece * 512),
                        in_=stl[:, :])
                poison = xtp.tile([H_LOC, HFRD_ROWS - 2112], BF16,
                                  name="poison")
                nc.vector.memset(poison[:, :], NEG)
                dma(out=_ap(hfrd, [[HFRD_ROWS, H_LOC],
                                   [1, HFRD_ROWS - 2112]], 2112),
                    in_=poison[:, :])

            early.__exit__(None, None, None)

            # ---------- Phase 3: attention + per-quarter out-proj + RS ------
            avT = [pers.tile([DH, N], BF16, tag=f"avT{h}", name=f"avT{h}")
                   for h in range(H_LOC)]
            mt = [pers.tile([128, MT_W], BF16, tag=f"mt{h}", name=f"mt{h}")
                  for h in range(H_LOC)]
            for h in range(H_LOC):
                dma(out=mt[h][:, :],
                    in_=_ap(hfrd, [[1, 128], [1, MT_W]], h * HFRD_ROWS + 63))
            if build.NO_AV:
                for h in range(H_LOC):
                    nc.vector.memset(avT[h][:, :], 0.0)

            with (
                tc.tile_pool(name="at", bufs=3) as atp,
                tc.tile_pool(name="sim4", bufs=2, space="PSUM") as simpp,
                tc.tile_pool(name="avps", bufs=2, space="PSUM") as avpp,
                tc.tile_pool(name="tps", bufs=1, space="PSUM") as tpp,
                tc.tile_pool(name="oq", bufs=2) as oqp,
            ):
                ones65 = atp.tile([DH + 1, DH], F32, tag="ones65",
                                  name="ones65")
                nc.vector.memset(ones65[:, :], 1.0)
                pend_tail = [None]

                def run_tail():
                    if pend_tail[0] is not None:
                        pend_tail[0]()
                        pend_tail[0] = None

                for q in range(4):
                    for m in (2 * q, 2 * q + 1):
                        i0 = m * 256
                        njt = 2 * m + 2
                        for h in range(H_LOC):
                            hp = (h % 2) * DH
                            qh = qT2[hp:hp + DH,
                                     (h // 2) * N + i0:(h // 2) * N + i0 + 256]
                            av_ps = avpp.tile([DH + 1, 256], F32, tag="av",
                                              name=f"av_{m}_{h}")
                            GSZ = 4
                            groups = [list(range(g, min(g + GSZ, njt)))
                                      for g in range(0, njt, GSZ)]
                            if len(groups[-1]) == GSZ:
                                # keep a spare exp column chunk for the
                                # null-key logits in the final group
                                groups[-1] = groups[-1][:GSZ - 1]
                                groups.append([njt - 1])
                            pend_av = None
                            av_state = [False]

                            def issue_av(pend, av_ps=av_ps, av_state=av_state):
                                pp4, pjts = pend
                                for ji, jt in enumerate(pjts):
                                    nc.tensor.matmul(
                                        out=av_ps[:, :],
                                        lhsT=v1[jt][:, :],
                                        rhs=pp4[:, ji * 256:ji * 256 + 256],
                                        start=(not av_state[0]), stop=False,
                                        skip_group_check=True)
                                    av_state[0] = True

                            for gi, jts in enumerate(groups):
                                gw = 256 * len(jts)
                                last = (gi == len(groups) - 1)
                                ps4 = simpp.tile([128, 1024], F32, tag="sim")
                                for ji, jt in enumerate(jts):
                                    j0 = jt * 128
                                    c0 = ji * 256
                                    # start=True only on the first chunk of
                                    # each 2KB psum bank: a start arms
                                    # zero-on-first-write for the whole bank
                                    nc.tensor.matmul(
                                        out=ps4[:, c0:c0 + 256],
                                        lhsT=kT[hp:hp + DH, j0:j0 + 128],
                                        rhs=qh,
                                        start=(c0 % 512 == 0), stop=False,
                                        skip_group_check=True)
                                # Toeplitz bias adds: two j-tiles merged per
                                # matmul via a 3D shifted AP (second touch of
                                # the armed bank, so plain accumulate)
                                for c0 in range(0, gw, 512):
                                    cn = min(2, (gw - c0) // 256)
                                    jt0 = jts[c0 // 256]
                                    u0 = 2048 - i0 + jt0 * 128
                                    mtr = mt[h][:, :].copy()
                                    pat = [list(p) for p in mtr.ap.to_list()]
                                    pat[1] = [128, cn]
                                    pat.append([-1, 256])
                                    mtr.ap = bass_rust.VecI64Pair(pat)
                                    mtr.offset = mtr.offset + u0
                                    nc.tensor.matmul(
                                        out=ps4[:, c0:c0 + cn * 256],
                                        lhsT=identb[:, :], rhs=mtr,
                                        start=False, stop=True,
                                        skip_group_check=True)
                                ew = gw
                                if last:
                                    # null-key logits ride along in the spare
                                    # columns of the final (partial) group
                                    nc.tensor.matmul(
                                        out=ps4[0:1, gw:gw + 256],
                                        lhsT=nkn[hp:hp + DH, :], rhs=qh,
                                        start=True, stop=True,
                                        skip_group_check=True)
                                    ew = gw + 256
                                p4 = atp.tile([128, 1024], BF16, tag="p4")
                                nc.scalar.activation(
                                    out=p4[:, 0:ew], in_=ps4[:, 0:ew],
                                    func=mybir.ActivationFunctionType.Exp)
                                # software pipeline: issue deferred work now so
                                # the PE queue never parks waiting on this exp
                                if gi == 0:
                                    run_tail()
                                else:
                                    issue_av(pend_av)
                                pend_av = (p4, jts)

                            def tail(h=h, i0=i0, av_ps=av_ps, pend_av=pend_av,
                                     issue_av=issue_av,
                                     gw_last=256 * len(groups[-1])):
                                issue_av(pend_av)
                                nc.tensor.matmul(
                                    out=av_ps[:, :],
                                    lhsT=nv1e[:, h * (DH + 1):
                                              (h + 1) * (DH + 1)],
                                    rhs=pend_av[0][0:1, gw_last:gw_last + 256],
                                    start=False, stop=True,
                                    skip_group_check=True)
                                # normalize columns by row-64 sums -> avT[h].
                                # full-height recip: base-partition-64 DVE
                                # slices silently no-op; only row 64 is read
                                # by the selector matmul below
                                rr = atp.tile([DH + 1, 256], F32, tag="rr")
                                nc.vector.reciprocal_approx_fast(
                                    out=rr[:, :], in_=av_ps[:, :])
                                psb = tpp.tile([DH, 256], F32, tag="bc")
                                nc.tensor.matmul(out=psb[:, :],
                                                 lhsT=ones65[DH:DH + 1, 0:DH],
                                                 rhs=rr[DH:DH + 1, :],
                                                 start=True, stop=True)
                                rb = atp.tile([DH, 256], BF16, tag="rb")
                                nc.scalar.copy(out=rb[:, :], in_=psb[:, :])
                                nc.vector.tensor_tensor(
                                    out=avT[h][:, i0:i0 + 256],
                                    in0=av_ps[0:DH, :], in1=rb[:, :],
                                    op=mybir.AluOpType.mult)
                            pend_tail[0] = tail

                    # out projection for this quarter
                    run_tail()
                    for tl in range(4):
                        tt = q * 4 + tl
                        ps_po = simpp.tile([128, 1024], F32, tag="sim")
                        for eb in range(2):
                            for ch in range(H_LOC):
                                nc.tensor.matmul(
                                    out=ps_po[:, eb * 512:eb * 512 + 512],
                                    lhsT=avT[ch][:, tt * 128:tt * 128 + 128],
                                    rhs=wout_sb[:, ch * D + eb * 512:
                                                ch * D + eb * 512 + 512],
                                    start=(ch == 0), stop=(ch == H_LOC - 1),
                                    skip_group_check=True)
                        po_sb = oqp.tile([128, D], BF16, tag="po")
                        nc.vector.tensor_copy(out=po_sb[:, :], in_=ps_po[:, :])
                        dma(out=po_q[q].ap()[tl * 128:(tl + 1) * 128, :],
                            in_=po_sb[:, :])
                    nc.gpsimd.collective_compute(
                        "ReduceScatter", mybir.AluOpType.add,
                        replica_groups=[[0, 1, 2, 3], [4, 5, 6, 7]],
                        ins=[po_q[q].ap().opt()],
                        outs=[rs_q[q].ap().opt()],
                    )
                    # final LN for the 128 owned rows of this quarter
                    y = oqp.tile([128, D], BF16, tag="y")
                    dma(out=y[:, :], in_=rs_q[q].ap()[:, :])
                    mean, var, sd, rstd = _ln_stats(nc, oqp, y[:, :], D)
                    nc.scalar.activation(out=sd[:, :], in_=var,
                                         func=mybir.ActivationFunctionType.Sqrt,
                                         bias=eps128[:, :])
                    nc.vector.reciprocal_approx_fast(out=rstd[:, :], in_=sd[:, :])
                    yf = oqp.tile([128, D], F32, tag="yf")
                    nc.vector.tensor_scalar(out=yf[:, :], in0=y[:, :],
                                            scalar1=mean,
                                            scalar2=rstd[:, :],
                                            op0=mybir.AluOpType.subtract,
                                            op1=mybir.AluOpType.mult)
                    nc.vector.tensor_tensor(out=yf[:, :], in0=yf[:, :],
                                            in1=goutb,
                                            op=mybir.AluOpType.mult)
                    dma(out=out_p.ap()[q * 128:(q + 1) * 128, :], in_=yf[:, :])
                if DBG:
                    dma(out=dbg_avT.ap(), in_=avT[0][:, :])
                    dma(out=dbg_mt.ap(), in_=mt[0][:, :])
                    dma(out=dbg_po.ap(), in_=po_q[0].ap()[0:128, :])

    nc.compile()
    return nc


def make_in_maps(x, mask, g_norm, Wq, Wkv, q_scale, k_scale, null_kv,
                 null_attn_bias, w0, b0, g0, w1, b1, g1, w2, b2, Wout, g_out):
    import ml_dtypes
    assert bool(np.asarray(mask).all()), "kernel assumes all-True mask"
    f = np.float32
    bf = ml_dtypes.bfloat16
    gn = np.asarray(g_norm, f)
    in_maps = []
    for c in range(N_CORES):
        bi, hg = c // 4, c % 4
        posb = (2047 - c * MLP_ROWS - np.arange(128)).astype(np.int32)
        wq_hg = np.asarray(Wq, f)[:, hg * INNER_LOC:(hg + 1) * INNER_LOC]
        wqkv = np.concatenate(
            [wq_hg, np.asarray(Wkv, f)], axis=1) * gn[:, None]
        wsum = -(wqkv.sum(axis=0))
        m = {
            "x": np.ascontiguousarray(np.asarray(x, f)[bi]).astype(bf),
            "wqkv": np.ascontiguousarray(wqkv).astype(bf),
            "wsum": np.ascontiguousarray(wsum.astype(f)),
            "qs8ks": np.ascontiguousarray(np.tile(
                8.0 * np.asarray(q_scale, f) * np.asarray(k_scale, f), 2)),
            "nkv": np.ascontiguousarray(np.tile(np.asarray(null_kv, f),
                                                (1, 2))),
            "nb": np.ascontiguousarray(np.exp(
                np.asarray(null_attn_bias, f)[hg * H_LOC:(hg + 1) * H_LOC])),
            "w0v": np.ascontiguousarray(np.asarray(w0, f).reshape(D)),
            "b0": np.ascontiguousarray(np.asarray(b0, f)),
            "g0": np.ascontiguousarray(np.asarray(g0, f)),
            "w1": np.ascontiguousarray(np.asarray(w1, f)).astype(bf),
            "b1": np.ascontiguousarray(np.asarray(b1, f)),
            "g1": np.ascontiguousarray(np.asarray(g1, f)),
            "w2": np.ascontiguousarray(np.asarray(w2, f)).astype(bf),
            "b2": np.ascontiguousarray(np.asarray(b2, f)),
            "wout": np.ascontiguousarray(
                np.asarray(Wout, f)[hg * INNER_LOC:(hg + 1) * INNER_LOC, :]
            ).astype(bf),
            "g_out": np.ascontiguousarray(np.asarray(g_out, f)),
            "posb": posb,
            "sel": np.eye(HEADS, dtype=f)[:, hg * H_LOC:(hg + 1) * H_LOC].copy(),
        }
        in_maps.append(m)
    return in_maps


_NC_CACHE = None


def kernel(**inputs):
    global _NC_CACHE
    from concourse.bass_utils import run_bass_kernel_spmd

    if _NC_CACHE is None:
        _NC_CACHE = build()
    nc = _NC_CACHE
    in_maps = make_in_maps(**inputs)
    res = run_bass_kernel_spmd(nc, in_maps, core_ids=list(range(N_CORES)))
    outs = res.results
    kernel.last_outs = outs
    full = np.empty((2, N, D), np.float32)
    for c in range(N_CORES):
        bi, hg = c // 4, c % 4
        o = np.asarray(outs[c]["out"], np.float32)
        for q in range(4):
            full[bi, q * 512 + hg * 128:q * 512 + (hg + 1) * 128, :] = \
                o[q * 128:(q + 1) * 128]
    return full


# revision 56
# speedup vs baseline: 1.4321x; 1.0078x over previous
import sys

for _p in ("/opt/trn_rl_repo", "/root/.axon_site/_ro/trn_rl_repo"):
    if _p not in sys.path:
        sys.path.insert(0, _p)

import numpy as np

from concourse import bacc, mybir, tile
import bass_rust

N_CORES = 8
N = 2048
D = 1024
HEADS = 16
DH = 64
H_LOC = 4          # heads per core
INNER_LOC = H_LOC * DH  # 256
QKV = INNER_LOC + 2 * DH  # 384 packed projection width
NEG = -1.0e30
EPS = 1e-5
F32 = mybir.dt.float32
F32R = mybir.dt.float32r
BF16 = mybir.dt.bfloat16
I32 = mybir.dt.int32

# rel-pos MLP sharding: 2048 useful reversed-position rows, 256 per core.
MLP_ROWS = 256
HFR_PAD = 64       # data lives at HFRD[64 : 64+2048]
HFRD_ROWS = 2752   # covers all reads [64, 2494]
MT_W = 2560        # master toeplitz width


def _ap(t, pattern, offset):
    a = t.ap().copy()
    a.ap = bass_rust.VecI64Pair(pattern)
    a.offset = offset
    return a


def _ln_stats(nc, pool, x_sb, width, sfx=""):
    """Row LayerNorm stats via one-pass bn_stats.

    Returns (mean, var, sd, rstd) where mean/var are slices of the bn_aggr
    output and sd/rstd are empty tiles for the caller's sqrt/recip."""
    nch = (width + 511) // 512
    stats = pool.tile([128, 6 * nch], F32, tag="ln_bns" + sfx)
    for c in range(nch):
        w = min(512, width - c * 512)
        nc.vector.bn_stats(out=stats[:, c * 6:(c + 1) * 6],
                           in_=x_sb[:, c * 512:c * 512 + w])
    mv = pool.tile([128, 2], F32, tag="ln_mv" + sfx)
    nc.vector.bn_aggr(out=mv[:, :], in_=stats[:, :])
    sd = pool.tile([128, 1], F32, tag="ln_sd" + sfx)
    rstd = pool.tile([128, 1], F32, tag="ln_rstd" + sfx)
    return mv[:, 0:1], mv[:, 1:2], sd, rstd


def build():
    build.NO_AV = globals().get('NO_AV', False)
    nc = bacc.Bacc("TRN2", target_bir_lowering=False, debug=False,
                   num_devices=N_CORES)

    # ---------------- parameters ----------------
    x_p = nc.declare_dram_parameter("x", [N, D], BF16, isOutput=False)
    wqkv_p = nc.declare_dram_parameter("wqkv", [D, QKV], BF16, isOutput=False)
    wsum_p = nc.declare_dram_parameter("wsum", [QKV], F32, isOutput=False)
    # qs8ks and null-k are passed partition-duplicated ([x | x] over 128
    # partitions) so odd heads can slice base-partition-64 operands.
    qs8ks_p = nc.declare_dram_parameter("qs8ks", [128], F32, isOutput=False)
    nkv_p = nc.declare_dram_parameter("nkv", [2, 128], F32, isOutput=False)
    nb_p = nc.declare_dram_parameter("nb", [H_LOC], F32, isOutput=False)
    w0_p = nc.declare_dram_parameter("w0v", [D], F32, isOutput=False)
    b0_p = nc.declare_dram_parameter("b0", [D], F32, isOutput=False)
    g0_p = nc.declare_dram_parameter("g0", [D], F32, isOutput=False)
    w1_p = nc.declare_dram_parameter("w1", [D, D], BF16, isOutput=False)
    b1_p = nc.declare_dram_parameter("b1", [D], F32, isOutput=False)
    g1_p = nc.declare_dram_parameter("g1", [D], F32, isOutput=False)
    w2_p = nc.declare_dram_parameter("w2", [D, HEADS], BF16, isOutput=False)
    b2_p = nc.declare_dram_parameter("b2", [HEADS], F32, isOutput=False)
    wout_p = nc.declare_dram_parameter("wout", [INNER_LOC, D], BF16,
                                       isOutput=False)
    gout_p = nc.declare_dram_parameter("g_out", [D], F32, isOutput=False)
    posb_p = nc.declare_dram_parameter("posb", [128], I32, isOutput=False)
    sel_p = nc.declare_dram_parameter("sel", [HEADS, H_LOC], F32, isOutput=False)
    out_p = nc.declare_dram_parameter("out", [N // 4, D], F32, isOutput=True)
    DBG = globals().get('DEBUG_TAPS', False)
    if DBG:
        dbg_qT = nc.declare_dram_parameter("dbg_qT", [128, N], BF16, isOutput=True)
        dbg_kT = nc.declare_dram_parameter("dbg_kT", [128, N], BF16, isOutput=True)
        dbg_v0 = nc.declare_dram_parameter("dbg_v0", [128, DH + 1], BF16, isOutput=True)
        dbg_avT = nc.declare_dram_parameter("dbg_avT", [DH, N], BF16, isOutput=True)
        dbg_po = nc.declare_dram_parameter("dbg_po", [128, D], BF16, isOutput=True)
        dbg_mt = nc.declare_dram_parameter("dbg_mt", [128, MT_W], BF16, isOutput=True)
        dbg_avps = nc.declare_dram_parameter("dbg_avps", [DH + 1, N], F32, isOutput=True)
        dbg_rb = nc.declare_dram_parameter("dbg_rb", [DH, N], BF16, isOutput=True)
        dbg_p4 = nc.declare_dram_parameter("dbg_p4", [128, 1024], BF16, isOutput=True)
        dbg_ps4 = nc.declare_dram_parameter("dbg_ps4", [128, 1024], F32, isOutput=True)

    # ---------------- internal DRAM ----------------
    hfr_loc = nc.dram_tensor("hfr_loc", [MLP_ROWS, HEADS], F32)
    hfr_g = nc.dram_tensor("hfr_g", [8 * MLP_ROWS, HEADS], F32)
    hfrd = nc.dram_tensor("hfrd", [H_LOC, HFRD_ROWS], BF16)
    po_q = [nc.dram_tensor(f"po_{i}", [N // 4, D], BF16) for i in range(4)]
    rs_q = [nc.dram_tensor(f"rs_{i}", [N // 16, D], BF16) for i in range(4)]

    dma = nc.sync.dma_start

    with tile.TileContext(nc) as tc:
        with (
            tc.tile_pool(name="const", bufs=1) as constp,
            tc.tile_pool(name="pers", bufs=1) as pers,
            tc.tile_pool(name="weights", bufs=1) as wp,
        ):
            ident = constp.tile([128, 128], F32)
            from concourse.masks import make_identity
            make_identity(nc, ident[:, :])
            identb = constp.tile([128, 128], BF16)
            nc.scalar.copy(out=identb[:, :], in_=ident[:, :])
            eps128 = constp.tile([128, 1], F32)
            nc.vector.memset(eps128[:, :], EPS)
            ones1 = constp.tile([1, 128], F32)
            nc.vector.memset(ones1[:, :], 1.0)

            # ---- all big input loads issued up front (single SP queue) ----
            early = tc.tile_pool(name="early", bufs=1)
            ep = early.__enter__()
            xall = ep.tile([128, 16 * D], BF16, name="xall")
            dma(out=xall[:, :], in_=_ap(x_p, [[D, 128], [128 * D, 16], [1, D]], 0))
            wqkv_sb = ep.tile([128, 8 * QKV], BF16)
            dma(out=wqkv_sb[:, :],
                in_=_ap(wqkv_p, [[QKV, 128], [128 * QKV, 8], [1, QKV]], 0))
            wout_sb = wp.tile([DH, 4 * D], BF16)
            dma(out=wout_sb[:, :],
                in_=_ap(wout_p, [[D, DH], [DH * D, 4], [1, D]], 0))
            w1_sb = ep.tile([128, 8 * D], BF16)  # chunk k at cols k*1024
            dma(out=w1_sb[:, :],
                in_=_ap(w1_p, [[D, 128], [128 * D, 8], [1, D]], 0))
            w2_sb = ep.tile([128, 8 * HEADS], BF16)
            dma(out=w2_sb[:, :],
                in_=_ap(w2_p, [[HEADS, 128], [128 * HEADS, 8], [1, HEADS]], 0))

            wsum_f32 = ep.tile([1, QKV], F32)
            dma(out=wsum_f32[:, :], in_=_ap(wsum_p, [[QKV, 1], [1, QKV]], 0))
            wsum_row = ep.tile([1, QKV], BF16)
            nc.vector.tensor_copy(out=wsum_row[:, :], in_=wsum_f32[:, :])
            qs8ks_sb = pers.tile([128, 1], F32)
            dma(out=qs8ks_sb[:, :], in_=_ap(qs8ks_p, [[1, 128], [1, 1]], 0))
            nkT = pers.tile([128, 1], F32)
            dma(out=nkT[:, :], in_=_ap(nkv_p, [[1, 128], [1, 1]], 0))
            nv_sb = pers.tile([1, DH], F32)
            dma(out=nv_sb[:, :], in_=nkv_p.ap()[1:2, 0:DH])
            nb_sb = pers.tile([1, H_LOC], F32)
            dma(out=nb_sb[:, :], in_=_ap(nb_p, [[H_LOC, 1], [1, H_LOC]], 0))
            posi_t = pers.tile([128, 1], I32, name="posi")
            dma(out=posi_t[:, :], in_=_ap(posb_p, [[1, 128], [1, 1]], 0))
            sel_sb = wp.tile([HEADS, H_LOC], F32)
            dma(out=sel_sb[:, :], in_=sel_p.ap())

            # [nv | 1] * exp(nb[h]) bf16 rows for null-key AV (nb_p holds
            # exp(null_attn_bias) so the bias add disappears into the AV)
            nv1f = pers.tile([1, DH + 1], F32)
            nc.vector.tensor_copy(out=nv1f[:, 0:DH], in_=nv_sb[:, :])
            nc.vector.memset(nv1f[:, DH:DH + 1], 1.0)
            nv1e = pers.tile([1, H_LOC * (DH + 1)], BF16, name="nv1e")
            for hh in range(H_LOC):
                nc.vector.tensor_scalar(
                    out=nv1e[:, hh * (DH + 1):(hh + 1) * (DH + 1)],
                    in0=nv1f[:, :], scalar1=nb_sb[:, hh:hh + 1], scalar2=None,
                    op0=mybir.AluOpType.mult)

            # ---------- Phases 0-2 interleaved: rel-pos MLP generator is ----
            # pumped between projection steps so its long serial chain fills
            # engine gaps instead of blocking the in-order queues.
            qT2 = pers.tile([128, 2 * N], BF16, name="qT2")
            kT = pers.tile([128, N], BF16, tag="kT", name="kT")
            v1 = [pers.tile([128, DH + 1], BF16, tag=f"v1_{j}", name=f"v1_{j}")
                  for j in range(16)]
            nkn = pers.tile([128, 1], BF16)

            with (
                tc.tile_pool(name="bc_ps", bufs=2, space="PSUM") as bpp,
                tc.tile_pool(name="vst", bufs=3) as vstp,
            ):
                bcast = ep.tile([128, 4 * D + HEADS], BF16)
                g1b = ep.tile([128, D], BF16)
                goutb = wp.tile([128, D], F32)
                chunks = []
                for pi, par in enumerate((w0_p, b0_p, g0_p, b1_p)):
                    for half in range(2):
                        chunks.append((par, half * 512, 512, bcast,
                                       pi * D + half * 512))
                chunks.append((b2_p, 0, HEADS, bcast, 4 * D))
                for half in range(2):
                    chunks.append((g1_p, half * 512, 512, g1b, half * 512))
                    chunks.append((gout_p, half * 512, 512, goutb,
                                   half * 512))
                for par, poff, wdt, dst, doff in chunks:
                    vstage = vstp.tile([1, 512], F32, tag="vstage")
                    dma(out=vstage[:, 0:wdt],
                        in_=_ap(par, [[wdt, 1], [1, wdt]], poff))
                    ps = bpp.tile([128, 512], F32, tag="bc")
                    nc.tensor.matmul(out=ps[:, 0:wdt],
                                     lhsT=ones1[:, :],
                                     rhs=vstage[:, 0:wdt],
                                     start=True, stop=True)
                    nc.scalar.copy(out=dst[:, doff:doff + wdt],
                                   in_=ps[:, 0:wdt])
            w0b = bcast[:, 0:D]
            b0b = bcast[:, D:2 * D]
            g0b = bcast[:, 2 * D:3 * D]
            b1b = bcast[:, 3 * D:4 * D]
            b2b = bcast[:, 4 * D:4 * D + HEADS]

            with (
                tc.tile_pool(name="mlp", bufs=1) as mp,
                tc.tile_pool(name="mlp_ps", bufs=1, space="PSUM") as mpp,
                tc.tile_pool(name="mlp_ps2", bufs=1, space="PSUM") as mpp2,
                tc.tile_pool(name="xT", bufs=1) as xTp,
                tc.tile_pool(name="xt", bufs=2) as xtp,
                tc.tile_pool(name="xps", bufs=2, space="PSUM") as xpp,
                tc.tile_pool(name="xps2", bufs=1, space="PSUM") as xpp2,
                tc.tile_pool(name="xps3", bufs=1, space="PSUM") as xpp3,
            ):
                xT = xTp.tile([128, 8 * N], BF16)  # d-chunk k at cols k*2048
                negrow = wp.tile([1, N], BF16, name="negrow")

                def mlp_gen(t):
                    if True:
                        m0 = t * 128
                        posf = mp.tile([128, 1], F32, tag=f"posf{t}",
                                       name=f"posf{t}")
                        nc.vector.tensor_scalar_add(out=posf[:, :],
                                                    in0=posi_t[:, :],
                                                    scalar1=float(-m0))
                        h0 = mp.tile([128, D], F32, tag=f"h0{t}", name=f"h0_{t}")
                        nc.vector.tensor_scalar(out=h0[:, :], in0=w0b,
                                                scalar1=posf[:, :], scalar2=None,
                                                op0=mybir.AluOpType.mult)
                        nc.vector.tensor_tensor(out=h0[:, :], in0=h0[:, :],
                                                in1=b0b,
                                                op=mybir.AluOpType.add)
                        yield
                        mean, var, sd, rstd = _ln_stats(nc, mp, h0[:, :], D, sfx=str(t))
                        nc.scalar.activation(
                            out=sd[:, :], in_=var,
                            func=mybir.ActivationFunctionType.Sqrt,
                            bias=eps128[:, :])
                        nc.vector.reciprocal_approx_fast(out=rstd[:, :],
                                                         in_=sd[:, :])
                        yield
                        nc.vector.tensor_scalar(out=h0[:, :], in0=h0[:, :],
                                                scalar1=mean,
                                                scalar2=rstd[:, :],
                                                op0=mybir.AluOpType.subtract,
                                                op1=mybir.AluOpType.mult)
                        nc.vector.tensor_tensor(out=h0[:, :], in0=h0[:, :],
                                                in1=g0b,
                                                op=mybir.AluOpType.mult)
                        h0b = mp.tile([128, D], BF16, tag=f"h0b{t}",
                                      name=f"h0b{t}")
                        nc.scalar.activation(
                            out=h0b[:, :], in_=h0[:, :],
                            func=mybir.ActivationFunctionType.Silu)
                        yield
                        h0T = mp.tile([128, D], BF16, tag=f"h0T{t}",
                                      name=f"h0T{t}")
                        pst8m = mpp2.tile([128, D], BF16, tag="tp",
                                          name=f"tp0_{t}")
                        for k in range(8):
                            nc.tensor.matmul(
                                out=pst8m[:, k * 128:(k + 1) * 128],
                                lhsT=h0b[:, k * 128:(k + 1) * 128],
                                rhs=identb[:, :], is_transpose=True,
                                start=True, stop=True)
                            if k == 3:
                                yield
                        nc.vector.tensor_copy(out=h0T[:, :], in_=pst8m[:, :])
                        yield
                        h1 = mp.tile([128, D], F32, tag=f"h0{t}", name=f"h1_{t}")
                        for eb in range(2):
                            ps = mpp.tile([128, 512], F32, tag="h1ps",
                                          name=f"h1ps{t}_{eb}")
                            for k in range(8):
                                nc.tensor.matmul(
                                    out=ps[:, :],
                                    lhsT=h0T[:, k * 128:(k + 1) * 128],
                                    rhs=w1_sb[:, k * D + eb * 512:
                                              k * D + eb * 512 + 512],
                                    start=(k == 0), stop=(k == 7))
                            nc.vector.tensor_tensor(
                                out=h1[:, eb * 512:eb * 512 + 512],
                                in0=ps[:, :],
                                in1=b1b[:, eb * 512:eb * 512 + 512],
                                op=mybir.AluOpType.add)
                            yield
                        mean, var, sd, rstd = _ln_stats(nc, mp, h1[:, :], D, sfx=str(t))
                        nc.scalar.activation(
                            out=sd[:, :], in_=var,
                            func=mybir.ActivationFunctionType.Sqrt,
                            bias=eps128[:, :])
                        nc.vector.reciprocal_approx_fast(out=rstd[:, :],
                                                         in_=sd[:, :])
                        yield
                        nc.vector.tensor_scalar(out=h1[:, :], in0=h1[:, :],
                                                scalar1=mean,
                                                scalar2=rstd[:, :],
                                                op0=mybir.AluOpType.subtract,
                                                op1=mybir.AluOpType.mult)
                        nc.vector.tensor_tensor(out=h1[:, :], in0=h1[:, :],
                                                in1=g1b,
                                                op=mybir.AluOpType.mult)
                        h1b = mp.tile([128, D], BF16, tag=f"h0b{t}",
                                      name=f"h1b{t}")
                        nc.scalar.activation(
                            out=h1b[:, :], in_=h1[:, :],
                            func=mybir.ActivationFunctionType.Silu)
                        yield
                        h1T = mp.tile([128, D], BF16, tag=f"h0T{t}",
                                      name=f"h1T{t}")
                        pst8n = mpp2.tile([128, D], BF16, tag="tp",
                                          name=f"tp1_{t}")
                        for k in range(8):
                            nc.tensor.matmul(
                                out=pst8n[:, k * 128:(k + 1) * 128],
                                lhsT=h1b[:, k * 128:(k + 1) * 128],
                                rhs=identb[:, :], is_transpose=True,
                                start=True, stop=True)
                            if k == 3:
                                yield
                        nc.vector.tensor_copy(out=h1T[:, :], in_=pst8n[:, :])
                        yield
                        psf = mpp2.tile([128, HEADS], F32, tag="hf",
                                        name=f"hf{t}")
                        for k in range(8):
                            nc.tensor.matmul(
                                out=psf[:, :],
                                lhsT=h1T[:, k * 128:(k + 1) * 128],
                                rhs=w2_sb[:, k * HEADS:(k + 1) * HEADS],
                                start=(k == 0), stop=(k == 7))
                        hfc = mp.tile([128, HEADS], F32, tag=f"hfc{t}",
                                      name=f"hfc{t}")
                        nc.vector.tensor_tensor(out=hfc[:, :], in0=psf[:, :],
                                                in1=b2b,
                                                op=mybir.AluOpType.add)
                        dma(out=hfr_loc.ap()[m0:m0 + 128, :], in_=hfc[:, :])
                        yield

                mgens = [mlp_gen(0), mlp_gen(1)]
                mstate = {"done": 0, "ag": False}

                def pump(n=1):
                    for _ in range(n):
                        alive = False
                        for g in mgens:
                            try:
                                next(g)
                                alive = True
                            except StopIteration:
                                pass
                        if not alive and not mstate["ag"]:
                            mstate["ag"] = True
                            nc.gpsimd.collective_compute(
                                "AllGather", mybir.AluOpType.bypass,
                                replica_groups=[list(range(N_CORES))],
                                ins=[hfr_loc.ap().opt()],
                                outs=[hfr_g.ap().opt()],
                            )
                            return

                for tt in range(16):
                    xs = xall[:, tt * D:(tt + 1) * D]
                    # stats (one-pass bn_stats; wsum carries -colsum so the
                    # rank-1 correction uses the mean row directly)
                    mean, var, sd, rstd = _ln_stats(nc, xtp, xs, D)
                    nc.scalar.activation(out=sd[:, :], in_=var,
                                         func=mybir.ActivationFunctionType.Sqrt,
                                         bias=eps128[:, :])
                    nc.vector.reciprocal_approx_fast(out=rstd[:, :],
                                                     in_=sd[:, :])
                    psr = xpp3.tile([128, 128], F32, tag="misc",
                                    name=f"psr{tt}")
                    nc.tensor.matmul(out=psr[0:1, :], lhsT=mean,
                                     rhs=ident[:, :], is_transpose=True,
                                     start=True, stop=True)
                    nc.vector.tensor_copy(out=negrow[:, tt * 128:tt * 128 + 128],
                                          in_=psr[0:1, :])
                    pump(1)
                    # x transposes (bf16): 8 into one psum tile, one fat copy
                    pst8 = xpp2.tile([128, D], BF16, tag="tp8",
                                     name=f"pst8_{tt}")
                    for k in range(8):
                        nc.tensor.matmul(out=pst8[:, k * 128:(k + 1) * 128],
                                         lhsT=xall[:, tt * D + k * 128:
                                                   tt * D + k * 128 + 128],
                                         rhs=identb[:, :], is_transpose=True,
                                         start=True, stop=True)
                    xTo = xT[:, :].copy()
                    xpat = [list(p) for p in xTo.ap.to_list()]
                    xpat = [xpat[0], [N, 8], [1, 128]]
                    xTo.ap = bass_rust.VecI64Pair(xpat)
                    xTo.offset = xTo.offset + tt * 128
                    nc.vector.tensor_copy(out=xTo, in_=pst8[:, :])
                    pump(1)
                    # packed q|k|v projection with rank-1 mean correction
                    psq = xpp.tile([128, QKV], F32, tag="qkv")
                    for k in range(8):
                        nc.tensor.matmul(
                            out=psq[:, :],
                            lhsT=xT[:, k * N + tt * 128:k * N + tt * 128 + 128],
                            rhs=wqkv_sb[:, k * QKV:(k + 1) * QKV],
                            start=(k == 0), stop=False,
                            skip_group_check=True)
                    nc.tensor.matmul(out=psq[:, :],
                                     lhsT=negrow[:, tt * 128:tt * 128 + 128],
                                     rhs=wsum_row[:, :],
                                     start=False, stop=True,
                                     skip_group_check=True)
                    # per-head l2 norms (4 q heads + k)
                    nrm = xtp.tile([128, 8], F32, tag="nrm")
                    scr2 = xtp.tile([128, DH], BF16, tag="scr2")
                    for j in range(5):
                        nc.scalar.activation(
                            out=scr2[:, :],
                            in_=psq[:, j * DH:(j + 1) * DH],
                            func=mybir.ActivationFunctionType.Square,
                            accum_out=nrm[:, j:j + 1])
                    sd5 = xtp.tile([128, 8], F32, tag="sd5")
                    rinv = xtp.tile([128, 8], F32, tag="rinv")
                    nc.scalar.activation(out=sd5[:, 0:5], in_=nrm[:, 0:5],
                                         func=mybir.ActivationFunctionType.Sqrt)
                    nc.vector.reciprocal(out=rinv[:, 0:5], in_=sd5[:, 0:5])
                    pump(1)
                    # scaled copies out of PSUM
                    qn = xtp.tile([128, INNER_LOC], BF16, tag="qn")
                    for h in range(4):
                        eng = nc.vector if h % 2 == 0 else None
                        if h % 2 == 0:
                            nc.vector.tensor_scalar(
                                out=qn[:, h * DH:(h + 1) * DH],
                                in0=psq[:, h * DH:(h + 1) * DH],
                                scalar1=rinv[:, h:h + 1], scalar2=None,
                                op0=mybir.AluOpType.mult)
                        else:
                            nc.scalar.activation(
                                out=qn[:, h * DH:(h + 1) * DH],
                                in_=psq[:, h * DH:(h + 1) * DH],
                                func=mybir.ActivationFunctionType.Copy,
                                scale=rinv[:, h:h + 1])
                    # kn duplicated into both column halves so the transpose
                    # yields kT stacked twice along partitions
                    kn = xtp.tile([128, 128], BF16, tag="kn")
                    for kh in range(2):
                        nc.vector.tensor_scalar(
                            out=kn[:, kh * DH:(kh + 1) * DH],
                            in0=psq[:, INNER_LOC:INNER_LOC + DH],
                            scalar1=rinv[:, 4:5], scalar2=None,
                            op0=mybir.AluOpType.mult)
                    nc.vector.tensor_scalar(out=v1[tt][:, 0:DH],
                                            in0=psq[:, INNER_LOC + DH:QKV],
                                            scalar1=rstd[:, :], scalar2=None,
                                            op0=mybir.AluOpType.mult)
                    nc.vector.memset(v1[tt][:, DH:DH + 1], 1.0)
                    # q pair + k transposes into one psum tile
                    pstqk = xpp2.tile([128, 384], BF16, tag="tpqk",
                                      name=f"pstqk{tt}")
                    for p in range(2):
                        nc.tensor.matmul(out=pstqk[:, p * 128:(p + 1) * 128],
                                         lhsT=qn[:, p * 128:(p + 1) * 128],
                                         rhs=identb[:, :], is_transpose=True,
                                         start=True, stop=True)
                    nc.tensor.matmul(out=pstqk[:, 256:384], lhsT=kn[:, :],
                                     rhs=identb[:, :], is_transpose=True,
                                     start=True, stop=True)
                    qTo = qT2[:, :].copy()
                    qpat = [list(p) for p in qTo.ap.to_list()]
                    qpat = [qpat[0], [N, 2], [1, 128]]
                    qTo.ap = bass_rust.VecI64Pair(qpat)
                    qTo.offset = qTo.offset + tt * 128
                    nc.scalar.copy(out=qTo, in_=pstqk[:, 0:256])
                    # k transpose with qs8ks scale folded in
                    nc.vector.tensor_scalar(out=kT[:, tt * 128:tt * 128 + 128],
                                            in0=pstqk[:, 256:384],
                                            scalar1=qs8ks_sb[:, :], scalar2=None,
                                            op0=mybir.AluOpType.mult)
                    pump(1)

                pump(100)

                if DBG:
                    dma(out=dbg_qT.ap(), in_=qT2[:, 0:N])
                    dma(out=dbg_kT.ap(), in_=kT[:, :])
                    dma(out=dbg_v0.ap(), in_=v1[0][:, :])

                # null key normalize: nkn = l2norm(nk) * qs8ks  (dup over 128)
                ones64c_f = constp.tile([DH, 1], F32)
                nc.vector.memset(ones64c_f[:, :], 1.0)
                nsq = xtp.tile([128, 1], F32, tag="nsq")
                nc.scalar.activation(out=nsq[:, :], in_=nkT[:, :],
                                     func=mybir.ActivationFunctionType.Square)
                psn1 = xpp3.tile([128, 128], F32, tag="misc", name="psn1")
                nc.tensor.matmul(out=psn1[0:1, 0:1], lhsT=ones64c_f[:, :],
                                 rhs=nsq[0:DH, :], start=True, stop=True)
                rn1 = xtp.tile([1, 1], F32, tag="rn1")
                nc.scalar.activation(out=rn1[:, :], in_=psn1[0:1, 0:1],
                                     func=mybir.ActivationFunctionType.Sqrt)
                with nc.allow_low_precision(reason="f32r same bits as f32"):
                    nc.vector.reciprocal(out=rn1[:, :], in_=rn1[:, :])
                psb1 = xpp3.tile([128, 128], F32, tag="misc", name="psb1")
                nc.tensor.matmul(out=psb1[:, 0:1], lhsT=ones1[:, :],
                                 rhs=rn1[:, :], start=True, stop=True)
                nc.vector.tensor_tensor(out=nkn[:, :], in0=nkT[:, :],
                                        in1=psb1[:, 0:1],
                                        op=mybir.AluOpType.mult)
                nc.vector.tensor_scalar(out=nkn[:, :], in0=nkn[:, :],
                                        scalar1=qs8ks_sb[:, :], scalar2=None,
                                        op0=mybir.AluOpType.mult)

                # ---- stage AllGathered MLP rows -> hfrd (batched) ----
                stg = xtp.tile([128, 16 * HEADS], F32, tag="stg",
                               name="stg_all")
                dma(out=stg[:, :],
                    in_=_ap(hfr_g, [[HEADS, 128], [128 * HEADS, 16],
                                    [1, HEADS]], 0))
                stgT = xTp.tile([HEADS, 16 * 128], BF16, name="stgT")
                selb = xtp.tile([HEADS, H_LOC], BF16, tag="selb",
                                name="selb")
                nc.vector.tensor_copy(out=selb[:, :], in_=sel_sb[:, :])
                for chunk in range(16):
                    pss = xpp.tile([128, QKV], F32, tag="qkv",
                                   name=f"stgps{chunk}")
                    nc.tensor.matmul(out=pss[0:HEADS, 0:128],
                                     lhsT=stg[:, chunk * HEADS:
                                              (chunk + 1) * HEADS],
                                     rhs=ident[:, :], is_transpose=True,
                                     start=True, stop=True)
                    eng = nc.scalar if chunk % 2 == 0 else nc.vector
                    if chunk % 2 == 0:
                        nc.scalar.copy(
                            out=stgT[:, chunk * 128:(chunk + 1) * 128],
                            in_=pss[0:HEADS, 0:128])
                    else:
                        nc.vector.tensor_copy(
                            out=stgT[:, chunk * 128:(chunk + 1) * 128],
                            in_=pss[0:HEADS, 0:128])
                # select local heads and write hfrd in 512-col pieces
                for piece in range(4):
                    psl = xpp3.tile([128, 128], F32, tag="misc",
                                    name=f"psl{piece}")
                    stl = xtp.tile([H_LOC, 512], BF16, tag="stl")
                    for sub in range(4):
                        col = piece * 512 + sub * 128
                        nc.tensor.matmul(out=psl[0:H_LOC, 0:128],
                                         lhsT=selb[:, :],
                                         rhs=stgT[:, col:col + 128],
                                         start=True, stop=True)
                        if sub % 2 == 0:
                            nc.scalar.copy(out=stl[:, sub * 128:sub * 128 + 128],
                                           in_=psl[0:H_LOC, 0:128])
                        else:
                            nc.vector.tensor_copy(
                                out=stl[:, sub * 128:sub * 128 + 128],
                                in_=psl[0:H_LOC, 0:128])
                    dma(out=_ap(hfrd, [[HFRD_ROWS, H_LOC], [1, 512]],
                                HFR_PAD + piece * 512),
                        in_=stl[:, :])
                poison = xtp.tile([H_LOC, HFRD_ROWS - 2112], BF16,
                                  name="poison")
                nc.vector.memset(poison[:, :], NEG)
                dma(out=_ap(hfrd, [[HFRD_ROWS, H_LOC],
                                   [1, HFRD_ROWS - 2112]], 2112),
                    in_=poison[:, :])

            early.__exit__(None, None, None)

            # ---------- Phase 3: attention + per-quarter out-proj + RS ------
            avT = [pers.tile([DH, N], BF16, tag=f"avT{h}", name=f"avT{h}")
                   for h in range(H_LOC)]
            mt = [pers.tile([128, MT_W], BF16, tag=f"mt{h}", name=f"mt{h}")
                  for h in range(H_LOC)]
            for h in range(H_LOC):
                dma(out=mt[h][:, :],
                    in_=_ap(hfrd, [[1, 128], [1, MT_W]], h * HFRD_ROWS + 63))
            if build.NO_AV:
                for h in range(H_LOC):
                    nc.vector.memset(avT[h][:, :], 0.0)

            with (
                tc.tile_pool(name="at", bufs=3) as atp,
                tc.tile_pool(name="sim4", bufs=2, space="PSUM") as simpp,
                tc.tile_pool(name="avps", bufs=2, space="PSUM") as avpp,
                tc.tile_pool(name="tps", bufs=1, space="PSUM") as tpp,
                tc.tile_pool(name="oq", bufs=2) as oqp,
            ):
                ones65 = atp.tile([DH + 1, DH], F32, tag="ones65",
                                  name="ones65")
                nc.vector.memset(ones65[:, :], 1.0)
                pend_tail = [None]

                def run_tail():
                    if pend_tail[0] is not None:
                        pend_tail[0]()
                        pend_tail[0] = None

                for q in range(4):
                    for m in (2 * q, 2 * q + 1):
                        i0 = m * 256
                        njt = 2 * m + 2
                        for h in range(H_LOC):
                            hp = (h % 2) * DH
                            qh = qT2[hp:hp + DH,
                                     (h // 2) * N + i0:(h // 2) * N + i0 + 256]
                            av_ps = avpp.tile([DH + 1, 256], F32, tag="av",
                                              name=f"av_{m}_{h}")
                            GSZ = 4
                            groups = [list(range(g, min(g + GSZ, njt)))
                                      for g in range(0, njt, GSZ)]
                            if len(groups[-1]) == GSZ:
                                # keep a spare exp column chunk for the
                                # null-key logits in the final group
                                groups[-1] = groups[-1][:GSZ - 1]
                                groups.append([njt - 1])
                            pend_av = None
                            av_state = [False]

                            def issue_av(pend, av_ps=av_ps, av_state=av_state):
                                pp4, pjts = pend
                                for ji, jt in enumerate(pjts):
                                    nc.tensor.matmul(
                                        out=av_ps[:, :],
                                        lhsT=v1[jt][:, :],
                                        rhs=pp4[:, ji * 256:ji * 256 + 256],
                                        start=(not av_state[0]), stop=False,
                                        skip_group_check=True)
                                    av_state[0] = True

                            for gi, jts in enumerate(groups):
                                gw = 256 * len(jts)
                                last = (gi == len(groups) - 1)
                                ps4 = simpp.tile([128, 1024], F32, tag="sim")
                                for ji, jt in enumerate(jts):
                                    j0 = jt * 128
                                    c0 = ji * 256
                                    # start=True only on the first chunk of
                                    # each 2KB psum bank: a start arms
                                    # zero-on-first-write for the whole bank
                                    nc.tensor.matmul(
                                        out=ps4[:, c0:c0 + 256],
                                        lhsT=kT[hp:hp + DH, j0:j0 + 128],
                                        rhs=qh,
                                        start=(c0 % 512 == 0), stop=False,
                                        skip_group_check=True)
                                # Toeplitz bias adds: two j-tiles merged per
                                # matmul via a 3D shifted AP (second touch of
                                # the armed bank, so plain accumulate)
                                for c0 in range(0, gw, 512):
                                    cn = min(2, (gw - c0) // 256)
                                    jt0 = jts[c0 // 256]
                                    u0 = 2048 - i0 + jt0 * 128
                                    mtr = mt[h][:, :].copy()
                                    pat = [list(p) for p in mtr.ap.to_list()]
                                    pat[1] = [128, cn]
                                    pat.append([-1, 256])
                                    mtr.ap = bass_rust.VecI64Pair(pat)
                                    mtr.offset = mtr.offset + u0
                                    nc.tensor.matmul(
                                        out=ps4[:, c0:c0 + cn * 256],
                                        lhsT=identb[:, :], rhs=mtr,
                                        start=False, stop=True,
                                        skip_group_check=True)
                                ew = gw
                                if last:
                                    # null-key logits ride along in the spare
                                    # columns of the final (partial) group
                                    nc.tensor.matmul(
                                        out=ps4[0:1, gw:gw + 256],
                                        lhsT=nkn[hp:hp + DH, :], rhs=qh,
                                        start=True, stop=True,
                                        skip_group_check=True)
                                    ew = gw + 256
                                p4 = atp.tile([128, 1024], BF16, tag="p4")
                                nc.scalar.activation(
                                    out=p4[:, 0:ew], in_=ps4[:, 0:ew],
                                    func=mybir.ActivationFunctionType.Exp)
                                # software pipeline: issue deferred work now so
                                # the PE queue never parks waiting on this exp
                                if gi == 0:
                                    run_tail()
                                else:
                                    issue_av(pend_av)
                                pend_av = (p4, jts)

                            def tail(h=h, i0=i0, av_ps=av_ps, pend_av=pend_av,
                                     issue_av=issue_av,
                                     gw_last=256 * len(groups[-1])):
                                issue_av(pend_av)
                                nc.tensor.matmul(
                                    out=av_ps[:, :],
                                    lhsT=nv1e[:, h * (DH + 1):
                                              (h + 1) * (DH + 1)],
                                    rhs=pend_av[0][0:1, gw_last:gw_last + 256],
                                    start=False, stop=True,
                                    skip_group_check=True)
                                # normalize columns by row-64 sums -> avT[h].
                                # full-height recip: base-partition-64 DVE
                                # slices silently no-op; only row 64 is read
                                # by the selector matmul below
                                rr = atp.tile([DH + 1, 256], F32, tag="rr")
                                nc.vector.reciprocal_approx_fast(
                                    out=rr[:, :], in_=av_ps[:, :])
                                psb = tpp.tile([DH, 256], F32, tag="bc")
                                nc.tensor.matmul(out=psb[:, :],
                                                 lhsT=ones65[DH:DH + 1, 0:DH],
                                                 rhs=rr[DH:DH + 1, :],
                                                 start=True, stop=True)
                                rb = atp.tile([DH, 256], BF16, tag="rb")
                                nc.scalar.copy(out=rb[:, :], in_=psb[:, :])
                                nc.vector.tensor_tensor(
                                    out=avT[h][:, i0:i0 + 256],
                                    in0=av_ps[0:DH, :], in1=rb[:, :],
                                    op=mybir.AluOpType.mult)
                            pend_tail[0] = tail

                    # out projection for this quarter
                    run_tail()
                    for tl in range(4):
                        tt = q * 4 + tl
                        ps_po = simpp.tile([128, 1024], F32, tag="sim")
                        for eb in range(2):
                            for ch in range(H_LOC):
                                nc.tensor.matmul(
                                    out=ps_po[:, eb * 512:eb * 512 + 512],
                                    lhsT=avT[ch][:, tt * 128:tt * 128 + 128],
                                    rhs=wout_sb[:, ch * D + eb * 512:
                                                ch * D + eb * 512 + 512],
                                    start=(ch == 0), stop=(ch == H_LOC - 1),
                                    skip_group_check=True)
                        po_sb = oqp.tile([128, D], BF16, tag="po")
                        nc.vector.tensor_copy(out=po_sb[:, :], in_=ps_po[:, :])
                        dma(out=po_q[q].ap()[tl * 128:(tl + 1) * 128, :],
                            in_=po_sb[:, :])
                    nc.gpsimd.collective_compute(
                        "ReduceScatter", mybir.AluOpType.add,
                        replica_groups=[[0, 1, 2, 3], [4, 5, 6, 7]],
                        ins=[po_q[q].ap().opt()],
                        outs=[rs_q[q].ap().opt()],
                    )
                    # final LN for the 128 owned rows of this quarter
                    y = oqp.tile([128, D], BF16, tag="y")
                    dma(out=y[:, :], in_=rs_q[q].ap()[:, :])
                    mean, var, sd, rstd = _ln_stats(nc, oqp, y[:, :], D)
                    nc.scalar.activation(out=sd[:, :], in_=var,
                                         func=mybir.ActivationFunctionType.Sqrt,
                                         bias=eps128[:, :])
                    nc.vector.reciprocal_approx_fast(out=rstd[:, :], in_=sd[:, :])
                    yf = oqp.tile([128, D], F32, tag="yf")
                    nc.vector.tensor_scalar(out=yf[:, :], in0=y[:, :],
                                            scalar1=mean,
                                            scalar2=rstd[:, :],
                                            op0=mybir.AluOpType.subtract,
                                            op1=mybir.AluOpType.mult)
                    nc.vector.tensor_tensor(out=yf[:, :], in0=yf[:, :],
                                            in1=goutb,
                                            op=mybir.AluOpType.mult)
                    dma(out=out_p.ap()[q * 128:(q + 1) * 128, :], in_=yf[:, :])
                if DBG:
                    dma(out=dbg_avT.ap(), in_=avT[0][:, :])
                    dma(out=dbg_mt.ap(), in_=mt[0][:, :])
                    dma(out=dbg_po.ap(), in_=po_q[0].ap()[0:128, :])

    nc.compile()
    return nc


def make_in_maps(x, mask, g_norm, Wq, Wkv, q_scale, k_scale, null_kv,
                 null_attn_bias, w0, b0, g0, w1, b1, g1, w2, b2, Wout, g_out):
    import ml_dtypes
    assert bool(np.asarray(mask).all()), "kernel assumes all-True mask"
    f = np.float32
    bf = ml_dtypes.bfloat16
    gn = np.asarray(g_norm, f)
    in_maps = []
    for c in range(N_CORES):
        bi, hg = c // 4, c % 4
        posb = (2047 - c * MLP_ROWS - np.arange(128)).astype(np.int32)
        wq_hg = np.asarray(Wq, f)[:, hg * INNER_LOC:(hg + 1) * INNER_LOC]
        wqkv = np.concatenate(
            [wq_hg, np.asarray(Wkv, f)], axis=1) * gn[:, None]
        wsum = -(wqkv.sum(axis=0))
        m = {
            "x": np.ascontiguousarray(np.asarray(x, f)[bi]).astype(bf),
            "wqkv": np.ascontiguousarray(wqkv).astype(bf),
            "wsum": np.ascontiguousarray(wsum.astype(f)),
            "qs8ks": np.ascontiguousarray(np.tile(
                8.0 * np.asarray(q_scale, f) * np.asarray(k_scale, f), 2)),
            "nkv": np.ascontiguousarray(np.tile(np.asarray(null_kv, f),
                                                (1, 2))),
            "nb": np.ascontiguousarray(np.exp(
                np.asarray(null_attn_bias, f)[hg * H_LOC:(hg + 1) * H_LOC])),
            "w0v": np.ascontiguousarray(np.asarray(w0, f).reshape(D)),
            "b0": np.ascontiguousarray(np.asarray(b0, f)),
            "g0": np.ascontiguousarray(np.asarray(g0, f)),
            "w1": np.ascontiguousarray(np.asarray(w1, f)).astype(bf),
            "b1": np.ascontiguousarray(np.asarray(b1, f)),
            "g1": np.ascontiguousarray(np.asarray(g1, f)),
            "w2": np.ascontiguousarray(np.asarray(w2, f)).astype(bf),
            "b2": np.ascontiguousarray(np.asarray(b2, f)),
            "wout": np.ascontiguousarray(
                np.asarray(Wout, f)[hg * INNER_LOC:(hg + 1) * INNER_LOC, :]
            ).astype(bf),
            "g_out": np.ascontiguousarray(np.asarray(g_out, f)),
            "posb": posb,
            "sel": np.eye(HEADS, dtype=f)[:, hg * H_LOC:(hg + 1) * H_LOC].copy(),
        }
        in_maps.append(m)
    return in_maps


_NC_CACHE = None


def kernel(**inputs):
    global _NC_CACHE
    from concourse.bass_utils import run_bass_kernel_spmd

    if _NC_CACHE is None:
        _NC_CACHE = build()
    nc = _NC_CACHE
    in_maps = make_in_maps(**inputs)
    res = run_bass_kernel_spmd(nc, in_maps, core_ids=list(range(N_CORES)))
    outs = res.results
    kernel.last_outs = outs
    full = np.empty((2, N, D), np.float32)
    for c in range(N_CORES):
        bi, hg = c // 4, c % 4
        o = np.asarray(outs[c]["out"], np.float32)
        for q in range(4):
            full[bi, q * 512 + hg * 128:q * 512 + (hg + 1) * 128, :] = \
                o[q * 128:(q + 1) * 128]
    return full


# revision 66
# speedup vs baseline: 1.4665x; 1.0240x over previous
import sys

for _p in ("/opt/trn_rl_repo", "/root/.axon_site/_ro/trn_rl_repo"):
    if _p not in sys.path:
        sys.path.insert(0, _p)

import numpy as np

from concourse import bacc, mybir, tile
import bass_rust

N_CORES = 8
N = 2048
D = 1024
HEADS = 16
DH = 64
H_LOC = 4          # heads per core
INNER_LOC = H_LOC * DH  # 256
QKV = INNER_LOC + 2 * DH  # 384 packed projection width
NEG = -1.0e30
EPS = 1e-5
F32 = mybir.dt.float32
F32R = mybir.dt.float32r
BF16 = mybir.dt.bfloat16
I32 = mybir.dt.int32

# rel-pos MLP sharding: 2048 useful reversed-position rows, 256 per core.
MLP_ROWS = 256
HFR_PAD = 64       # data lives at HFRD[64 : 64+2048]
HFRD_ROWS = 2752   # covers all reads [64, 2494]
MT_W = 2560        # master toeplitz width


def _ap(t, pattern, offset):
    a = t.ap().copy()
    a.ap = bass_rust.VecI64Pair(pattern)
    a.offset = offset
    return a


def _ln_stats(nc, pool, x_sb, width, sfx=""):
    """Row LayerNorm stats via one-pass bn_stats.

    Returns (mean, var, sd, rstd) where mean/var are slices of the bn_aggr
    output and sd/rstd are empty tiles for the caller's sqrt/recip."""
    nch = (width + 511) // 512
    stats = pool.tile([128, 6 * nch], F32, tag="ln_bns" + sfx)
    for c in range(nch):
        w = min(512, width - c * 512)
        nc.vector.bn_stats(out=stats[:, c * 6:(c + 1) * 6],
                           in_=x_sb[:, c * 512:c * 512 + w])
    mv = pool.tile([128, 2], F32, tag="ln_mv" + sfx)
    nc.vector.bn_aggr(out=mv[:, :], in_=stats[:, :])
    sd = pool.tile([128, 1], F32, tag="ln_sd" + sfx)
    rstd = pool.tile([128, 1], F32, tag="ln_rstd" + sfx)
    return mv[:, 0:1], mv[:, 1:2], sd, rstd


def build():
    build.NO_AV = globals().get('NO_AV', False)
    nc = bacc.Bacc("TRN2", target_bir_lowering=False, debug=False,
                   num_devices=N_CORES)

    # ---------------- parameters ----------------
    x_p = nc.declare_dram_parameter("x", [N, D], BF16, isOutput=False)
    wqkv_p = nc.declare_dram_parameter("wqkv", [D, QKV], BF16, isOutput=False)
    wsum_p = nc.declare_dram_parameter("wsum", [QKV], F32, isOutput=False)
    # qs8ks and null-k are passed partition-duplicated ([x | x] over 128
    # partitions) so odd heads can slice base-partition-64 operands.
    qs8ks_p = nc.declare_dram_parameter("qs8ks", [128], F32, isOutput=False)
    nkv_p = nc.declare_dram_parameter("nkv", [2, 128], F32, isOutput=False)
    nb_p = nc.declare_dram_parameter("nb", [H_LOC], F32, isOutput=False)
    w0_p = nc.declare_dram_parameter("w0v", [D], F32, isOutput=False)
    b0_p = nc.declare_dram_parameter("b0", [D], F32, isOutput=False)
    g0_p = nc.declare_dram_parameter("g0", [D], F32, isOutput=False)
    w1_p = nc.declare_dram_parameter("w1", [D, D], BF16, isOutput=False)
    b1_p = nc.declare_dram_parameter("b1", [D], F32, isOutput=False)
    g1_p = nc.declare_dram_parameter("g1", [D], F32, isOutput=False)
    w2_p = nc.declare_dram_parameter("w2", [D, HEADS], BF16, isOutput=False)
    b2_p = nc.declare_dram_parameter("b2", [HEADS], F32, isOutput=False)
    wout_p = nc.declare_dram_parameter("wout", [INNER_LOC, D], BF16,
                                       isOutput=False)
    gout_p = nc.declare_dram_parameter("g_out", [D], F32, isOutput=False)
    posb_p = nc.declare_dram_parameter("posb", [128], I32, isOutput=False)
    sel_p = nc.declare_dram_parameter("sel", [HEADS, H_LOC], F32, isOutput=False)
    out_p = nc.declare_dram_parameter("out", [N // 4, D], F32, isOutput=True)
    DBG = globals().get('DEBUG_TAPS', False)
    if DBG:
        dbg_qT = nc.declare_dram_parameter("dbg_qT", [128, N], BF16, isOutput=True)
        dbg_kT = nc.declare_dram_parameter("dbg_kT", [128, N], BF16, isOutput=True)
        dbg_v0 = nc.declare_dram_parameter("dbg_v0", [128, DH + 1], BF16, isOutput=True)
        dbg_avT = nc.declare_dram_parameter("dbg_avT", [DH, N], BF16, isOutput=True)
        dbg_po = nc.declare_dram_parameter("dbg_po", [128, D], BF16, isOutput=True)
        dbg_mt = nc.declare_dram_parameter("dbg_mt", [128, MT_W], BF16, isOutput=True)
        dbg_avps = nc.declare_dram_parameter("dbg_avps", [DH + 1, N], F32, isOutput=True)
        dbg_rb = nc.declare_dram_parameter("dbg_rb", [DH, N], BF16, isOutput=True)
        dbg_p4 = nc.declare_dram_parameter("dbg_p4", [128, 1024], BF16, isOutput=True)
        dbg_ps4 = nc.declare_dram_parameter("dbg_ps4", [128, 1024], F32, isOutput=True)

    # ---------------- internal DRAM ----------------
    hfr_loc = nc.dram_tensor("hfr_loc", [MLP_ROWS, HEADS], F32)
    hfr_g = nc.dram_tensor("hfr_g", [8 * MLP_ROWS, HEADS], F32)
    hfrd = nc.dram_tensor("hfrd", [H_LOC, HFRD_ROWS], BF16)
    po_q = [nc.dram_tensor(f"po_{i}", [N // 4, D], BF16) for i in range(4)]
    rs_q = [nc.dram_tensor(f"rs_{i}", [N // 16, D], BF16) for i in range(4)]

    dma = nc.sync.dma_start

    with tile.TileContext(nc) as tc:
        with (
            tc.tile_pool(name="const", bufs=1) as constp,
            tc.tile_pool(name="pers", bufs=1) as pers,
            tc.tile_pool(name="weights", bufs=1) as wp,
        ):
            ident = constp.tile([128, 128], F32)
            from concourse.masks import make_identity
            make_identity(nc, ident[:, :])
            identb = constp.tile([128, 128], BF16)
            nc.scalar.copy(out=identb[:, :], in_=ident[:, :])
            eps128 = constp.tile([128, 1], F32)
            nc.vector.memset(eps128[:, :], EPS)
            ones1 = constp.tile([1, 128], F32)
            nc.vector.memset(ones1[:, :], 1.0)

            # ---- all big input loads issued up front (single SP queue) ----
            early = tc.tile_pool(name="early", bufs=1)
            ep = early.__enter__()
            xall = ep.tile([128, 16 * D], BF16, name="xall")
            dma(out=xall[:, :], in_=_ap(x_p, [[D, 128], [128 * D, 16], [1, D]], 0))
            wqkv_sb = ep.tile([128, 8 * QKV], BF16)
            dma(out=wqkv_sb[:, :],
                in_=_ap(wqkv_p, [[QKV, 128], [128 * QKV, 8], [1, QKV]], 0))
            wout_sb = wp.tile([DH, 4 * D], BF16)
            dma(out=wout_sb[:, :],
                in_=_ap(wout_p, [[D, DH], [DH * D, 4], [1, D]], 0))
            w1_sb = ep.tile([128, 8 * D], BF16)  # chunk k at cols k*1024
            dma(out=w1_sb[:, :],
                in_=_ap(w1_p, [[D, 128], [128 * D, 8], [1, D]], 0))
            w2_sb = ep.tile([128, 8 * HEADS], BF16)
            dma(out=w2_sb[:, :],
                in_=_ap(w2_p, [[HEADS, 128], [128 * HEADS, 8], [1, HEADS]], 0))

            wsum_f32 = ep.tile([1, QKV], F32)
            dma(out=wsum_f32[:, :], in_=_ap(wsum_p, [[QKV, 1], [1, QKV]], 0))
            wsum_row = ep.tile([1, QKV], BF16)
            nc.vector.tensor_copy(out=wsum_row[:, :], in_=wsum_f32[:, :])
            qs8ks_sb = pers.tile([128, 1], F32)
            dma(out=qs8ks_sb[:, :], in_=_ap(qs8ks_p, [[1, 128], [1, 1]], 0))
            nkT = pers.tile([128, 1], F32)
            dma(out=nkT[:, :], in_=_ap(nkv_p, [[1, 128], [1, 1]], 0))
            nv_sb = pers.tile([1, DH], F32)
            dma(out=nv_sb[:, :], in_=nkv_p.ap()[1:2, 0:DH])
            nb_sb = pers.tile([1, H_LOC], F32)
            dma(out=nb_sb[:, :], in_=_ap(nb_p, [[H_LOC, 1], [1, H_LOC]], 0))
            posi_t = pers.tile([128, 1], I32, name="posi")
            dma(out=posi_t[:, :], in_=_ap(posb_p, [[1, 128], [1, 1]], 0))
            sel_sb = wp.tile([HEADS, H_LOC], F32)
            dma(out=sel_sb[:, :], in_=sel_p.ap())

            # [nv | 1] * exp(nb[h]) bf16 rows for null-key AV (nb_p holds
            # exp(null_attn_bias) so the bias add disappears into the AV)
            nv1f = pers.tile([1, DH + 1], F32)
            nc.vector.tensor_copy(out=nv1f[:, 0:DH], in_=nv_sb[:, :])
            nc.vector.memset(nv1f[:, DH:DH + 1], 1.0)
            nv1e = pers.tile([1, H_LOC * (DH + 1)], BF16, name="nv1e")
            for hh in range(H_LOC):
                nc.vector.tensor_scalar(
                    out=nv1e[:, hh * (DH + 1):(hh + 1) * (DH + 1)],
                    in0=nv1f[:, :], scalar1=nb_sb[:, hh:hh + 1], scalar2=None,
                    op0=mybir.AluOpType.mult)

            # ---------- Phases 0-2 interleaved: rel-pos MLP generator is ----
            # pumped between projection steps so its long serial chain fills
            # engine gaps instead of blocking the in-order queues.
            qT2 = pers.tile([128, 2 * N], BF16, name="qT2")
            mt = [pers.tile([128, MT_W], BF16, tag=f"mt{h}", name=f"mt{h}")
                  for h in range(H_LOC)]
            kT = pers.tile([128, N], BF16, tag="kT", name="kT")
            v1 = [pers.tile([128, DH + 1], BF16, tag=f"v1_{j}", name=f"v1_{j}")
                  for j in range(16)]
            nkn = pers.tile([128, 1], BF16)

            with (
                tc.tile_pool(name="bc_ps", bufs=2, space="PSUM") as bpp,
                tc.tile_pool(name="vst", bufs=3) as vstp,
            ):
                bcast = ep.tile([128, 4 * D + HEADS], BF16)
                g1b = ep.tile([128, D], BF16)
                goutb = wp.tile([128, D], F32)
                chunks = []
                for pi, par in enumerate((w0_p, b0_p, g0_p, b1_p)):
                    for half in range(2):
                        chunks.append((par, half * 512, 512, bcast,
                                       pi * D + half * 512))
                chunks.append((b2_p, 0, HEADS, bcast, 4 * D))
                for half in range(2):
                    chunks.append((g1_p, half * 512, 512, g1b, half * 512))
                    chunks.append((gout_p, half * 512, 512, goutb,
                                   half * 512))
                for par, poff, wdt, dst, doff in chunks:
                    vstage = vstp.tile([1, 512], F32, tag="vstage")
                    dma(out=vstage[:, 0:wdt],
                        in_=_ap(par, [[wdt, 1], [1, wdt]], poff))
                    ps = bpp.tile([128, 512], F32, tag="bc")
                    nc.tensor.matmul(out=ps[:, 0:wdt],
                                     lhsT=ones1[:, :],
                                     rhs=vstage[:, 0:wdt],
                                     start=True, stop=True)
                    nc.scalar.copy(out=dst[:, doff:doff + wdt],
                                   in_=ps[:, 0:wdt])
            w0b = bcast[:, 0:D]
            b0b = bcast[:, D:2 * D]
            g0b = bcast[:, 2 * D:3 * D]
            b1b = bcast[:, 3 * D:4 * D]
            b2b = bcast[:, 4 * D:4 * D + HEADS]

            with (
                tc.tile_pool(name="mlp", bufs=1) as mp,
                tc.tile_pool(name="mlp_ps", bufs=1, space="PSUM") as mpp,
                tc.tile_pool(name="mlp_ps2", bufs=1, space="PSUM") as mpp2,
                tc.tile_pool(name="xT", bufs=1) as xTp,
                tc.tile_pool(name="xt", bufs=2) as xtp,
                tc.tile_pool(name="xps", bufs=2, space="PSUM") as xpp,
                tc.tile_pool(name="xps2", bufs=1, space="PSUM") as xpp2,
                tc.tile_pool(name="xps3", bufs=1, space="PSUM") as xpp3,
            ):
                xT = xTp.tile([128, 8 * N], BF16)  # d-chunk k at cols k*2048
                negrow = wp.tile([1, N], BF16, name="negrow")

                def mlp_gen(t):
                    if True:
                        m0 = t * 128
                        posf = mp.tile([128, 1], F32, tag=f"posf{t}",
                                       name=f"posf{t}")
                        nc.vector.tensor_scalar_add(out=posf[:, :],
                                                    in0=posi_t[:, :],
                                                    scalar1=float(-m0))
                        h0 = mp.tile([128, D], F32, tag=f"h0{t}", name=f"h0_{t}")
                        nc.vector.tensor_scalar(out=h0[:, :], in0=w0b,
                                                scalar1=posf[:, :], scalar2=None,
                                                op0=mybir.AluOpType.mult)
                        nc.vector.tensor_tensor(out=h0[:, :], in0=h0[:, :],
                                                in1=b0b,
                                                op=mybir.AluOpType.add)
                        yield
                        mean, var, sd, rstd = _ln_stats(nc, mp, h0[:, :], D, sfx=str(t))
                        nc.scalar.activation(
                            out=sd[:, :], in_=var,
                            func=mybir.ActivationFunctionType.Sqrt,
                            bias=eps128[:, :])
                        nc.vector.reciprocal_approx_fast(out=rstd[:, :],
                                                         in_=sd[:, :])
                        yield
                        nc.vector.tensor_scalar(out=h0[:, :], in0=h0[:, :],
                                                scalar1=mean,
                                                scalar2=rstd[:, :],
                                                op0=mybir.AluOpType.subtract,
                                                op1=mybir.AluOpType.mult)
                        nc.vector.tensor_tensor(out=h0[:, :], in0=h0[:, :],
                                                in1=g0b,
                                                op=mybir.AluOpType.mult)
                        h0b = mp.tile([128, D], BF16, tag=f"h0b{t}",
                                      name=f"h0b{t}")
                        nc.scalar.activation(
                            out=h0b[:, :], in_=h0[:, :],
                            func=mybir.ActivationFunctionType.Silu)
                        yield
                        h0T = mp.tile([128, D], BF16, tag=f"h0T{t}",
                                      name=f"h0T{t}")
                        pst8m = mpp2.tile([128, D], BF16, tag="tp",
                                          name=f"tp0_{t}")
                        for k in range(8):
                            nc.tensor.matmul(
                                out=pst8m[:, k * 128:(k + 1) * 128],
                                lhsT=h0b[:, k * 128:(k + 1) * 128],
                                rhs=identb[:, :], is_transpose=True,
                                start=True, stop=True)
                            if k == 3:
                                yield
                        nc.vector.tensor_copy(out=h0T[:, :], in_=pst8m[:, :])
                        yield
                        h1 = mp.tile([128, D], F32, tag=f"h0{t}", name=f"h1_{t}")
                        for eb in range(2):
                            ps = mpp.tile([128, 512], F32, tag="h1ps",
                                          name=f"h1ps{t}_{eb}")
                            for k in range(8):
                                nc.tensor.matmul(
                                    out=ps[:, :],
                                    lhsT=h0T[:, k * 128:(k + 1) * 128],
                                    rhs=w1_sb[:, k * D + eb * 512:
                                              k * D + eb * 512 + 512],
                                    start=(k == 0), stop=(k == 7))
                            nc.vector.tensor_tensor(
                                out=h1[:, eb * 512:eb * 512 + 512],
                                in0=ps[:, :],
                                in1=b1b[:, eb * 512:eb * 512 + 512],
                                op=mybir.AluOpType.add)
                            yield
                        mean, var, sd, rstd = _ln_stats(nc, mp, h1[:, :], D, sfx=str(t))
                        nc.scalar.activation(
                            out=sd[:, :], in_=var,
                            func=mybir.ActivationFunctionType.Sqrt,
                            bias=eps128[:, :])
                        nc.vector.reciprocal_approx_fast(out=rstd[:, :],
                                                         in_=sd[:, :])
                        yield
                        nc.vector.tensor_scalar(out=h1[:, :], in0=h1[:, :],
                                                scalar1=mean,
                                                scalar2=rstd[:, :],
                                                op0=mybir.AluOpType.subtract,
                                                op1=mybir.AluOpType.mult)
                        nc.vector.tensor_tensor(out=h1[:, :], in0=h1[:, :],
                                                in1=g1b,
                                                op=mybir.AluOpType.mult)
                        h1b = mp.tile([128, D], BF16, tag=f"h0b{t}",
                                      name=f"h1b{t}")
                        nc.scalar.activation(
                            out=h1b[:, :], in_=h1[:, :],
                            func=mybir.ActivationFunctionType.Silu)
                        yield
                        h1T = mp.tile([128, D], BF16, tag=f"h0T{t}",
                                      name=f"h1T{t}")
                        pst8n = mpp2.tile([128, D], BF16, tag="tp",
                                          name=f"tp1_{t}")
                        for k in range(8):
                            nc.tensor.matmul(
                                out=pst8n[:, k * 128:(k + 1) * 128],
                                lhsT=h1b[:, k * 128:(k + 1) * 128],
                                rhs=identb[:, :], is_transpose=True,
                                start=True, stop=True)
                            if k == 3:
                                yield
                        nc.vector.tensor_copy(out=h1T[:, :], in_=pst8n[:, :])
                        yield
                        psf = mpp2.tile([128, HEADS], F32, tag="hf",
                                        name=f"hf{t}")
                        for k in range(8):
                            nc.tensor.matmul(
                                out=psf[:, :],
                                lhsT=h1T[:, k * 128:(k + 1) * 128],
                                rhs=w2_sb[:, k * HEADS:(k + 1) * HEADS],
                                start=(k == 0), stop=(k == 7))
                        hfc = mp.tile([128, HEADS], F32, tag=f"hfc{t}",
                                      name=f"hfc{t}")
                        nc.vector.tensor_tensor(out=hfc[:, :], in0=psf[:, :],
                                                in1=b2b,
                                                op=mybir.AluOpType.add)
                        dma(out=hfr_loc.ap()[m0:m0 + 128, :], in_=hfc[:, :])
                        yield

                mgens = [mlp_gen(0), mlp_gen(1)]
                mstate = {"done": 0, "ag": False}

                def pump(n=1):
                    for _ in range(n):
                        alive = False
                        for g in mgens:
                            try:
                                next(g)
                                alive = True
                            except StopIteration:
                                pass
                        if not alive and not mstate["ag"]:
                            mstate["ag"] = True
                            nc.gpsimd.collective_compute(
                                "AllGather", mybir.AluOpType.bypass,
                                replica_groups=[list(range(N_CORES))],
                                ins=[hfr_loc.ap().opt()],
                                outs=[hfr_g.ap().opt()],
                            )
                            return

                def do_staging():
                    stg = xtp.tile([128, 16 * HEADS], F32, tag="stg",
                                   name="stg_all")
                    dma(out=stg[:, :],
                        in_=_ap(hfr_g, [[HEADS, 128], [128 * HEADS, 16],
                                        [1, HEADS]], 0))
                    stgT = xTp.tile([HEADS, 16 * 128], BF16, name="stgT")
                    selb = xtp.tile([HEADS, H_LOC], BF16, tag="selb",
                                    name="selb")
                    nc.vector.tensor_copy(out=selb[:, :], in_=sel_sb[:, :])
                    for chunk in range(16):
                        pss = xpp.tile([128, QKV], F32, tag="qkv",
                                       name=f"stgps{chunk}")
                        nc.tensor.matmul(out=pss[0:HEADS, 0:128],
                                         lhsT=stg[:, chunk * HEADS:
                                                  (chunk + 1) * HEADS],
                                         rhs=ident[:, :], is_transpose=True,
                                         start=True, stop=True)
                        eng = nc.scalar if chunk % 2 == 0 else nc.vector
                        if chunk % 2 == 0:
                            nc.scalar.copy(
                                out=stgT[:, chunk * 128:(chunk + 1) * 128],
                                in_=pss[0:HEADS, 0:128])
                        else:
                            nc.vector.tensor_copy(
                                out=stgT[:, chunk * 128:(chunk + 1) * 128],
                                in_=pss[0:HEADS, 0:128])
                    # select local heads and write hfrd in 512-col pieces
                    for piece in range(4):
                        psl = xpp3.tile([128, 128], F32, tag="misc",
                                        name=f"psl{piece}")
                        stl = xtp.tile([H_LOC, 512], BF16, tag="stl")
                        for sub in range(4):
                            col = piece * 512 + sub * 128
                            nc.tensor.matmul(out=psl[0:H_LOC, 0:128],
                                             lhsT=selb[:, :],
                                             rhs=stgT[:, col:col + 128],
                                             start=True, stop=True)
                            if sub % 2 == 0:
                                nc.scalar.copy(out=stl[:, sub * 128:sub * 128 + 128],
                                               in_=psl[0:H_LOC, 0:128])
                            else:
                                nc.vector.tensor_copy(
                                    out=stl[:, sub * 128:sub * 128 + 128],
                                    in_=psl[0:H_LOC, 0:128])
                        dma(out=_ap(hfrd, [[HFRD_ROWS, H_LOC], [1, 512]],
                                    HFR_PAD + piece * 512),
                            in_=stl[:, :])
                    poison = xtp.tile([H_LOC, HFRD_ROWS - 2112], BF16,
                                      name="poison")
                    nc.vector.memset(poison[:, :], NEG)
                    dma(out=_ap(hfrd, [[HFRD_ROWS, H_LOC],
                                       [1, HFRD_ROWS - 2112]], 2112),
                        in_=poison[:, :])


                for tt in range(16):
                    xs = xall[:, tt * D:(tt + 1) * D]
                    # stats (one-pass bn_stats; wsum carries -colsum so the
                    # rank-1 correction uses the mean row directly)
                    mean, var, sd, rstd = _ln_stats(nc, xtp, xs, D)
                    nc.scalar.activation(out=sd[:, :], in_=var,
                                         func=mybir.ActivationFunctionType.Sqrt,
                                         bias=eps128[:, :])
                    nc.vector.reciprocal_approx_fast(out=rstd[:, :],
                                                     in_=sd[:, :])
                    psr = xpp3.tile([128, 128], F32, tag="misc",
                                    name=f"psr{tt}")
                    nc.tensor.matmul(out=psr[0:1, :], lhsT=mean,
                                     rhs=ident[:, :], is_transpose=True,
                                     start=True, stop=True)
                    nc.vector.tensor_copy(out=negrow[:, tt * 128:tt * 128 + 128],
                                          in_=psr[0:1, :])
                    pump(2 if tt < 8 else 1)
                    # x transposes (bf16): 8 into one psum tile, one fat copy
                    pst8 = xpp2.tile([128, D], BF16, tag="tp8",
                                     name=f"pst8_{tt}")
                    for k in range(8):
                        nc.tensor.matmul(out=pst8[:, k * 128:(k + 1) * 128],
                                         lhsT=xall[:, tt * D + k * 128:
                                                   tt * D + k * 128 + 128],
                                         rhs=identb[:, :], is_transpose=True,
                                         start=True, stop=True)
                    xTo = xT[:, :].copy()
                    xpat = [list(p) for p in xTo.ap.to_list()]
                    xpat = [xpat[0], [N, 8], [1, 128]]
                    xTo.ap = bass_rust.VecI64Pair(xpat)
                    xTo.offset = xTo.offset + tt * 128
                    nc.vector.tensor_copy(out=xTo, in_=pst8[:, :])
                    pump(2 if tt < 8 else 1)
                    # packed q|k|v projection with rank-1 mean correction
                    psq = xpp.tile([128, QKV], F32, tag="qkv")
                    for k in range(8):
                        nc.tensor.matmul(
                            out=psq[:, :],
                            lhsT=xT[:, k * N + tt * 128:k * N + tt * 128 + 128],
                            rhs=wqkv_sb[:, k * QKV:(k + 1) * QKV],
                            start=(k == 0), stop=False,
                            skip_group_check=True)
                    nc.tensor.matmul(out=psq[:, :],
                                     lhsT=negrow[:, tt * 128:tt * 128 + 128],
                                     rhs=wsum_row[:, :],
                                     start=False, stop=True,
                                     skip_group_check=True)
                    # per-head l2 norms (4 q heads + k)
                    nrm = xtp.tile([128, 8], F32, tag="nrm")
                    scr2 = xtp.tile([128, DH], BF16, tag="scr2")
                    for j in range(5):
                        nc.scalar.activation(
                            out=scr2[:, :],
                            in_=psq[:, j * DH:(j + 1) * DH],
                            func=mybir.ActivationFunctionType.Square,
                            accum_out=nrm[:, j:j + 1])
                    sd5 = xtp.tile([128, 8], F32, tag="sd5")
                    rinv = xtp.tile([128, 8], F32, tag="rinv")
                    nc.scalar.activation(out=sd5[:, 0:5], in_=nrm[:, 0:5],
                                         func=mybir.ActivationFunctionType.Sqrt)
                    nc.vector.reciprocal(out=rinv[:, 0:5], in_=sd5[:, 0:5])
                    pump(1)
                    # scaled copies out of PSUM
                    qn = xtp.tile([128, INNER_LOC], BF16, tag="qn")
                    for h in range(4):
                        if h % 2 == 0 or tt >= 8:
                            nc.vector.tensor_scalar(
                                out=qn[:, h * DH:(h + 1) * DH],
                                in0=psq[:, h * DH:(h + 1) * DH],
                                scalar1=rinv[:, h:h + 1], scalar2=None,
                                op0=mybir.AluOpType.mult)
                        else:
                            nc.scalar.activation(
                                out=qn[:, h * DH:(h + 1) * DH],
                                in_=psq[:, h * DH:(h + 1) * DH],
                                func=mybir.ActivationFunctionType.Copy,
                                scale=rinv[:, h:h + 1])
                    # kn duplicated into both column halves so the transpose
                    # yields kT stacked twice along partitions
                    kn = xtp.tile([128, 128], BF16, tag="kn")
                    for kh in range(2):
                        nc.vector.tensor_scalar(
                            out=kn[:, kh * DH:(kh + 1) * DH],
                            in0=psq[:, INNER_LOC:INNER_LOC + DH],
                            scalar1=rinv[:, 4:5], scalar2=None,
                            op0=mybir.AluOpType.mult)
                    nc.vector.tensor_scalar(out=v1[tt][:, 0:DH],
                                            in0=psq[:, INNER_LOC + DH:QKV],
                                            scalar1=rstd[:, :], scalar2=None,
                                            op0=mybir.AluOpType.mult)
                    nc.vector.memset(v1[tt][:, DH:DH + 1], 1.0)
                    # q pair + k transposes into one psum tile
                    pstqk = xpp2.tile([128, 384], BF16, tag="tpqk",
                                      name=f"pstqk{tt}")
                    for p in range(2):
                        nc.tensor.matmul(out=pstqk[:, p * 128:(p + 1) * 128],
                                         lhsT=qn[:, p * 128:(p + 1) * 128],
                                         rhs=identb[:, :], is_transpose=True,
                                         start=True, stop=True)
                    nc.tensor.matmul(out=pstqk[:, 256:384], lhsT=kn[:, :],
                                     rhs=identb[:, :], is_transpose=True,
                                     start=True, stop=True)
                    qTo = qT2[:, :].copy()
                    qpat = [list(p) for p in qTo.ap.to_list()]
                    qpat = [qpat[0], [N, 2], [1, 128]]
                    qTo.ap = bass_rust.VecI64Pair(qpat)
                    qTo.offset = qTo.offset + tt * 128
                    if tt >= 8:
                        nc.vector.tensor_copy(out=qTo, in_=pstqk[:, 0:256])
                    else:
                        nc.scalar.copy(out=qTo, in_=pstqk[:, 0:256])
                    # k transpose with qs8ks scale folded in
                    nc.vector.tensor_scalar(out=kT[:, tt * 128:tt * 128 + 128],
                                            in0=pstqk[:, 256:384],
                                            scalar1=qs8ks_sb[:, :], scalar2=None,
                                            op0=mybir.AluOpType.mult)
                    pump(1)
                    if tt == 13:
                        pump(30)
                        do_staging()
                        for hh in range(H_LOC):
                            dma(out=mt[hh][:, :],
                                in_=_ap(hfrd, [[1, 128], [1, MT_W]],
                                        hh * HFRD_ROWS + 63))

                pump(100)

                if DBG:
                    dma(out=dbg_qT.ap(), in_=qT2[:, 0:N])
                    dma(out=dbg_kT.ap(), in_=kT[:, :])
                    dma(out=dbg_v0.ap(), in_=v1[0][:, :])

                # null key normalize: nkn = l2norm(nk) * qs8ks  (dup over 128)
                ones64c_f = constp.tile([DH, 1], F32)
                nc.vector.memset(ones64c_f[:, :], 1.0)
                nsq = xtp.tile([128, 1], F32, tag="nsq")
                nc.scalar.activation(out=nsq[:, :], in_=nkT[:, :],
                                     func=mybir.ActivationFunctionType.Square)
                psn1 = xpp3.tile([128, 128], F32, tag="misc", name="psn1")
                nc.tensor.matmul(out=psn1[0:1, 0:1], lhsT=ones64c_f[:, :],
                                 rhs=nsq[0:DH, :], start=True, stop=True)
                rn1 = xtp.tile([1, 1], F32, tag="rn1")
                nc.scalar.activation(out=rn1[:, :], in_=psn1[0:1, 0:1],
                                     func=mybir.ActivationFunctionType.Sqrt)
                with nc.allow_low_precision(reason="f32r same bits as f32"):
                    nc.vector.reciprocal(out=rn1[:, :], in_=rn1[:, :])
                psb1 = xpp3.tile([128, 128], F32, tag="misc", name="psb1")
                nc.tensor.matmul(out=psb1[:, 0:1], lhsT=ones1[:, :],
                                 rhs=rn1[:, :], start=True, stop=True)
                nc.vector.tensor_tensor(out=nkn[:, :], in0=nkT[:, :],
                                        in1=psb1[:, 0:1],
                                        op=mybir.AluOpType.mult)
                nc.vector.tensor_scalar(out=nkn[:, :], in0=nkn[:, :],
                                        scalar1=qs8ks_sb[:, :], scalar2=None,
                                        op0=mybir.AluOpType.mult)

            early.__exit__(None, None, None)

            # ---------- Phase 3: attention + per-quarter out-proj + RS ------
            avT = [pers.tile([DH, N], BF16, tag=f"avT{h}", name=f"avT{h}")
                   for h in range(H_LOC)]
            if build.NO_AV:
                for h in range(H_LOC):
                    nc.vector.memset(avT[h][:, :], 0.0)

            with (
                tc.tile_pool(name="at", bufs=3) as atp,
                tc.tile_pool(name="sim4", bufs=2, space="PSUM") as simpp,
                tc.tile_pool(name="avps", bufs=2, space="PSUM") as avpp,
                tc.tile_pool(name="tps", bufs=2, space="PSUM") as tpp,
                tc.tile_pool(name="oq", bufs=2) as oqp,
            ):
                ones65 = atp.tile([DH + 1, DH], F32, tag="ones65",
                                  name="ones65")
                nc.vector.memset(ones65[:, :], 1.0)
                pend_tail = [None]

                def run_tail():
                    if pend_tail[0] is not None:
                        pend_tail[0]()
                        pend_tail[0] = None

                for q in range(4):
                    for m in (2 * q, 2 * q + 1):
                        i0 = m * 256
                        njt = 2 * m + 2
                        for h in range(H_LOC):
                            hp = (h % 2) * DH
                            qh = qT2[hp:hp + DH,
                                     (h // 2) * N + i0:(h // 2) * N + i0 + 256]
                            av_ps = avpp.tile([DH + 1, 256], F32, tag="av",
                                              name=f"av_{m}_{h}")
                            GSZ = 4
                            groups = [list(range(g, min(g + GSZ, njt)))
                                      for g in range(0, njt, GSZ)]
                            if len(groups[-1]) == GSZ:
                                # keep a spare exp column chunk for the
                                # null-key logits in the final group
                                groups[-1] = groups[-1][:GSZ - 1]
                                groups.append([njt - 1])
                            pend_av = None
                            av_state = [False]

                            def issue_av(pend, av_ps=av_ps, av_state=av_state):
                                pp4, pjts = pend
                                for ji, jt in enumerate(pjts):
                                    nc.tensor.matmul(
                                        out=av_ps[:, :],
                                        lhsT=v1[jt][:, :],
                                        rhs=pp4[:, ji * 256:ji * 256 + 256],
                                        start=(not av_state[0]), stop=False,
                                        skip_group_check=True)
                                    av_state[0] = True

                            for gi, jts in enumerate(groups):
                                gw = 256 * len(jts)
                                last = (gi == len(groups) - 1)
                                ps4 = simpp.tile([128, 1024], F32, tag="sim")
                                for ji, jt in enumerate(jts):
                                    j0 = jt * 128
                                    c0 = ji * 256
                                    # start=True only on the first chunk of
                                    # each 2KB psum bank: a start arms
                                    # zero-on-first-write for the whole bank
                                    nc.tensor.matmul(
                                        out=ps4[:, c0:c0 + 256],
                                        lhsT=kT[hp:hp + DH, j0:j0 + 128],
                                        rhs=qh,
                                        start=(c0 % 512 == 0), stop=False,
                                        skip_group_check=True)
                                # Toeplitz bias adds: two j-tiles merged per
                                # matmul via a 3D shifted AP (second touch of
                                # the armed bank, so plain accumulate)
                                for c0 in range(0, gw, 512):
                                    cn = min(2, (gw - c0) // 256)
                                    jt0 = jts[c0 // 256]
                                    u0 = 2048 - i0 + jt0 * 128
                                    mtr = mt[h][:, :].copy()
                                    pat = [list(p) for p in mtr.ap.to_list()]
                                    pat[1] = [128, cn]
                                    pat.append([-1, 256])
                                    mtr.ap = bass_rust.VecI64Pair(pat)
                                    mtr.offset = mtr.offset + u0
                                    nc.tensor.matmul(
                                        out=ps4[:, c0:c0 + cn * 256],
                                        lhsT=identb[:, :], rhs=mtr,
                                        start=False, stop=True,
                                        skip_group_check=True)
                                ew = gw
                                if last:
                                    # null-key logits ride along in the spare
                                    # columns of the final (partial) group
                                    nc.tensor.matmul(
                                        out=ps4[0:1, gw:gw + 256],
                                        lhsT=nkn[hp:hp + DH, :], rhs=qh,
                                        start=True, stop=True,
                                        skip_group_check=True)
                                    ew = gw + 256
                                p4 = atp.tile([128, 1024], BF16, tag="p4")
                                nc.scalar.activation(
                                    out=p4[:, 0:ew], in_=ps4[:, 0:ew],
                                    func=mybir.ActivationFunctionType.Exp)
                                # software pipeline: issue deferred work now so
                                # the PE queue never parks waiting on this exp
                                if gi == 0:
                                    run_tail()
                                else:
                                    issue_av(pend_av)
                                pend_av = (p4, jts)

                            def tail(h=h, i0=i0, av_ps=av_ps, pend_av=pend_av,
                                     issue_av=issue_av,
                                     gw_last=256 * len(groups[-1])):
                                issue_av(pend_av)
                                nc.tensor.matmul(
                                    out=av_ps[:, :],
                                    lhsT=nv1e[:, h * (DH + 1):
                                              (h + 1) * (DH + 1)],
                                    rhs=pend_av[0][0:1, gw_last:gw_last + 256],
                                    start=False, stop=True,
                                    skip_group_check=True)
                                # normalize columns by row-64 sums -> avT[h].
                                # full-height recip: base-partition-64 DVE
                                # slices silently no-op; only row 64 is read
                                # by the selector matmul below
                                rr = atp.tile([DH + 1, 256], F32, tag="rr")
                                nc.vector.reciprocal_approx_fast(
                                    out=rr[:, :], in_=av_ps[:, :])
                                psb = tpp.tile([DH, 256], F32, tag="bc")
                                nc.tensor.matmul(out=psb[:, :],
                                                 lhsT=ones65[DH:DH + 1, 0:DH],
                                                 rhs=rr[DH:DH + 1, :],
                                                 start=True, stop=True)
                                rb = atp.tile([DH, 256], BF16, tag="rb")
                                nc.scalar.copy(out=rb[:, :], in_=psb[:, :])
                                nc.vector.tensor_tensor(
                                    out=avT[h][:, i0:i0 + 256],
                                    in0=av_ps[0:DH, :], in1=rb[:, :],
                                    op=mybir.AluOpType.mult)
                            pend_tail[0] = tail

                    # out projection for this quarter
                    run_tail()
                    for tl in range(4):
                        tt = q * 4 + tl
                        ps_po = simpp.tile([128, 1024], F32, tag="sim")
                        for eb in range(2):
                            for ch in range(H_LOC):
                                nc.tensor.matmul(
                                    out=ps_po[:, eb * 512:eb * 512 + 512],
                                    lhsT=avT[ch][:, tt * 128:tt * 128 + 128],
                                    rhs=wout_sb[:, ch * D + eb * 512:
                                                ch * D + eb * 512 + 512],
                                    start=(ch == 0), stop=(ch == H_LOC - 1),
                                    skip_group_check=True)
                        po_sb = oqp.tile([128, D], BF16, tag="po")
                        nc.vector.tensor_copy(out=po_sb[:, :], in_=ps_po[:, :])
                        dma(out=po_q[q].ap()[tl * 128:(tl + 1) * 128, :],
                            in_=po_sb[:, :])
                    nc.gpsimd.collective_compute(
                        "ReduceScatter", mybir.AluOpType.add,
                        replica_groups=[[0, 1, 2, 3], [4, 5, 6, 7]],
                        ins=[po_q[q].ap().opt()],
                        outs=[rs_q[q].ap().opt()],
                    )
                    # final LN for the 128 owned rows of this quarter
                    y = oqp.tile([128, D], BF16, tag="y")
                    dma(out=y[:, :], in_=rs_q[q].ap()[:, :])
                    mean, var, sd, rstd = _ln_stats(nc, oqp, y[:, :], D)
                    nc.scalar.activation(out=sd[:, :], in_=var,
                                         func=mybir.ActivationFunctionType.Sqrt,
                                         bias=eps128[:, :])
                    nc.vector.reciprocal_approx_fast(out=rstd[:, :], in_=sd[:, :])
                    yf = oqp.tile([128, D], F32, tag="yf")
                    nc.vector.tensor_scalar(out=yf[:, :], in0=y[:, :],
                                            scalar1=mean,
                                            scalar2=rstd[:, :],
                                            op0=mybir.AluOpType.subtract,
                                            op1=mybir.AluOpType.mult)
                    nc.vector.tensor_tensor(out=yf[:, :], in0=yf[:, :],
                                            in1=goutb,
                                            op=mybir.AluOpType.mult)
                    dma(out=out_p.ap()[q * 128:(q + 1) * 128, :], in_=yf[:, :])
                if DBG:
                    dma(out=dbg_avT.ap(), in_=avT[0][:, :])
                    dma(out=dbg_mt.ap(), in_=mt[0][:, :])
                    dma(out=dbg_po.ap(), in_=po_q[0].ap()[0:128, :])

    nc.compile()
    return nc


def make_in_maps(x, mask, g_norm, Wq, Wkv, q_scale, k_scale, null_kv,
                 null_attn_bias, w0, b0, g0, w1, b1, g1, w2, b2, Wout, g_out):
    import ml_dtypes
    assert bool(np.asarray(mask).all()), "kernel assumes all-True mask"
    f = np.float32
    bf = ml_dtypes.bfloat16
    gn = np.asarray(g_norm, f)
    in_maps = []
    for c in range(N_CORES):
        bi, hg = c // 4, c % 4
        posb = (2047 - c * MLP_ROWS - np.arange(128)).astype(np.int32)
        wq_hg = np.asarray(Wq, f)[:, hg * INNER_LOC:(hg + 1) * INNER_LOC]
        wqkv = np.concatenate(
            [wq_hg, np.asarray(Wkv, f)], axis=1) * gn[:, None]
        wsum = -(wqkv.sum(axis=0))
        m = {
            "x": np.ascontiguousarray(np.asarray(x, f)[bi]).astype(bf),
            "wqkv": np.ascontiguousarray(wqkv).astype(bf),
            "wsum": np.ascontiguousarray(wsum.astype(f)),
            "qs8ks": np.ascontiguousarray(np.tile(
                8.0 * np.asarray(q_scale, f) * np.asarray(k_scale, f), 2)),
            "nkv": np.ascontiguousarray(np.tile(np.asarray(null_kv, f),
                                                (1, 2))),
            "nb": np.ascontiguousarray(np.exp(
                np.asarray(null_attn_bias, f)[hg * H_LOC:(hg + 1) * H_LOC])),
            "w0v": np.ascontiguousarray(np.asarray(w0, f).reshape(D)),
            "b0": np.ascontiguousarray(np.asarray(b0, f)),
            "g0": np.ascontiguousarray(np.asarray(g0, f)),
            "w1": np.ascontiguousarray(np.asarray(w1, f)).astype(bf),
            "b1": np.ascontiguousarray(np.asarray(b1, f)),
            "g1": np.ascontiguousarray(np.asarray(g1, f)),
            "w2": np.ascontiguousarray(np.asarray(w2, f)).astype(bf),
            "b2": np.ascontiguousarray(np.asarray(b2, f)),
            "wout": np.ascontiguousarray(
                np.asarray(Wout, f)[hg * INNER_LOC:(hg + 1) * INNER_LOC, :]
            ).astype(bf),
            "g_out": np.ascontiguousarray(np.asarray(g_out, f)),
            "posb": posb,
            "sel": np.eye(HEADS, dtype=f)[:, hg * H_LOC:(hg + 1) * H_LOC].copy(),
        }
        in_maps.append(m)
    return in_maps


_NC_CACHE = None


def kernel(**inputs):
    global _NC_CACHE
    from concourse.bass_utils import run_bass_kernel_spmd

    if _NC_CACHE is None:
        _NC_CACHE = build()
    nc = _NC_CACHE
    in_maps = make_in_maps(**inputs)
    res = run_bass_kernel_spmd(nc, in_maps, core_ids=list(range(N_CORES)))
    outs = res.results
    kernel.last_outs = outs
    full = np.empty((2, N, D), np.float32)
    for c in range(N_CORES):
        bi, hg = c // 4, c % 4
        o = np.asarray(outs[c]["out"], np.float32)
        for q in range(4):
            full[bi, q * 512 + hg * 128:q * 512 + (hg + 1) * 128, :] = \
                o[q * 128:(q + 1) * 128]
    return full
